# revision 1
# baseline (speedup 1.0000x reference)
"""Trainium2 Bass kernel for nn_PlainDecoder (2-layer 2-direction GRU decoder
+ vocab projection + log_softmax).

Sharding:
  - GRU scan: data-parallel over batch (32 batches -> 4 per core). Each core
    runs both "directions" of both layers for its 4 batches.
  - Logits/log_softmax: ROW-parallel. Each core computes the full 32000-wide
    logits + log_softmax for its own 512 (t, b) rows, streaming the full
    fc_w from DRAM (fp8-e4m3). No collectives anywhere in the kernel.

Scan layout (gate-major / weight-stationary): the recurrent gate matmuls put
the GATE dim on PSUM partitions: lhsT = Whh^T chunk [128 (h-sub), 128 (gate
sub)], rhs = h^T [128 (h-sub), 4 (batch)]. The output h'^T lands directly in
the [h-sub partitions, batch] layout the next step's rhs needs -> zero
transposes. Precomputed gi (+ fused biases) and b_hh_n ride into PSUM via
identity-matmul injections. The gates are split r|n|z across PSUM banks so
the sigmoid(r) chain head fires after ~1/3 of the step's matmuls. The two
layer scans are software-pipelined (layer 1 lags layer 0 by LAG slots) with
both layers' gi built chunk-by-chunk on spare PE cycles inside the slot loop.

Everything wide is float16 (state, weights, gi, output): fp16 matmuls run at
1 cycle/row on the PE at any p-state, and fp16's 10-bit mantissa keeps the
recurrent rounding walk ~8x below bf16's.
"""

import os
import sys
from contextlib import ExitStack

for _p in ("/opt/trn_rl_repo", "/root/.axon_site/_ro/trn_rl_repo"):
    if os.path.isdir(_p) and _p not in sys.path:
        sys.path.insert(0, _p)

import numpy as np  # noqa: E402

V, E, H, L, B, S = 32000, 512, 512, 2, 32, 128
NC_ = 8                      # cores
BPC = B // NC_               # batches per core = 4
R = BPC * S                  # rows per core = 512 (row = 4*t + b)
G = 3 * H                    # 1536 gates per direction
CH = 512                     # vocab chunk width
NCH = 63                     # chunks -> 32256 padded vocab
VP = NCH * CH
NEG = -80.0                  # pad bias -> exp() ~ 0
GCH = 8                      # gi chunk = 8 timesteps
GC_NR = GCH * BPC            # rows per gi chunk

_BUILT = {}


def _build_nc(T=S, n_cores=NC_, sim=False, nblk_lim=None, skip_gi=False):
    """Build the Bass program (same NEFF for all cores; per-core data only).

    sim is accepted for compatibility; the program has no collectives so the
    TimelineSim build is identical.
    """
    import concourse.bass as bass  # noqa: F401
    import concourse.mybir as mybir
    import concourse.tile as tile
    from concourse import bacc
    from concourse.masks import make_identity

    dt = mybir.dt
    f32 = dt.float32
    f16 = dt.float16
    AF = mybir.ActivationFunctionType
    OP = mybir.AluOpType
    AX = mybir.AxisListType

    nc = bacc.Bacc("TRN2", target_bir_lowering=False, debug=False,
                   num_devices=n_cores)

    TR = BPC * T               # rows actually scanned

    # ---------------- DRAM I/O ----------------
    embT = nc.dram_tensor("embT", [128, 4, R], f16, kind="ExternalInput")
    h0T = nc.dram_tensor("h0T", [128, 2, 2, 4, BPC], f16,
                         kind="ExternalInput")          # (p, l, d, k, b)
    WhhT0 = nc.dram_tensor("WhhT0", [128, 4, 2, G], f16, kind="ExternalInput")
    WhhT1 = nc.dram_tensor("WhhT1", [128, 4, 2, G], f16, kind="ExternalInput")
    WihT0 = nc.dram_tensor("WihT0", [128, 4, 2, G], f16, kind="ExternalInput")
    WihT1 = nc.dram_tensor("WihT1", [128, 8, 2, G], f16, kind="ExternalInput")
    # b_ih (+b_hh for r/z) broadcast over a gi-chunk's (t, b) columns:
    # rides each gi chunk's PSUM as one identity-matmul injection
    bGiB = nc.dram_tensor("bGiB", [128, 2, 2, 12, GC_NR], f16,
                          kind="ExternalInput")
    # b_hh n-part, broadcast over (t, b): injected into the n-gate PSUM.
    bHhnB0 = nc.dram_tensor("bHhnB0", [128, 2, 4, S, BPC], f16,
                            kind="ExternalInput")
    bHhnB1 = nc.dram_tensor("bHhnB1", [128, 2, 4, S, BPC], f16,
                            kind="ExternalInput")
    f8 = dt.float8e4
    # fc_w streams in fp8-e4m3 (halves the dominant DMA stream; the
    # weight-quantization noise is ~0.06 max on a ~10.4-magnitude output,
    # well inside the rel-err budget). x2 (the lhsT) stays fp16.
    fcwT = nc.dram_tensor("fcwT", [NCH, 128, 8, CH], f8,
                          kind="ExternalInput")
    # fc_b pre-broadcast across partitions: the bias rides a DVE add
    # (copy-from-PSUM fused) instead of a 512-cycle PE matmul per chunk
    fcbB = nc.dram_tensor("fcbB", [128, NCH, CH], f16, kind="ExternalInput")

    out_d = nc.dram_tensor("out", [R, V], f16, kind="ExternalOutput")

    with tile.TileContext(nc) as tc, ExitStack() as top:
        constp = top.enter_context(tc.tile_pool(name="const", bufs=1))
        ident = constp.tile([128, 128], f16)
        make_identity(nc, ident[:])
        h0T_sb = constp.tile([128, 2, 2, 4, BPC], f16)
        nc.sync.dma_start(h0T_sb[:], h0T[:])

        histp = top.enter_context(tc.tile_pool(name="hist", bufs=1))
        x2T = histp.tile([128, 2, 4, T, BPC], f16, tag="x2T")
        fwp = top.enter_context(tc.tile_pool(name="fwp", bufs=3))
        fbp = top.enter_context(tc.tile_pool(name="fbp", bufs=3))
        fw_pre = {}

        def load_fwb(c):
            fw = fwp.tile([128, 8, CH], f8, tag="fw", name=f"fw{c % 3}")
            nc.sync.dma_start(fw[:], fcwT[c])
            fb = fbp.tile([128, CH], f16, tag="fb", name=f"fb{c % 3}")
            nc.sync.dma_start(fb[:], fcbB[:, c, :])
            return fw, fb

        with ExitStack() as scan_stack:
            wres = scan_stack.enter_context(tc.tile_pool(name="wres", bufs=1))
            gip = scan_stack.enter_context(tc.tile_pool(name="gip", bufs=1))
            hist1 = scan_stack.enter_context(tc.tile_pool(name="hist1",
                                                          bufs=1))

            # One SHARED gi buffer for both layers: L0 consumes slot t at
            # slot t, and gi1's chunk for steps [t0, t0+GCH) is only
            # written after L0's reads of those slots — so layer 1's gi
            # overwrites layer 0's in place. Both gi builds run chunked
            # inside the slot loop on spare PE cycles.
            giRZB = gip.tile([128, 2, 12, T, BPC], f16, tag="giRZB")
            giN = gip.tile([128, 2, 4, T, BPC], f16, tag="giN")
            giR = [giRZB, giRZB]
            giNl = [giN, giN]
            x1T = hist1.tile([128, 2, 4, T, BPC], f16, tag="x1T")
            hists = [x1T, x2T]

            # input DMAs ordered by first use: gi chunks 0/1 (emb, wih0,
            # bias), L0 step 0 (b_hh0, whh0), then gi1/L1 weights
            embT_sb = wres.tile([128, 4, R], f16, tag="embT")
            nc.sync.dma_start(embT_sb[:], embT[:])
            bgiB = wres.tile([128, 2, 2, 12, GC_NR], f16, tag="bgiB")
            nc.sync.dma_start(bgiB[:], bGiB[:])
            wih0 = wres.tile([128, 4, 2, G], f16, tag="wih0")
            for d in range(2):
                nc.sync.dma_start(wih0[:, :, d, :], WihT0[:, :, d, :])
            whh0 = wres.tile([128, 4, 2, G], f16, tag="whh0")
            nc.sync.dma_start(whh0[:], WhhT0[:])
            # b_hh0 n-part upfront; b_hh1's rides each gi1 chunk
            nc.sync.dma_start(giRZB[:, :, 8:12, :, :],
                              bHhnB0[:, :, :, 0:T, :])
            wih1 = wres.tile([128, 8, 2, G], f16, tag="wih1")
            nc.sync.dma_start(wih1[:], WihT1[:])
            whh1 = wres.tile([128, 4, 2, G], f16, tag="whh1")
            nc.sync.dma_start(whh1[:], WhhT1[:])
            whhs = [whh0, whh1]

            # ---------- pipelined two-layer scan ----------
            LAG = 10
            with (
                tc.tile_pool(name="spr", bufs=2, space="PSUM") as przp,
                tc.tile_pool(name="spn", bufs=2, space="PSUM") as pnp,
                tc.tile_pool(name="spz", bufs=2, space="PSUM") as pzp,
                tc.tile_pool(name="gcp", bufs=2, space="PSUM") as gcp,
                tc.tile_pool(name="sch", bufs=8) as chp,
            ):
                def emit_step(l, t):
                    whh = whhs[l]
                    histT = hists[l]
                    gR = giR[l]
                    gN = giNl[l]

                    def rhs(d, k):
                        if t == 0:
                            return h0T_sb[:, l, d, k, :]
                        return histT[:, d, k, t - 1, :]

                    def gate_group(pool, jlo, jhi, nm):
                        ps = pool.tile([128, 2, jhi - jlo, BPC], f32,
                                       tag=nm, name=f"{nm}{l}")
                        for d in range(2):
                            nc.tensor.matmul(
                                ps[:, d, :, :], ident[:],
                                gR[:, d, jlo:jhi, t, :],
                                start=(d == 0), stop=False)
                            for j in range(jlo, jhi):
                                for k in range(4):
                                    nc.tensor.matmul(
                                        ps[:, d, j - jlo, :],
                                        whh[:, k, d, 128 * j:128 * (j + 1)],
                                        rhs(d, k), start=False,
                                        stop=(d == 1 and j == jhi - 1
                                              and k == 3))
                        return ps
                    # r first (heads the chain), then n (needed next), z last
                    pr = gate_group(przp, 0, 4, "pr")
                    pn = gate_group(pnp, 8, 12, "pn")
                    pz = gate_group(pzp, 4, 8, "pz")
                    rp = chp.tile([128, 2, 3, 4, BPC], f32, tag="rp",
                                  name=f"rp{l}")
                    n1 = chp.tile([128, 2, 4, BPC], f32, tag="n1",
                                  name=f"n1{l}")
                    oz = chp.tile([128, 2, 4, BPC], f32, tag="oz",
                                  name=f"oz{l}")
                    m1 = chp.tile([128, 2, 4, BPC], f32, tag="m1",
                                  name=f"m1{l}")
                    m2 = chp.tile([128, 2, 4, BPC], f32, tag="m2",
                                  name=f"m2{l}")
                    hprev = (h0T_sb[:, l] if t == 0
                             else histT[:, :, :, t - 1, :])
                    # r|z = sigmoid(gh + gi); n = tanh(gi_n + r*(gh_n+b))
                    nc.scalar.activation(rp[:, :, 0], pr[:], AF.Sigmoid)
                    nc.vector.tensor_mul(n1[:], pn[:], rp[:, :, 0])
                    nc.vector.tensor_add(n1[:], n1[:], gN[:, :, :, t, :])
                    nc.scalar.activation(rp[:, :, 1], pz[:], AF.Sigmoid)
                    nc.scalar.activation(rp[:, :, 2], n1[:], AF.Tanh)
                    # h' = z*h + (1-z)*n  (oz on gpsimd, off the chain)
                    nc.vector.tensor_scalar(oz[:], rp[:, :, 1], -1.0, 1.0,
                                            OP.mult, OP.add)
                    nc.vector.tensor_mul(m1[:], rp[:, :, 1], hprev)
                    nc.vector.tensor_mul(m2[:], oz[:], rp[:, :, 2])
                    nc.vector.tensor_add(histT[:, :, :, t, :], m1[:], m2[:])

                def gi_chunk(l, c):
                    t0, t1 = GCH * c, GCH * (c + 1)
                    nr = GC_NR
                    if l == 1:
                        # layer 1's b_hh_n replaces layer 0's in the shared
                        # injection slots (L0 consumed them slots ago)
                        for d in range(2):
                            nc.sync.dma_start(giRZB[:, d, 8:12, t0:t1, :],
                                              bHhnB1[:, d, :, t0:t1, :])
                    wih, kc = (wih0, 4) if l == 0 else (wih1, 8)
                    for d in range(2):
                        for grp, (j0, nj) in enumerate([(0, 8), (8, 4)]):
                            ps = gcp.tile([128, nj, nr], f32, tag="gc",
                                          name=f"gc{grp}")
                            # bias seeds the whole group (zeroes the bank)
                            nc.tensor.matmul(
                                ps[:], ident[:],
                                bgiB[:, l, d, j0:j0 + nj, :],
                                start=True, stop=False)
                            for jj in range(nj):
                                j = j0 + jj
                                for kk in range(kc):
                                    if l == 0:
                                        xs = embT_sb[:, kk,
                                                     BPC * t0:BPC * t1]
                                    else:
                                        xs = x1T[:, kk // 4, kk % 4,
                                                 t0:t1, :]
                                        xs = xs.rearrange("p t b -> p (t b)")
                                    nc.tensor.matmul(
                                        ps[:, jj, :],
                                        wih[:, kk, d, 128 * j:128 * (j + 1)],
                                        xs, start=False,
                                        stop=(jj == nj - 1 and kk == kc - 1))
                            if grp == 0:
                                dst = giRZB[:, d, 0:8, t0:t1, :]
                            else:
                                dst = giN[:, d, :, t0:t1, :]
                            dst = dst.rearrange("p j t b -> p j (t b)")
                            if (2 * d + grp) % 2 == 0:
                                nc.scalar.copy(dst, ps[:])
                            else:
                                nc.vector.tensor_copy(dst, ps[:])

                NGC = (T + GCH - 1) // GCH
                gi_chunk(0, 0)
                gi_chunk(0, 1)
                for s in range(T + LAG):
                    if s < T:
                        emit_step(0, s)
                    if s >= LAG:
                        emit_step(1, s - LAG)
                    if (s + 5) % GCH == 0 and 2 <= (s + 5) // GCH < NGC:
                        gi_chunk(0, (s + 5) // GCH)
                    if s >= 8 and (s - 8) % GCH == 0 and (s - 8) // GCH < NGC:
                        gi_chunk(1, (s - 8) // GCH)
                    if s == T + LAG - 6:
                        # prefetch the first logits chunks on the idle DMA
                        for c in range(2):
                            fw_pre[c] = load_fwb(c)

        # ---------- logits + log_softmax (row-parallel, full vocab) --------
        NBLK = TR // 128
        nblk = NBLK if nblk_lim is None else min(nblk_lim, NBLK)
        with (
            tc.tile_pool(name="ltp", bufs=1) as ltp,
            tc.tile_pool(name="scrp", bufs=3) as scrp,
            tc.tile_pool(name="accp", bufs=1) as accp,
            tc.tile_pool(name="spp", bufs=2) as spp,
            tc.tile_pool(name="lpsp", bufs=8, space="PSUM") as lpsp,
        ):
            # lt/acc allocated ONCE and reused across passes: reallocating
            # per pass makes the first write of pass p+1 wait (whole-tile
            # WAR) on all 126 finalize-piece reads of pass p.
            ltbuf = [ltp.tile([128, NCH, CH], f16, name=f"ltb{i}",
                              tag=f"lt{i}", bufs=1) for i in range(2)]
            accbuf = [accp.tile([128, NCH], f32, name=f"accb{i}",
                                tag=f"acc{i}", bufs=1) for i in range(2)]

            # finalize-piece emitters from the previous pass, interleaved
            # into the next pass's chunk loop so the DMA queue alternates
            # output pieces with fc_w loads (no head-of-line stall).
            pend = []

            def emit_piece():
                if pend:
                    pend.pop(0)()

            def finalize(blk, lt_t, acc_t):
                ssum = spp.tile([128, 1], f32, tag="ssum",
                                name=f"ssum{blk}")
                nc.vector.tensor_reduce(ssum[:], acc_t[:], axis=AX.X,
                                        op=OP.add)
                lnS = spp.tile([128, 1], f32, tag=f"lnS{blk % 2}",
                               name=f"lnS{blk}")
                nc.scalar.activation(lnS[:], ssum[:], AF.Ln)

                def piece(c, blk=blk, lt_t=lt_t, lnS=lnS):
                    pc = lt_t[:, c, :]
                    nc.vector.tensor_scalar_sub(pc, pc, lnS[:])
                    w = min(CH, V - CH * c)
                    nc.sync.dma_start(
                        out_d[128 * blk:128 * (blk + 1), CH * c:CH * c + w],
                        pc[:, 0:w])
                return [lambda c=c: piece(c) for c in range(NCH)]

            for p2 in range((nblk + 1) // 2):
                blks = [b2 for b2 in (2 * p2, 2 * p2 + 1) if b2 < nblk]
                lt = {blk: ltbuf[blk % 2] for blk in blks}
                acc = {blk: accbuf[blk % 2] for blk in blks}
                for c in range(NCH):
                    if p2 == 0 and c in fw_pre:
                        fw, fb = fw_pre[c]
                    else:
                        fw, fb = load_fwb(c)
                    emit_piece()
                    emit_piece()
                    for blk in blks:
                        ps = lpsp.tile([128, CH], f32, tag="lp")
                        for kk in range(8):
                            lhsT = x2T[:, kk // 4, kk % 4,
                                       32 * blk:32 * (blk + 1), :]
                            nc.tensor.matmul(ps[:], lhsT, fw[:, kk, :],
                                             start=(kk == 0),
                                             stop=(kk == 7))
                        # l = matmul + bias into the block tile (DVE),
                        # exp into a scratch tile for the row-sum
                        nc.vector.tensor_add(lt[blk][:, c, :], ps[:], fb[:])
                        scr = scrp.tile([128, CH], f16, tag="scr")
                        nc.scalar.activation(scr[:], lt[blk][:, c, :],
                                             AF.Exp,
                                             accum_out=acc[blk][:, c:c + 1])
                # zip the two blocks' piece lists so chunk-c pieces of BOTH
                # buffers come out before the next pass overwrites chunk c
                plists = [finalize(blk, lt[blk], acc[blk]) for blk in blks]
                for tup in zip(*plists):
                    pend.extend(tup)
            while pend:
                emit_piece()

    nc.compile()
    return nc


def _get_nc():
    if "nc" not in _BUILT:
        _BUILT["nc"] = _build_nc()
    return _BUILT["nc"]


def _prep_inputs(inputs):
    """Host-side shard + relayout. Returns in_maps for 8 cores."""
    f16 = np.float16

    tgt = np.asarray(inputs["target"])
    ctx = np.asarray(inputs["context"], np.float32)
    emb_t = np.asarray(inputs["embed_table"], np.float32)
    fc_w = np.asarray(inputs["fc_w"], np.float32)
    fc_b = np.asarray(inputs["fc_b"], np.float32)
    w_ih0 = np.asarray(inputs["w_ih0"], np.float32)
    w_hh0 = np.asarray(inputs["w_hh0"], np.float32)
    w_ih1 = np.asarray(inputs["w_ih1"], np.float32)
    w_hh1 = np.asarray(inputs["w_hh1"], np.float32)
    b_ih0 = np.asarray(inputs["b_ih0"], np.float32)
    b_hh0 = np.asarray(inputs["b_hh0"], np.float32)
    b_ih1 = np.asarray(inputs["b_ih1"], np.float32)
    b_hh1 = np.asarray(inputs["b_hh1"], np.float32)

    def wT(w, kc):     # [2, G, I] -> [128, kc, 2, G]
        return np.ascontiguousarray(
            w.transpose(2, 0, 1).reshape(kc, 128, 2, G).transpose(1, 0, 2, 3)
        ).astype(f16)

    gmask_rz = (np.arange(G) < 2 * H)

    def bgi(b_ih, b_hh):   # [128, 2, 12, GC_NR]
        v = b_ih + np.where(gmask_rz[None, :], b_hh, 0.0)
        v = v.reshape(2, 12, 128).transpose(2, 0, 1)
        return np.ascontiguousarray(np.broadcast_to(
            v[:, :, :, None], (128, 2, 12, GC_NR))).astype(f16)

    def bhhnB(b_hh):   # [128, 2, 4, S, BPC] broadcast of the n-part
        bn = b_hh[:, 2 * H:].reshape(2, 4, 128).transpose(2, 0, 1)
        return np.ascontiguousarray(np.broadcast_to(
            bn[:, :, :, None, None], (128, 2, 4, S, BPC))).astype(f16)

    fcw_pad = np.zeros((VP, 2 * H), np.float32)
    fcw_pad[:V] = fc_w
    fcb_pad = np.full((VP,), NEG, np.float32)
    fcb_pad[:V] = fc_b
    import ml_dtypes
    fcwT = np.ascontiguousarray(
        fcw_pad.reshape(NCH, CH, 2 * H).transpose(0, 2, 1)
        .reshape(NCH, 8, 128, CH).transpose(0, 2, 1, 3)).astype(
            ml_dtypes.float8_e4m3)
    fcbB = np.ascontiguousarray(np.broadcast_to(
        fcb_pad.reshape(1, NCH, CH), (128, NCH, CH))).astype(f16)

    bGiB = np.ascontiguousarray(np.stack(
        [bgi(b_ih0, b_hh0), bgi(b_ih1, b_hh1)], axis=1))

    shared = {
        "WihT0": wT(w_ih0, 4), "WhhT0": wT(w_hh0, 4),
        "WihT1": wT(w_ih1, 8), "WhhT1": wT(w_hh1, 4),
        "bGiB": bGiB,
        "bHhnB0": bhhnB(b_hh0), "bHhnB1": bhhnB(b_hh1),
        "fcwT": fcwT, "fcbB": fcbB,
    }

    emb = emb_t[tgt]                      # [B, S, E]
    ctx4 = ctx.reshape(L, 2, B, H)        # [l, d, b, h]

    in_maps = []
    for c in range(NC_):
        bs = slice(BPC * c, BPC * (c + 1))
        er = emb[bs].transpose(1, 0, 2).reshape(R, E)   # row = 4t + b
        embTc = np.ascontiguousarray(
            er.T.reshape(4, 128, R).transpose(1, 0, 2)).astype(f16)
        cc = ctx4[:, :, bs, :]                          # [l, d, 4, h]
        h0Tc = np.ascontiguousarray(
            cc.transpose(3, 0, 1, 2).reshape(4, 128, L, 2, BPC)
            .transpose(1, 2, 3, 0, 4)).astype(f16)
        m = {"embT": embTc, "h0T": h0Tc}
        m.update(shared)
        in_maps.append(m)
    return in_maps


def _unshard(results):
    outs = [np.asarray(results[c]["out"], np.float32) for c in range(NC_)]
    full = np.stack(outs)                 # [8, R, V], row = 4t + b
    return np.ascontiguousarray(
        full.reshape(NC_, S, BPC, V).transpose(0, 2, 1, 3).reshape(B, S, V))


def kernel(**inputs):
    from concourse.bass_utils import run_bass_kernel_spmd
    nc = _get_nc()
    in_maps = _prep_inputs(inputs)
    res = run_bass_kernel_spmd(nc, in_maps, core_ids=list(range(NC_)))
    return _unshard(res.results)



# revision 5
# speedup vs baseline: 1.1871x; 1.1871x over previous
"""Trainium2 Bass kernel for nn_PlainDecoder (2-layer 2-direction GRU decoder
+ vocab projection + log_softmax).

Sharding:
  - GRU scan: data-parallel over batch (32 batches -> 4 per core). Each core
    runs both "directions" of both layers for its 4 batches.
  - Logits/log_softmax: ROW-parallel. Each core computes the full 32000-wide
    logits + log_softmax for its own 512 (t, b) rows, streaming the full
    fc_w from DRAM (fp8-e4m3). No collectives anywhere in the kernel.

Scan layout (gate-major / weight-stationary): the recurrent gate matmuls put
the GATE dim on PSUM partitions: lhsT = Whh^T chunk [128 (h-sub), 128 (gate
sub)], rhs = h^T [128 (h-sub), 4 (batch)]. The output h'^T lands directly in
the [h-sub partitions, batch] layout the next step's rhs needs -> zero
transposes. Precomputed gi (+ fused biases) and b_hh_n ride into PSUM via
identity-matmul injections. The gates are split r|n|z across PSUM banks so
the sigmoid(r) chain head fires after ~1/3 of the step's matmuls. The two
layer scans are software-pipelined (layer 1 lags layer 0 by LAG slots) with
both layers' gi built chunk-by-chunk on spare PE cycles inside the slot loop.

Everything wide is float16 (state, weights, gi, output): fp16 matmuls run at
1 cycle/row on the PE at any p-state, and fp16's 10-bit mantissa keeps the
recurrent rounding walk ~8x below bf16's.
"""

import os
import sys
from contextlib import ExitStack

for _p in ("/opt/trn_rl_repo", "/root/.axon_site/_ro/trn_rl_repo"):
    if os.path.isdir(_p) and _p not in sys.path:
        sys.path.insert(0, _p)

import numpy as np  # noqa: E402

V, E, H, L, B, S = 32000, 512, 512, 2, 32, 128
NC_ = 8                      # cores
BPC = B // NC_               # batches per core = 4
R = BPC * S                  # rows per core = 512 (row = 4*t + b)
G = 3 * H                    # 1536 gates per direction
CH = 512                     # vocab chunk width
NCH = 63                     # chunks -> 32256 padded vocab
VP = NCH * CH
NEG = -80.0                  # pad bias -> exp() ~ 0
GCH = 8                      # gi chunk = 8 timesteps
GC_NR = GCH * BPC            # rows per gi chunk

_BUILT = {}


def _build_nc(T=S, n_cores=NC_, sim=False, nblk_lim=None, skip_gi=False):
    """Build the Bass program (same NEFF for all cores; per-core data only).

    sim is accepted for compatibility; the program has no collectives so the
    TimelineSim build is identical.
    """
    import concourse.bass as bass  # noqa: F401
    import concourse.mybir as mybir
    import concourse.tile as tile
    from concourse import bacc
    from concourse.masks import make_identity

    dt = mybir.dt
    f32 = dt.float32
    f16 = dt.float16
    AF = mybir.ActivationFunctionType
    OP = mybir.AluOpType
    AX = mybir.AxisListType
    DR = mybir.MatmulPerfMode.DoubleRow

    nc = bacc.Bacc("TRN2", target_bir_lowering=False, debug=False,
                   num_devices=n_cores)

    TR = BPC * T               # rows actually scanned

    # ---------------- DRAM I/O ----------------
    embT = nc.dram_tensor("embT", [128, 4, R], f16, kind="ExternalInput")
    h0T = nc.dram_tensor("h0T", [128, 2, 2, 4, BPC], f16,
                         kind="ExternalInput")          # (p, l, d, k, b)
    WhhT0 = nc.dram_tensor("WhhT0", [128, 4, 2, G], f16, kind="ExternalInput")
    WhhT1 = nc.dram_tensor("WhhT1", [128, 4, 2, G], f16, kind="ExternalInput")
    WihT0 = nc.dram_tensor("WihT0", [128, 4, 2, G], f16, kind="ExternalInput")
    WihT1 = nc.dram_tensor("WihT1", [128, 8, 2, G], f16, kind="ExternalInput")
    # b_ih (+b_hh for r/z) broadcast over a gi-chunk's (t, b) columns:
    # rides each gi chunk's PSUM as one identity-matmul injection
    bGiB = nc.dram_tensor("bGiB", [128, 2, 2, 12, GC_NR], f16,
                          kind="ExternalInput")
    # b_hh n-part, broadcast over (t, b): injected into the n-gate PSUM.
    bHhnB0 = nc.dram_tensor("bHhnB0", [128, 2, 4, S, BPC], f16,
                            kind="ExternalInput")
    bHhnB1 = nc.dram_tensor("bHhnB1", [128, 2, 4, S, BPC], f16,
                            kind="ExternalInput")
    f8 = dt.float8e4
    # fc_w streams in fp8-e4m3 (halves the dominant DMA stream; the
    # weight-quantization noise is ~0.06 max on a ~10.4-magnitude output,
    # well inside the rel-err budget). x2 (the lhsT) stays fp16.
    fcwT = nc.dram_tensor("fcwT", [NCH, 128, 8, CH], f8,
                          kind="ExternalInput")
    # fc_b pre-broadcast across partitions: the bias rides a DVE add
    # (copy-from-PSUM fused) instead of a 512-cycle PE matmul per chunk
    fcbB = nc.dram_tensor("fcbB", [128, NCH, CH], f16, kind="ExternalInput")

    out_d = nc.dram_tensor("out", [R, V], f16, kind="ExternalOutput")

    with tile.TileContext(nc) as tc, ExitStack() as top:
        constp = top.enter_context(tc.tile_pool(name="const", bufs=1))
        ident = constp.tile([128, 128], f16)
        make_identity(nc, ident[:])
        h0T_sb = constp.tile([128, 2, 2, 4, BPC], f16)
        nc.sync.dma_start(h0T_sb[:], h0T[:])

        histp = top.enter_context(tc.tile_pool(name="hist", bufs=1))
        x2T = histp.tile([128, 2, 4, T, BPC], f16, tag="x2T")
        # fp8 copy of the scan output: lhsT for the DoubleRow logits matmuls
        x2f8 = histp.tile([128, 2, 4, T, BPC], f8, tag="x2f8")
        fwp = top.enter_context(tc.tile_pool(name="fwp", bufs=3))
        fbp = top.enter_context(tc.tile_pool(name="fbp", bufs=3))
        fw_pre = {}

        def load_fwb(c):
            fw = fwp.tile([128, 8, CH], f8, tag="fw", name=f"fw{c % 3}")
            nc.sync.dma_start(fw[:], fcwT[c])
            fb = fbp.tile([128, CH], f16, tag="fb", name=f"fb{c % 3}")
            nc.sync.dma_start(fb[:], fcbB[:, c, :])
            return fw, fb

        with ExitStack() as scan_stack:
            wres = scan_stack.enter_context(tc.tile_pool(name="wres", bufs=1))
            gip = scan_stack.enter_context(tc.tile_pool(name="gip", bufs=1))
            hist1 = scan_stack.enter_context(tc.tile_pool(name="hist1",
                                                          bufs=1))

            # One SHARED gi buffer for both layers: L0 consumes slot t at
            # slot t, and gi1's chunk for steps [t0, t0+GCH) is only
            # written after L0's reads of those slots — so layer 1's gi
            # overwrites layer 0's in place. Both gi builds run chunked
            # inside the slot loop on spare PE cycles.
            giRZB = gip.tile([128, 2, 12, T, BPC], f16, tag="giRZB")
            giN = gip.tile([128, 2, 4, T, BPC], f16, tag="giN")
            giR = [giRZB, giRZB]
            giNl = [giN, giN]
            x1T = hist1.tile([128, 2, 4, T, BPC], f16, tag="x1T")
            hists = [x1T, x2T]

            # input DMAs ordered by first use: gi chunks 0/1 (emb, wih0,
            # bias), L0 step 0 (b_hh0, whh0), then gi1/L1 weights
            embT_sb = wres.tile([128, 4, R], f16, tag="embT")
            nc.sync.dma_start(embT_sb[:], embT[:])
            bgiB = wres.tile([128, 2, 2, 12, GC_NR], f16, tag="bgiB")
            nc.sync.dma_start(bgiB[:], bGiB[:])
            wih0 = wres.tile([128, 4, 2, G], f16, tag="wih0")
            for d in range(2):
                nc.sync.dma_start(wih0[:, :, d, :], WihT0[:, :, d, :])
            whh0 = wres.tile([128, 4, 2, G], f16, tag="whh0")
            nc.sync.dma_start(whh0[:], WhhT0[:])
            # b_hh0 n-part upfront; b_hh1's rides each gi1 chunk
            nc.sync.dma_start(giRZB[:, :, 8:12, :, :],
                              bHhnB0[:, :, :, 0:T, :])
            wih1 = wres.tile([128, 8, 2, G], f16, tag="wih1")
            nc.sync.dma_start(wih1[:], WihT1[:])
            whh1 = wres.tile([128, 4, 2, G], f16, tag="whh1")
            nc.sync.dma_start(whh1[:], WhhT1[:])
            whhs = [whh0, whh1]

            # ---------- pipelined two-layer scan ----------
            LAG = 10
            with (
                tc.tile_pool(name="spr", bufs=2, space="PSUM") as przp,
                tc.tile_pool(name="spn", bufs=2, space="PSUM") as pnp,
                tc.tile_pool(name="spz", bufs=2, space="PSUM") as pzp,
                tc.tile_pool(name="gcp", bufs=2, space="PSUM") as gcp,
                tc.tile_pool(name="sch", bufs=8) as chp,
            ):
                def emit_step(l, t):
                    whh = whhs[l]
                    histT = hists[l]
                    gR = giR[l]
                    gN = giNl[l]

                    def rhs(d, k):
                        if t == 0:
                            return h0T_sb[:, l, d, k, :]
                        return histT[:, d, k, t - 1, :]

                    def gate_group(pool, jlo, jhi, nm):
                        ps = pool.tile([128, 2, jhi - jlo, BPC], f32,
                                       tag=nm, name=f"{nm}{l}")
                        for d in range(2):
                            nc.tensor.matmul(
                                ps[:, d, :, :], ident[:],
                                gR[:, d, jlo:jhi, t, :],
                                start=(d == 0), stop=False)
                            for j in range(jlo, jhi):
                                for k in range(4):
                                    nc.tensor.matmul(
                                        ps[:, d, j - jlo, :],
                                        whh[:, k, d, 128 * j:128 * (j + 1)],
                                        rhs(d, k), start=False,
                                        stop=(d == 1 and j == jhi - 1
                                              and k == 3))
                        return ps
                    # r first (heads the chain), then n (needed next), z last
                    pr = gate_group(przp, 0, 4, "pr")
                    pn = gate_group(pnp, 8, 12, "pn")
                    pz = gate_group(pzp, 4, 8, "pz")
                    rp = chp.tile([128, 2, 3, 4, BPC], f32, tag="rp",
                                  name=f"rp{l}")
                    n1 = chp.tile([128, 2, 4, BPC], f32, tag="n1",
                                  name=f"n1{l}")
                    oz = chp.tile([128, 2, 4, BPC], f32, tag="oz",
                                  name=f"oz{l}")
                    m1 = chp.tile([128, 2, 4, BPC], f32, tag="m1",
                                  name=f"m1{l}")
                    m2 = chp.tile([128, 2, 4, BPC], f32, tag="m2",
                                  name=f"m2{l}")
                    hprev = (h0T_sb[:, l] if t == 0
                             else histT[:, :, :, t - 1, :])
                    # r|z = sigmoid(gh + gi); n = tanh(gi_n + r*(gh_n+b))
                    nc.scalar.activation(rp[:, :, 0], pr[:], AF.Sigmoid)
                    nc.vector.tensor_mul(n1[:], pn[:], rp[:, :, 0])
                    nc.vector.tensor_add(n1[:], n1[:], gN[:, :, :, t, :])
                    nc.scalar.activation(rp[:, :, 1], pz[:], AF.Sigmoid)
                    nc.scalar.activation(rp[:, :, 2], n1[:], AF.Tanh)
                    # h' = z*h + (1-z)*n  (oz on gpsimd, off the chain)
                    nc.vector.tensor_scalar(oz[:], rp[:, :, 1], -1.0, 1.0,
                                            OP.mult, OP.add)
                    nc.vector.tensor_mul(m1[:], rp[:, :, 1], hprev)
                    nc.vector.tensor_mul(m2[:], oz[:], rp[:, :, 2])
                    nc.vector.tensor_add(histT[:, :, :, t, :], m1[:], m2[:])

                def gi_chunk(l, c):
                    t0, t1 = GCH * c, GCH * (c + 1)
                    nr = GC_NR
                    if l == 1:
                        # layer 1's b_hh_n replaces layer 0's in the shared
                        # injection slots (L0 consumed them slots ago)
                        for d in range(2):
                            nc.sync.dma_start(giRZB[:, d, 8:12, t0:t1, :],
                                              bHhnB1[:, d, :, t0:t1, :])
                    wih, kc = (wih0, 4) if l == 0 else (wih1, 8)
                    for d in range(2):
                        for grp, (j0, nj) in enumerate([(0, 8), (8, 4)]):
                            ps = gcp.tile([128, nj, nr], f32, tag="gc",
                                          name=f"gc{grp}")
                            # bias seeds the whole group (zeroes the bank)
                            nc.tensor.matmul(
                                ps[:], ident[:],
                                bgiB[:, l, d, j0:j0 + nj, :],
                                start=True, stop=False)
                            for jj in range(nj):
                                j = j0 + jj
                                for kk in range(kc):
                                    if l == 0:
                                        xs = embT_sb[:, kk,
                                                     BPC * t0:BPC * t1]
                                    else:
                                        xs = x1T[:, kk // 4, kk % 4,
                                                 t0:t1, :]
                                        xs = xs.rearrange("p t b -> p (t b)")
                                    nc.tensor.matmul(
                                        ps[:, jj, :],
                                        wih[:, kk, d, 128 * j:128 * (j + 1)],
                                        xs, start=False,
                                        stop=(jj == nj - 1 and kk == kc - 1))
                            if grp == 0:
                                dst = giRZB[:, d, 0:8, t0:t1, :]
                            else:
                                dst = giN[:, d, :, t0:t1, :]
                            dst = dst.rearrange("p j t b -> p j (t b)")
                            if (2 * d + grp) % 2 == 0:
                                nc.scalar.copy(dst, ps[:])
                            else:
                                nc.vector.tensor_copy(dst, ps[:])

                NGC = (T + GCH - 1) // GCH
                gi_chunk(0, 0)
                gi_chunk(0, 1)
                for s in range(T + LAG):
                    if s < T:
                        emit_step(0, s)
                    if s >= LAG:
                        emit_step(1, s - LAG)
                    if (s + 5) % GCH == 0 and 2 <= (s + 5) // GCH < NGC:
                        gi_chunk(0, (s + 5) // GCH)
                    if s >= 8 and (s - 8) % GCH == 0 and (s - 8) // GCH < NGC:
                        gi_chunk(1, (s - 8) // GCH)
                    if s >= LAG and (s - LAG + 1) % 32 == 0:
                        # cast the finished quarter of the scan output to
                        # fp8 (lhsT of the DoubleRow logits matmuls)
                        q = (s - LAG + 1) // 32 - 1
                        src = x2T[:, :, :, 32 * q:32 * (q + 1), :]
                        dst = x2f8[:, :, :, 32 * q:32 * (q + 1), :]
                        if q % 2 == 0:
                            nc.scalar.copy(dst, src)
                        else:
                            nc.vector.tensor_copy(dst, src)
                    if s == T + LAG - 6:
                        # prefetch the first logits chunks on the idle DMA
                        for c in range(2):
                            fw_pre[c] = load_fwb(c)

        # ---------- logits + log_softmax (row-parallel, full vocab) --------
        NBLK = TR // 128
        nblk = NBLK if nblk_lim is None else min(nblk_lim, NBLK)
        with (
            tc.tile_pool(name="ltp", bufs=1) as ltp,
            tc.tile_pool(name="scrp", bufs=3) as scrp,
            tc.tile_pool(name="accp", bufs=1) as accp,
            tc.tile_pool(name="spp", bufs=2) as spp,
            tc.tile_pool(name="lpsp", bufs=8, space="PSUM") as lpsp,
        ):
            # lt/acc allocated ONCE and reused across passes: reallocating
            # per pass makes the first write of pass p+1 wait (whole-tile
            # WAR) on all 126 finalize-piece reads of pass p.
            ltbuf = [ltp.tile([128, NCH, CH], f16, name=f"ltb{i}",
                              tag=f"lt{i}", bufs=1) for i in range(2)]
            accbuf = [accp.tile([128, NCH], f32, name=f"accb{i}",
                                tag=f"acc{i}", bufs=1) for i in range(2)]

            # finalize-piece emitters from the previous pass, interleaved
            # into the next pass's chunk loop so the DMA queue alternates
            # output pieces with fc_w loads (no head-of-line stall).
            pend = []

            def emit_piece():
                if pend:
                    pend.pop(0)()

            def finalize(blk, lt_t, acc_t):
                ssum = spp.tile([128, 1], f32, tag="ssum",
                                name=f"ssum{blk}")
                nc.vector.tensor_reduce(ssum[:], acc_t[:], axis=AX.X,
                                        op=OP.add)
                lnS = spp.tile([128, 1], f32, tag=f"lnS{blk % 2}",
                               name=f"lnS{blk}")
                nc.scalar.activation(lnS[:], ssum[:], AF.Ln)

                def piece(c, blk=blk, lt_t=lt_t, lnS=lnS):
                    pc = lt_t[:, c, :]
                    nc.vector.tensor_scalar_sub(pc, pc, lnS[:])
                    w = min(CH, V - CH * c)
                    nc.sync.dma_start(
                        out_d[128 * blk:128 * (blk + 1), CH * c:CH * c + w],
                        pc[:, 0:w])
                return [lambda c=c: piece(c) for c in range(NCH)]

            for p2 in range((nblk + 1) // 2):
                blks = [b2 for b2 in (2 * p2, 2 * p2 + 1) if b2 < nblk]
                lt = {blk: ltbuf[blk % 2] for blk in blks}
                acc = {blk: accbuf[blk % 2] for blk in blks}
                for c in range(NCH):
                    if p2 == 0 and c in fw_pre:
                        fw, fb = fw_pre[c]
                    else:
                        fw, fb = load_fwb(c)
                    emit_piece()
                    emit_piece()
                    for blk in blks:
                        ps = lpsp.tile([128, CH], f32, tag="lp")
                        # fp8 DoubleRow: each matmul contracts TWO 128-tiles
                        # (pair dim1) at 0.5 cycles/row -> 4x the fp16 rate
                        for kp in range(4):
                            lhsT = x2f8[:, kp // 2,
                                        2 * (kp % 2):2 * (kp % 2) + 2,
                                        32 * blk:32 * (blk + 1), :]
                            nc.tensor.matmul(ps[:], lhsT,
                                             fw[:, 2 * kp:2 * kp + 2, :],
                                             start=(kp == 0),
                                             stop=(kp == 3),
                                             perf_mode=DR)
                        # l = matmul + bias into the block tile (DVE),
                        # exp into a scratch tile for the row-sum
                        nc.vector.tensor_add(lt[blk][:, c, :], ps[:], fb[:])
                        scr = scrp.tile([128, CH], f16, tag="scr")
                        nc.scalar.activation(scr[:], lt[blk][:, c, :],
                                             AF.Exp,
                                             accum_out=acc[blk][:, c:c + 1])
                # zip the two blocks' piece lists so chunk-c pieces of BOTH
                # buffers come out before the next pass overwrites chunk c
                plists = [finalize(blk, lt[blk], acc[blk]) for blk in blks]
                for tup in zip(*plists):
                    pend.extend(tup)
            while pend:
                emit_piece()

    nc.compile()
    return nc


def _get_nc():
    if "nc" not in _BUILT:
        _BUILT["nc"] = _build_nc()
    return _BUILT["nc"]


def _prep_inputs(inputs):
    """Host-side shard + relayout. Returns in_maps for 8 cores."""
    f16 = np.float16

    tgt = np.asarray(inputs["target"])
    ctx = np.asarray(inputs["context"], np.float32)
    emb_t = np.asarray(inputs["embed_table"], np.float32)
    fc_w = np.asarray(inputs["fc_w"], np.float32)
    fc_b = np.asarray(inputs["fc_b"], np.float32)
    w_ih0 = np.asarray(inputs["w_ih0"], np.float32)
    w_hh0 = np.asarray(inputs["w_hh0"], np.float32)
    w_ih1 = np.asarray(inputs["w_ih1"], np.float32)
    w_hh1 = np.asarray(inputs["w_hh1"], np.float32)
    b_ih0 = np.asarray(inputs["b_ih0"], np.float32)
    b_hh0 = np.asarray(inputs["b_hh0"], np.float32)
    b_ih1 = np.asarray(inputs["b_ih1"], np.float32)
    b_hh1 = np.asarray(inputs["b_hh1"], np.float32)

    def wT(w, kc):     # [2, G, I] -> [128, kc, 2, G]
        return np.ascontiguousarray(
            w.transpose(2, 0, 1).reshape(kc, 128, 2, G).transpose(1, 0, 2, 3)
        ).astype(f16)

    gmask_rz = (np.arange(G) < 2 * H)

    def bgi(b_ih, b_hh):   # [128, 2, 12, GC_NR]
        v = b_ih + np.where(gmask_rz[None, :], b_hh, 0.0)
        v = v.reshape(2, 12, 128).transpose(2, 0, 1)
        return np.ascontiguousarray(np.broadcast_to(
            v[:, :, :, None], (128, 2, 12, GC_NR))).astype(f16)

    def bhhnB(b_hh):   # [128, 2, 4, S, BPC] broadcast of the n-part
        bn = b_hh[:, 2 * H:].reshape(2, 4, 128).transpose(2, 0, 1)
        return np.ascontiguousarray(np.broadcast_to(
            bn[:, :, :, None, None], (128, 2, 4, S, BPC))).astype(f16)

    fcw_pad = np.zeros((VP, 2 * H), np.float32)
    fcw_pad[:V] = fc_w
    fcb_pad = np.full((VP,), NEG, np.float32)
    fcb_pad[:V] = fc_b
    import ml_dtypes
    fcwT = np.ascontiguousarray(
        fcw_pad.reshape(NCH, CH, 2 * H).transpose(0, 2, 1)
        .reshape(NCH, 8, 128, CH).transpose(0, 2, 1, 3)).astype(
            ml_dtypes.float8_e4m3)
    fcbB = np.ascontiguousarray(np.broadcast_to(
        fcb_pad.reshape(1, NCH, CH), (128, NCH, CH))).astype(f16)

    bGiB = np.ascontiguousarray(np.stack(
        [bgi(b_ih0, b_hh0), bgi(b_ih1, b_hh1)], axis=1))

    shared = {
        "WihT0": wT(w_ih0, 4), "WhhT0": wT(w_hh0, 4),
        "WihT1": wT(w_ih1, 8), "WhhT1": wT(w_hh1, 4),
        "bGiB": bGiB,
        "bHhnB0": bhhnB(b_hh0), "bHhnB1": bhhnB(b_hh1),
        "fcwT": fcwT, "fcbB": fcbB,
    }

    emb = emb_t[tgt]                      # [B, S, E]
    ctx4 = ctx.reshape(L, 2, B, H)        # [l, d, b, h]

    in_maps = []
    for c in range(NC_):
        bs = slice(BPC * c, BPC * (c + 1))
        er = emb[bs].transpose(1, 0, 2).reshape(R, E)   # row = 4t + b
        embTc = np.ascontiguousarray(
            er.T.reshape(4, 128, R).transpose(1, 0, 2)).astype(f16)
        cc = ctx4[:, :, bs, :]                          # [l, d, 4, h]
        h0Tc = np.ascontiguousarray(
            cc.transpose(3, 0, 1, 2).reshape(4, 128, L, 2, BPC)
            .transpose(1, 2, 3, 0, 4)).astype(f16)
        m = {"embT": embTc, "h0T": h0Tc}
        m.update(shared)
        in_maps.append(m)
    return in_maps


def _unshard(results):
    outs = [np.asarray(results[c]["out"], np.float32) for c in range(NC_)]
    full = np.stack(outs)                 # [8, R, V], row = 4t + b
    return np.ascontiguousarray(
        full.reshape(NC_, S, BPC, V).transpose(0, 2, 1, 3).reshape(B, S, V))


def kernel(**inputs):
    from concourse.bass_utils import run_bass_kernel_spmd
    nc = _get_nc()
    in_maps = _prep_inputs(inputs)
    res = run_bass_kernel_spmd(nc, in_maps, core_ids=list(range(NC_)))
    return _unshard(res.results)



# revision 44
# speedup vs baseline: 1.3081x; 1.1019x over previous
"""Trainium2 Bass kernel for nn_PlainDecoder (2-layer 2-direction GRU decoder
+ vocab projection + log_softmax).

Sharding:
  - GRU scan: data-parallel over batch (32 batches -> 4 per core). Each core
    runs both "directions" of both layers for its 4 batches.
  - Logits/log_softmax: ROW-parallel. Each core computes the full 32000-wide
    logits + log_softmax for its own 512 (t, b) rows, streaming the full
    fc_w from DRAM (fp8-e4m3). No collectives anywhere in the kernel.

Scan layout (gate-major / weight-stationary): the recurrent gate matmuls put
the GATE dim on PSUM partitions: lhsT = Whh^T chunk [128 (h-sub), 128 (gate
sub)], rhs = h^T [128 (h-sub), 4 (batch)]. The output h'^T lands directly in
the [h-sub partitions, batch] layout the next step's rhs needs -> zero
transposes. Precomputed gi (+ fused biases) and b_hh_n ride into PSUM via
identity-matmul injections. The gates are split r|n|z across PSUM banks so
the sigmoid(r) chain head fires after ~1/3 of the step's matmuls. The two
layer scans are software-pipelined (layer 1 lags layer 0 by LAG slots) with
both layers' gi built chunk-by-chunk on spare PE cycles inside the slot loop.

Everything wide is float16 (state, weights, gi, output): fp16 matmuls run at
1 cycle/row on the PE at any p-state, and fp16's 10-bit mantissa keeps the
recurrent rounding walk ~8x below bf16's.
"""

import os
import sys
from contextlib import ExitStack

for _p in ("/opt/trn_rl_repo", "/root/.axon_site/_ro/trn_rl_repo"):
    if os.path.isdir(_p) and _p not in sys.path:
        sys.path.insert(0, _p)

import numpy as np  # noqa: E402

V, E, H, L, B, S = 32000, 512, 512, 2, 32, 128
NC_ = 8                      # cores
BPC = B // NC_               # batches per core = 4
R = BPC * S                  # rows per core = 512 (row = 4*t + b)
G = 3 * H                    # 1536 gates per direction
CH = 512                     # vocab chunk width
VS = V // NC_                # per-core vocab shard = 4000
VSC = 8                      # chunks per shard -> 4096 padded
NEG = -80.0                  # pad bias -> exp() ~ 0
GCH = 8                      # gi chunk = 8 timesteps
GC_NR = GCH * BPC            # rows per gi chunk

_BUILT = {}


def _build_nc(T=S, n_cores=NC_, sim=False, nblk_lim=None, skip_gi=False):
    """Build the Bass program (same NEFF for all cores; per-core data only).

    sim is accepted for compatibility; the program has no collectives so the
    TimelineSim build is identical.
    """
    import concourse.bass as bass  # noqa: F401
    import concourse.mybir as mybir
    import concourse.tile as tile
    from concourse import bacc
    from concourse.masks import make_identity

    dt = mybir.dt
    f32 = dt.float32
    f16 = dt.float16
    AF = mybir.ActivationFunctionType
    OP = mybir.AluOpType
    AX = mybir.AxisListType
    DR = mybir.MatmulPerfMode.DoubleRow

    nc = bacc.Bacc("TRN2", target_bir_lowering=False, debug=False,
                   num_devices=n_cores)

    TR = BPC * T               # rows actually scanned

    # ---------------- DRAM I/O ----------------
    embT = nc.dram_tensor("embT", [128, 4, R], f16, kind="ExternalInput")
    h0T = nc.dram_tensor("h0T", [128, 2, 2, 4, BPC], f16,
                         kind="ExternalInput")          # (p, l, d, k, b)
    WhhT0 = nc.dram_tensor("WhhT0", [128, 4, 2, G], f16, kind="ExternalInput")
    WhhT1 = nc.dram_tensor("WhhT1", [128, 4, 2, G], f16, kind="ExternalInput")
    WihT0 = nc.dram_tensor("WihT0", [128, 4, 2, G], f16, kind="ExternalInput")
    WihT1 = nc.dram_tensor("WihT1", [128, 8, 2, G], f16, kind="ExternalInput")
    # b_ih (+b_hh for r/z) broadcast over a gi-chunk's (t, b) columns:
    # rides each gi chunk's PSUM as one identity-matmul injection
    bGiB = nc.dram_tensor("bGiB", [128, 2, 2, 12, GC_NR], f16,
                          kind="ExternalInput")
    # b_hh n-part, broadcast over (t, b): injected into the n-gate PSUM.
    bHhnB0 = nc.dram_tensor("bHhnB0", [128, 2, 4, S, BPC], f16,
                            kind="ExternalInput")
    bHhnB1 = nc.dram_tensor("bHhnB1", [128, 2, 4, S, BPC], f16,
                            kind="ExternalInput")
    f8 = dt.float8e4
    # Vocab-parallel logits: this core holds fc_w rows [VS*k, VS*k + VSC*CH)
    # (zero-padded), fp8-e4m3.  [chunk, p, h-tile, col]
    fcw8_d = nc.dram_tensor("fcw8", [VSC, 128, 8, CH], f8,
                            kind="ExternalInput")
    # fc_b shard as the rhs of a 5th DoubleRow pair: chunk ch's bias lives
    # on partition ch (pad cols carry NEG so their exp vanishes)
    biasT_d = nc.dram_tensor("biasT", [128, 2, CH], f8,
                             kind="ExternalInput")
    # lhsT of the bias pair for chunk ch: 1.0 at [p=ch, i=0, :], else 0
    e8_d = nc.dram_tensor("e8", [128, 2, VSC, 128], f8,
                          kind="ExternalInput")

    # this core's vocab slice for ALL rows: [src_core, row, vs]
    out_d = nc.dram_tensor("out", [NC_, R, VS], f16, kind="ExternalOutput")

    with tile.TileContext(nc) as tc, ExitStack() as top:
        constp = top.enter_context(tc.tile_pool(name="const", bufs=1))
        ident = constp.tile([128, 128], f16)
        make_identity(nc, ident[:])
        h0T_sb = constp.tile([128, 2, 2, 4, BPC], f16)
        nc.sync.dma_start(h0T_sb[:], h0T[:])

        # resident logits operands (fcw8 itself loads post-scan into the
        # space the scan frees); gathered fp8 scan outputs
        gathp = top.enter_context(tc.tile_pool(name="gath", bufs=1))
        biasT = gathp.tile([128, 2, CH], f8, tag="biasT")
        e8 = gathp.tile([128, 2, VSC, 128], f8, tag="e8")
        # all cores' scan outputs: [p, h-tile, src_core, local row (4t+b)],
        # one tile per quarter so block reads only dep on their own gather
        QR = R // 4
        x2all = [gathp.tile([128, 8, NC_, QR], f8, name=f"x2all{q}")
                 for q in range(4)]
        # DRAM bounce buffers for the x2 all-gathers + sum all-reduces
        dramp = top.enter_context(tc.tile_pool(name="dram", bufs=1,
                                               space="DRAM"))
        x2qi = [dramp.tile([128, 8, QR], f8, name=f"x2qi{q}")
                for q in range(4)]
        x2qo = [dramp.tile([NC_, 128, 8, QR], f8, name=f"x2qo{q}")
                for q in range(4)]
        ari = [dramp.tile([128, 8], f32, name=f"ari{g}") for g in range(5)]
        aro = [dramp.tile([NC_, 128, 8], f32, name=f"aro{g}")
               for g in range(5)]

        with ExitStack() as scan_stack:
            wres = scan_stack.enter_context(tc.tile_pool(name="wres", bufs=1))
            gip = scan_stack.enter_context(tc.tile_pool(name="gip", bufs=1))
            hist1 = scan_stack.enter_context(tc.tile_pool(name="hist1",
                                                          bufs=1))
            # histories are 32-step rings (slot = t % RW): the recurrence
            # only needs t-1, gi1 builds lag <= 12 slots, and x2 is cast to
            # fp8 + staged to DRAM the moment each 32-step quarter finishes
            RW = 32
            x2T = hist1.tile([128, 2, 4, RW, BPC], f16, tag="x2T")
            x2f8 = hist1.tile([128, 2, 4, RW, BPC], f8, tag="x2f8")

            # One SHARED gi buffer for both layers: L0 consumes slot t at
            # slot t, and gi1's chunk for steps [t0, t0+GCH) is only
            # written after L0's reads of those slots — so layer 1's gi
            # overwrites layer 0's in place. Both gi builds run chunked
            # inside the slot loop on spare PE cycles.
            giRZB = gip.tile([128, 2, 12, T, BPC], f16, tag="giRZB")
            giN = gip.tile([128, 2, 4, T, BPC], f16, tag="giN")
            giR = [giRZB, giRZB]
            giNl = [giN, giN]
            x1T = hist1.tile([128, 2, 4, RW, BPC], f16, tag="x1T")
            hists = [x1T, x2T]

            # input DMAs ordered by first use: gi chunks 0/1 (emb, wih0,
            # bias), L0 step 0 (b_hh0, whh0), then gi1/L1 weights
            embT_sb = wres.tile([128, 4, R], f16, tag="embT")
            nc.sync.dma_start(embT_sb[:], embT[:])
            bgiB = wres.tile([128, 2, 2, 12, GC_NR], f16, tag="bgiB")
            nc.sync.dma_start(bgiB[:], bGiB[:])
            wih0 = wres.tile([128, 4, 2, G], f16, tag="wih0")
            for d in range(2):
                nc.sync.dma_start(wih0[:, :, d, :], WihT0[:, :, d, :])
            whh0 = wres.tile([128, 4, 2, G], f16, tag="whh0")
            nc.sync.dma_start(whh0[:], WhhT0[:])
            # b_hh0 n-part upfront; b_hh1's rides each gi1 chunk
            nc.sync.dma_start(giRZB[:, :, 8:12, :, :],
                              bHhnB0[:, :, :, 0:T, :])
            wih1 = wres.tile([128, 8, 2, G], f16, tag="wih1")
            nc.sync.dma_start(wih1[:], WihT1[:])
            whh1 = wres.tile([128, 4, 2, G], f16, tag="whh1")
            nc.sync.dma_start(whh1[:], WhhT1[:])
            whhs = [whh0, whh1]
            # small resident logits operands ride the idle DMA now; the
            # 16KB fcw8 tile loads post-scan into freed scan space
            nc.sync.dma_start(biasT[:], biasT_d[:])
            nc.sync.dma_start(e8[:], e8_d[:])

            # ---------- pipelined two-layer scan ----------
            LAG = 10
            with (
                tc.tile_pool(name="spr", bufs=2, space="PSUM") as przp,
                tc.tile_pool(name="spn", bufs=2, space="PSUM") as pnp,
                tc.tile_pool(name="spz", bufs=2, space="PSUM") as pzp,
                tc.tile_pool(name="gcp", bufs=2, space="PSUM") as gcp,
                tc.tile_pool(name="sch", bufs=8) as chp,
            ):
                def emit_step(l, t):
                    whh = whhs[l]
                    histT = hists[l]
                    gR = giR[l]
                    gN = giNl[l]

                    def rhs(d, k):
                        if t == 0:
                            return h0T_sb[:, l, d, k, :]
                        return histT[:, d, k, (t - 1) % RW, :]

                    def gate_group(pool, jlo, jhi, nm):
                        ps = pool.tile([128, 2, jhi - jlo, BPC], f32,
                                       tag=nm, name=f"{nm}{l}")
                        for d in range(2):
                            nc.tensor.matmul(
                                ps[:, d, :, :], ident[:],
                                gR[:, d, jlo:jhi, t, :],
                                start=(d == 0), stop=False)
                            for j in range(jlo, jhi):
                                for k in range(4):
                                    nc.tensor.matmul(
                                        ps[:, d, j - jlo, :],
                                        whh[:, k, d, 128 * j:128 * (j + 1)],
                                        rhs(d, k), start=False,
                                        stop=(d == 1 and j == jhi - 1
                                              and k == 3))
                        return ps
                    # r first (heads the chain), then n (needed next), z last
                    pr = gate_group(przp, 0, 4, "pr")
                    pn = gate_group(pnp, 8, 12, "pn")
                    pz = gate_group(pzp, 4, 8, "pz")
                    rp = chp.tile([128, 2, 3, 4, BPC], f32, tag="rp",
                                  name=f"rp{l}")
                    n1 = chp.tile([128, 2, 4, BPC], f32, tag="n1",
                                  name=f"n1{l}")
                    oz = chp.tile([128, 2, 4, BPC], f32, tag="oz",
                                  name=f"oz{l}")
                    m1 = chp.tile([128, 2, 4, BPC], f32, tag="m1",
                                  name=f"m1{l}")
                    m2 = chp.tile([128, 2, 4, BPC], f32, tag="m2",
                                  name=f"m2{l}")
                    hprev = (h0T_sb[:, l] if t == 0
                             else histT[:, :, :, (t - 1) % RW, :])
                    # r|z = sigmoid(gh + gi); n = tanh(gi_n + r*(gh_n+b))
                    nc.scalar.activation(rp[:, :, 0], pr[:], AF.Sigmoid)
                    nc.vector.tensor_mul(n1[:], pn[:], rp[:, :, 0])
                    nc.vector.tensor_add(n1[:], n1[:], gN[:, :, :, t, :])
                    nc.scalar.activation(rp[:, :, 1], pz[:], AF.Sigmoid)
                    nc.scalar.activation(rp[:, :, 2], n1[:], AF.Tanh)
                    # h' = z*h + (1-z)*n  (oz on gpsimd, off the chain)
                    nc.vector.tensor_scalar(oz[:], rp[:, :, 1], -1.0, 1.0,
                                            OP.mult, OP.add)
                    nc.vector.tensor_mul(m1[:], rp[:, :, 1], hprev)
                    nc.vector.tensor_mul(m2[:], oz[:], rp[:, :, 2])
                    nc.vector.tensor_add(histT[:, :, :, t % RW, :],
                                         m1[:], m2[:])

                def gi_chunk(l, c):
                    t0, t1 = GCH * c, GCH * (c + 1)
                    nr = GC_NR
                    if l == 1:
                        # layer 1's b_hh_n replaces layer 0's in the shared
                        # injection slots (L0 consumed them slots ago)
                        for d in range(2):
                            nc.sync.dma_start(giRZB[:, d, 8:12, t0:t1, :],
                                              bHhnB1[:, d, :, t0:t1, :])
                    wih, kc = (wih0, 4) if l == 0 else (wih1, 8)
                    for d in range(2):
                        for grp, (j0, nj) in enumerate([(0, 8), (8, 4)]):
                            ps = gcp.tile([128, nj, nr], f32, tag="gc",
                                          name=f"gc{grp}")
                            # bias seeds the whole group (zeroes the bank)
                            nc.tensor.matmul(
                                ps[:], ident[:],
                                bgiB[:, l, d, j0:j0 + nj, :],
                                start=True, stop=False)
                            for jj in range(nj):
                                j = j0 + jj
                                for kk in range(kc):
                                    if l == 0:
                                        xs = embT_sb[:, kk,
                                                     BPC * t0:BPC * t1]
                                    else:
                                        r0 = t0 % RW
                                        xs = x1T[:, kk // 4, kk % 4,
                                                 r0:r0 + GCH, :]
                                        xs = xs.rearrange("p t b -> p (t b)")
                                    nc.tensor.matmul(
                                        ps[:, jj, :],
                                        wih[:, kk, d, 128 * j:128 * (j + 1)],
                                        xs, start=False,
                                        stop=(jj == nj - 1 and kk == kc - 1))
                            if grp == 0:
                                dst = giRZB[:, d, 0:8, t0:t1, :]
                            else:
                                dst = giN[:, d, :, t0:t1, :]
                            dst = dst.rearrange("p j t b -> p j (t b)")
                            if (2 * d + grp) % 2 == 0:
                                nc.scalar.copy(dst, ps[:])
                            else:
                                nc.vector.tensor_copy(dst, ps[:])

                NGC = (T + GCH - 1) // GCH
                gi_chunk(0, 0)
                gi_chunk(0, 1)
                for s in range(T + LAG):
                    if s < T:
                        emit_step(0, s)
                    if s >= LAG:
                        emit_step(1, s - LAG)
                    if (s + 5) % GCH == 0 and 2 <= (s + 5) // GCH < NGC:
                        gi_chunk(0, (s + 5) // GCH)
                    if s >= 8 and (s - 8) % GCH == 0 and (s - 8) // GCH < NGC:
                        gi_chunk(1, (s - 8) // GCH)
                    if s >= LAG and (s - LAG + 1) % 32 == 0:
                        # a quarter of the scan output is done: cast it to
                        # fp8, bounce to DRAM, all-gather it, and scatter
                        # the gathered copies into x2all
                        q = (s - LAG + 1) // 32 - 1
                        # the ring holds exactly quarter q (slot j = t - 32q)
                        if q % 2 == 0:
                            nc.scalar.copy(x2f8[:], x2T[:])
                        else:
                            nc.vector.tensor_copy(x2f8[:], x2T[:])
                        nc.sync.dma_start(
                            x2qi[q][:],
                            x2f8[:].rearrange("p d k t b -> p (d k) (t b)"))
                        nc.gpsimd.collective_compute(
                            "AllGather", OP.bypass,
                            replica_groups=[list(range(NC_))],
                            ins=[x2qi[q].opt()], outs=[x2qo[q].opt()])
                        # gpsimd queue: this DMA waits ~41us for the
                        # collective, which would head-of-line-block the
                        # scan's bias DMAs if it sat on the SP sequencer
                        nc.gpsimd.dma_start(
                            x2all[q][:],
                            x2qo[q][:].rearrange("c p h r -> p h c r"))

        # ------- logits + log_softmax (vocab-parallel over the 8 cores) ----
        # Every core computes its VS-wide vocab shard for ALL 4096 rows.
        # Blocks: (quarter i, src core c) -> 128 rows of x2all.  Per block:
        # 4 chunk-pairs of 5 DoubleRow matmuls (4 x2 pairs + bias pair),
        # exp+accum straight off PSUM, PSUM->lt copies split DVE/Pool.
        # Per quarter: one [128, 8] partial-sum AllReduce; subtract + out
        # DMA of quarter g interleaves into quarter g+1's compute.
        ngrp = 4 if nblk_lim is None else min(nblk_lim, 4)
        with (
            tc.tile_pool(name="fwp", bufs=1) as fwp,
            tc.tile_pool(name="ltp", bufs=14) as ltp,
            tc.tile_pool(name="scrp", bufs=4) as scrp,
            tc.tile_pool(name="accp", bufs=16) as accp,
            tc.tile_pool(name="arp", bufs=2) as arp,
            tc.tile_pool(name="pcp", bufs=4) as pcp,
            tc.tile_pool(name="lpsp", bufs=4, space="PSUM") as lpsp,
        ):
            # the weight shard loads into space the scan just freed; per-
            # chunk regions unblock the first blocks' matmuls as they land
            fcw8 = fwp.tile([128, VSC, 8, CH], f8, tag="fcw8")
            for ch in range(VSC):
                nc.sync.dma_start(fcw8[:, ch], fcw8_d[ch])
            pend = []

            def emit_piece():
                if pend:
                    pend.pop(0)()

            def emit_block(q, c, arin, slot, nblk_):
                lt = ltp.tile([128, VSC, CH], f16, tag="lt")
                acc = accp.tile([128, 4], f32, tag="acc")
                for kp in range(4):
                    ps = lpsp.tile([128, 2, CH], f32, tag="lp")
                    for j in range(2):
                        for hp in range(4):
                            lhsT = x2all[q][:, 2 * hp:2 * hp + 2, c, :]
                            nc.tensor.matmul(
                                ps[:, j, :], lhsT,
                                fcw8[:, 2 * kp + j, 2 * hp:2 * hp + 2, :],
                                start=(hp == 0), stop=False, perf_mode=DR)
                        nc.tensor.matmul(
                            ps[:, j, :], e8[:, :, 2 * kp + j, :],
                            biasT[:], start=False, stop=True, perf_mode=DR)
                    # exp of both chunks + row-sum accum, off PSUM
                    scr = scrp.tile([128, 2, CH], f16, tag="scr")
                    nc.scalar.activation(scr[:], ps[:], AF.Exp,
                                         accum_out=acc[:, kp:kp + 1])
                    # copy l out of PSUM (GPSIMD cannot touch PSUM, so the
                    # copies split DVE/ACT; Pool gets SBUF-side subtracts)
                    dst = lt[:, 2 * kp:2 * kp + 2, :]
                    if kp == 3:
                        nc.scalar.copy(dst, ps[:])
                    else:
                        nc.vector.tensor_copy(dst, ps[:])
                nc.vector.tensor_reduce(arin[:, slot:slot + 1], acc[:],
                                        axis=AX.X, op=OP.add)
                return lt

            def finalize(g, q, blks, lts):
                # partial sums were AllGathered (15us constant beats the
                # AllReduce's 28us): sum the 8 cores' partials locally
                arb = arp.tile([128, 8, NC_], f32, tag="arb")
                nc.sync.dma_start(arb[:], aro[g][:].rearrange(
                    "c p b -> p b c"))
                ssum = arp.tile([128, 8], f32, tag="ssum")
                nc.vector.tensor_reduce(ssum[:], arb[:], axis=AX.X,
                                        op=OP.add)
                lnS = arp.tile([128, 8], f32, tag="lnS")
                nc.scalar.activation(lnS[:], ssum[:], AF.Ln)

                def piece(i, c, kp, q=q, lts=lts, lnS=lnS):
                    w = min(2 * CH, VS - 2 * CH * kp)
                    pc = pcp.tile([128, 2, CH], f16, tag="pc")
                    src = lts[i][:, 2 * kp:2 * kp + 2, :]
                    if kp % 2 == 0:
                        nc.gpsimd.tensor_scalar_sub(pc[:], src,
                                                    lnS[:, i:i + 1])
                    else:
                        nc.vector.tensor_scalar_sub(pc[:], src,
                                                    lnS[:, i:i + 1])
                    nc.sync.dma_start(
                        out_d[c, 128 * q:128 * (q + 1),
                              2 * CH * kp:2 * CH * kp + w],
                        pc[:].rearrange("p a b -> p (a b)")[:, 0:w])
                return [lambda i=i, c=c, kp=kp: piece(i, c, kp)
                        for i, c in enumerate(blks) for kp in range(4)]

            # groups of (quarter, src-core) blocks; the last two are small
            # so the tail exchange+drain after the final blocks is short
            groups = [(0, list(range(8))), (1, list(range(8))),
                      (2, list(range(8))), (3, [0, 1, 2, 3]),
                      (3, [4, 5, 6, 7])][:ngrp if ngrp < 4 else 5]
            nblk_ = 0
            for g, (q, blks) in enumerate(groups):
                arin = arp.tile([128, 8], f32, tag="arin", name=f"ari{g}")
                lts = []
                for slot, c in enumerate(blks):
                    lts.append(emit_block(q, c, arin, slot, nblk_))
                    nblk_ += 1
                    for _ in range(4):
                        emit_piece()
                if len(blks) < 8:
                    nc.vector.memset(arin[:, len(blks):], 0.0)
                nc.sync.dma_start(ari[g][:], arin[:])
                nc.gpsimd.collective_compute(
                    "AllGather", OP.bypass,
                    replica_groups=[list(range(NC_))],
                    ins=[ari[g].opt()], outs=[aro[g].opt()])
                pend.extend(finalize(g, q, blks, lts))
            while pend:
                emit_piece()

    nc.compile()
    return nc


def _get_nc():
    if "nc" not in _BUILT:
        _BUILT["nc"] = _build_nc()
    return _BUILT["nc"]


def _prep_inputs(inputs):
    """Host-side shard + relayout. Returns in_maps for 8 cores."""
    f16 = np.float16

    tgt = np.asarray(inputs["target"])
    ctx = np.asarray(inputs["context"], np.float32)
    emb_t = np.asarray(inputs["embed_table"], np.float32)
    fc_w = np.asarray(inputs["fc_w"], np.float32)
    fc_b = np.asarray(inputs["fc_b"], np.float32)
    w_ih0 = np.asarray(inputs["w_ih0"], np.float32)
    w_hh0 = np.asarray(inputs["w_hh0"], np.float32)
    w_ih1 = np.asarray(inputs["w_ih1"], np.float32)
    w_hh1 = np.asarray(inputs["w_hh1"], np.float32)
    b_ih0 = np.asarray(inputs["b_ih0"], np.float32)
    b_hh0 = np.asarray(inputs["b_hh0"], np.float32)
    b_ih1 = np.asarray(inputs["b_ih1"], np.float32)
    b_hh1 = np.asarray(inputs["b_hh1"], np.float32)

    def wT(w, kc):     # [2, G, I] -> [128, kc, 2, G]
        return np.ascontiguousarray(
            w.transpose(2, 0, 1).reshape(kc, 128, 2, G).transpose(1, 0, 2, 3)
        ).astype(f16)

    gmask_rz = (np.arange(G) < 2 * H)

    def bgi(b_ih, b_hh):   # [128, 2, 12, GC_NR]
        v = b_ih + np.where(gmask_rz[None, :], b_hh, 0.0)
        v = v.reshape(2, 12, 128).transpose(2, 0, 1)
        return np.ascontiguousarray(np.broadcast_to(
            v[:, :, :, None], (128, 2, 12, GC_NR))).astype(f16)

    def bhhnB(b_hh):   # [128, 2, 4, S, BPC] broadcast of the n-part
        bn = b_hh[:, 2 * H:].reshape(2, 4, 128).transpose(2, 0, 1)
        return np.ascontiguousarray(np.broadcast_to(
            bn[:, :, :, None, None], (128, 2, 4, S, BPC))).astype(f16)

    import ml_dtypes
    f8 = ml_dtypes.float8_e4m3
    # fc_w rows padded so shard 7's 4096-col window stays in-bounds
    fcw_pad = np.zeros((NC_ * VS + VSC * CH, 2 * H), np.float32)
    fcw_pad[:V] = fc_w

    # bias-pair lhsT: chunk ch's selector is 1.0 at [p=ch, i=0, :]
    e8 = np.zeros((128, 2, VSC, 128), np.float32)
    for ch in range(VSC):
        e8[ch, 0, ch, :] = 1.0

    bGiB = np.ascontiguousarray(np.stack(
        [bgi(b_ih0, b_hh0), bgi(b_ih1, b_hh1)], axis=1))

    shared = {
        "WihT0": wT(w_ih0, 4), "WhhT0": wT(w_hh0, 4),
        "WihT1": wT(w_ih1, 8), "WhhT1": wT(w_hh1, 4),
        "bGiB": bGiB,
        "bHhnB0": bhhnB(b_hh0), "bHhnB1": bhhnB(b_hh1),
        "e8": e8.astype(f8),
    }

    emb = emb_t[tgt]                      # [B, S, E]
    ctx4 = ctx.reshape(L, 2, B, H)        # [l, d, b, h]

    in_maps = []
    for c in range(NC_):
        bs = slice(BPC * c, BPC * (c + 1))
        er = emb[bs].transpose(1, 0, 2).reshape(R, E)   # row = 4t + b
        embTc = np.ascontiguousarray(
            er.T.reshape(4, 128, R).transpose(1, 0, 2)).astype(f16)
        cc = ctx4[:, :, bs, :]                          # [l, d, 4, h]
        h0Tc = np.ascontiguousarray(
            cc.transpose(3, 0, 1, 2).reshape(4, 128, L, 2, BPC)
            .transpose(1, 2, 3, 0, 4)).astype(f16)
        # vocab shard c: fc_w rows [VS*c, VS*c + VSC*CH), cols >= VS are
        # pad -> NEG bias so their exp vanishes from this core's partials
        wsh = fcw_pad[VS * c:VS * c + VSC * CH]
        fcw8 = np.ascontiguousarray(
            wsh.reshape(VSC, CH, 8, 128).transpose(0, 3, 2, 1)).astype(f8)
        bsh = np.full((VSC * CH,), NEG, np.float32)
        bsh[:VS] = fc_b[VS * c:VS * (c + 1)]
        biasT = np.zeros((128, 2, CH), np.float32)
        biasT[:VSC, 0, :] = bsh.reshape(VSC, CH)
        m = {"embT": embTc, "h0T": h0Tc,
             "fcw8": fcw8, "biasT": biasT.astype(f8)}
        m.update(shared)
        in_maps.append(m)
    return in_maps


def _unshard(results):
    full = np.empty((B, S, V), np.float32)
    for k in range(NC_):
        o = np.asarray(results[k]["out"], np.float32)   # [src, 4t+b, vv]
        o = o.reshape(NC_, S, BPC, VS).transpose(0, 2, 1, 3)
        full[:, :, VS * k:VS * (k + 1)] = o.reshape(B, S, VS)
    return full


def kernel(**inputs):
    from concourse.bass_utils import run_bass_kernel_spmd
    nc = _get_nc()
    in_maps = _prep_inputs(inputs)
    res = run_bass_kernel_spmd(nc, in_maps, core_ids=list(range(NC_)))
    return _unshard(res.results)



# revision 85
# speedup vs baseline: 1.4019x; 1.0716x over previous
"""Trainium2 Bass kernel for nn_PlainDecoder (2-layer 2-direction GRU decoder
+ vocab projection + log_softmax), vocab-parallel across 8 NeuronCores.

Sharding:
  - GRU scan: data-parallel over batch (32 batches -> 4 per core). Each core
    runs both "directions" of both layers for its 4 batches.
  - Logits/log_softmax: VOCAB-parallel. As each 32-step quarter of the scan
    finishes, its fp8 output is AllGathered (DRAM bounce) so every core
    holds all 4096 (t, b) rows; each core then computes its resident
    4000-wide fc_w shard (fp8 DoubleRow matmuls: two 128-contractions per
    instruction at 0.5 cycles/row, bias folded in as a 5th pair) for all
    rows. Per row-group, the per-shard exp-sums are exchanged with a small
    AllGather and summed locally; log_softmax subtracts ln(S) as pieces
    stream out. Collective latency hides behind the compute pipeline.

Scan layout (gate-major / weight-stationary): the recurrent gate matmuls put
the GATE dim on PSUM partitions: lhsT = Whh^T chunk [128 (h-sub), 128 (gate
sub)], rhs = h^T [128 (h-sub), 4 (batch)]. The output h'^T lands directly in
the [h-sub partitions, batch] layout the next step's rhs needs -> zero
transposes. Precomputed gi (+ fused biases) ride into PSUM via identity-
matmul injections; b_hh's n-part is (t, b)-constant and injects from a tiny
resident tile. Histories and the gi buffers are 32-step rings, freeing SBUF
for the resident fc_w shard. The two layer scans are software-pipelined
(layer 1 lags layer 0 by LAG slots) with both layers' gi built chunk-by-
chunk on spare PE cycles inside the slot loop.

The scan state is float16 (fp16 matmuls run at 1 cycle/row at any p-state,
and fp16's 10-bit mantissa keeps the recurrent rounding walk ~8x below
bf16's); everything on the logits path is fp8-e4m3.
"""

import os
import sys
from contextlib import ExitStack

for _p in ("/opt/trn_rl_repo", "/root/.axon_site/_ro/trn_rl_repo"):
    if os.path.isdir(_p) and _p not in sys.path:
        sys.path.insert(0, _p)

import numpy as np  # noqa: E402

V, E, H, L, B, S = 32000, 512, 512, 2, 32, 128
NC_ = 8                      # cores
BPC = B // NC_               # batches per core = 4
R = BPC * S                  # rows per core = 512 (row = 4*t + b)
G = 3 * H                    # 1536 gates per direction
CH = 512                     # vocab chunk width
VS = V // NC_                # per-core vocab shard = 4000
VSC = 8                      # chunks per shard -> 4096 padded
NEG = -80.0                  # pad bias -> exp() ~ 0
GCH = 8                      # gi chunk = 8 timesteps
GC_NR = GCH * BPC            # rows per gi chunk

_BUILT = {}


def _build_nc(T=S, n_cores=NC_, sim=False, nblk_lim=None, skip_gi=False):
    """Build the Bass program (same NEFF for all cores; per-core data only).

    sim is accepted for compatibility; the program has no collectives so the
    TimelineSim build is identical.
    """
    import concourse.bass as bass  # noqa: F401
    import concourse.mybir as mybir
    import concourse.tile as tile
    from concourse import bacc
    from concourse.masks import make_identity

    dt = mybir.dt
    f32 = dt.float32
    f16 = dt.float16
    AF = mybir.ActivationFunctionType
    OP = mybir.AluOpType
    AX = mybir.AxisListType
    DR = mybir.MatmulPerfMode.DoubleRow

    nc = bacc.Bacc("TRN2", target_bir_lowering=False, debug=False,
                   num_devices=n_cores)

    TR = BPC * T               # rows actually scanned

    # ---------------- DRAM I/O ----------------
    embT = nc.dram_tensor("embT", [128, 4, R], f16, kind="ExternalInput")
    h0T = nc.dram_tensor("h0T", [128, 2, 2, 4, BPC], f16,
                         kind="ExternalInput")          # (p, l, d, k, b)
    WhhT0 = nc.dram_tensor("WhhT0", [128, 4, 2, G], f16, kind="ExternalInput")
    WhhT1 = nc.dram_tensor("WhhT1", [128, 4, 2, G], f16, kind="ExternalInput")
    WihT0 = nc.dram_tensor("WihT0", [128, 4, 2, G], f16, kind="ExternalInput")
    WihT1 = nc.dram_tensor("WihT1", [128, 8, 2, G], f16, kind="ExternalInput")
    # b_ih (+b_hh for r/z) broadcast over a gi-chunk's (t, b) columns:
    # rides each gi chunk's PSUM as one identity-matmul injection
    bGiB = nc.dram_tensor("bGiB", [128, 2, 2, 12, GC_NR], f16,
                          kind="ExternalInput")
    # b_hh n-part, broadcast over (t, b): injected into the n-gate PSUM.
    bHhnB0 = nc.dram_tensor("bHhnB0", [128, 2, 4, S, BPC], f16,
                            kind="ExternalInput")
    bHhnB1 = nc.dram_tensor("bHhnB1", [128, 2, 4, S, BPC], f16,
                            kind="ExternalInput")
    f8 = dt.float8e4
    # Vocab-parallel logits: this core holds fc_w rows [VS*k, VS*k + VSC*CH)
    # (zero-padded), fp8-e4m3.  [chunk, p, h-tile, col]
    fcw8_d = nc.dram_tensor("fcw8", [VSC, 128, 8, CH], f8,
                            kind="ExternalInput")
    # fc_b shard as the rhs of a 5th DoubleRow pair: chunk ch's bias lives
    # on partition ch (pad cols carry NEG so their exp vanishes)
    biasT_d = nc.dram_tensor("biasT", [128, 2, CH], f8,
                             kind="ExternalInput")
    # lhsT of the bias pair for chunk ch: 1.0 at [p=ch, i=0, :], else 0
    e8_d = nc.dram_tensor("e8", [128, 2, VSC, 128], f8,
                          kind="ExternalInput")

    # this core's vocab slice for ALL rows: [src_core, row, vs]
    out_d = nc.dram_tensor("out", [NC_, R, VS], f16, kind="ExternalOutput")

    with tile.TileContext(nc) as tc, ExitStack() as top:
        constp = top.enter_context(tc.tile_pool(name="const", bufs=1))
        ident = constp.tile([128, 128], f16)
        make_identity(nc, ident[:])
        h0T_sb = constp.tile([128, 2, 2, 4, BPC], f16)
        nc.sync.dma_start(h0T_sb[:], h0T[:])
        # b_hh n-part is constant over (t, b): one tiny tile per layer
        # feeds every step's pn injection (no ring traffic at all)
        bnT = constp.tile([128, 2, 2, 4, BPC], f16, tag="bnT")
        nc.sync.dma_start(bnT[:, 0], bHhnB0[:, :, :, 0, :])
        nc.sync.dma_start(bnT[:, 1], bHhnB1[:, :, :, 0, :])

        # resident logits operands (fcw8 itself loads post-scan into the
        # space the scan frees); gathered fp8 scan outputs
        gathp = top.enter_context(tc.tile_pool(name="gath", bufs=1))
        biasT = gathp.tile([128, 2, CH], f8, tag="biasT")
        e8 = gathp.tile([128, 2, VSC, 128], f8, tag="e8")
        # first two fc_w chunks load during the scan so the first logits
        # blocks start immediately; the rest load into freed scan space
        fcw8a = gathp.tile([128, 2, 8, CH], f8, tag="fcw8a")
        # all cores' scan outputs: [p, h-tile, src_core, local row (4t+b)],
        # one tile per quarter so block reads only dep on their own gather
        QR = R // 4
        x2all = [gathp.tile([128, 8, NC_, QR], f8, name=f"x2all{q}")
                 for q in range(4)]
        # DRAM bounce buffers for the x2 all-gathers + sum all-reduces
        dramp = top.enter_context(tc.tile_pool(name="dram", bufs=1,
                                               space="DRAM"))
        x2qi = [dramp.tile([128, 8, QR], f8, name=f"x2qi{q}")
                for q in range(4)]
        x2qo = [dramp.tile([NC_, 128, 8, QR], f8, name=f"x2qo{q}")
                for q in range(4)]
        ari = [dramp.tile([128, 8], f32, name=f"ari{g}") for g in range(6)]
        aro = [dramp.tile([NC_, 128, 8], f32, name=f"aro{g}")
               for g in range(6)]

        with ExitStack() as scan_stack:
            wres = scan_stack.enter_context(tc.tile_pool(name="wres", bufs=1))
            gip = scan_stack.enter_context(tc.tile_pool(name="gip", bufs=1))
            hist1 = scan_stack.enter_context(tc.tile_pool(name="hist1",
                                                          bufs=1))
            # histories are 32-step rings (slot = t % RW): the recurrence
            # only needs t-1, gi1 builds lag <= 12 slots, and x2 is cast to
            # fp8 + staged to DRAM the moment each 32-step quarter finishes
            RW = 32
            x2T = hist1.tile([128, 2, 4, RW, BPC], f16, tag="x2T")
            x2f8 = hist1.tile([128, 2, 4, RW, BPC], f8, tag="x2f8")

            # One SHARED gi ring for both layers (32-step window, slot =
            # t % RW): L0 consumes slot t at slot t, and gi1's chunk for
            # steps [t0, t0+GCH) is only written after L0's reads of those
            # slots — so layer 1's gi overwrites layer 0's in place, and
            # the ring reuses each slot every 32 steps with ~9 slots of
            # margin. Both gi builds run chunked inside the slot loop.
            giRZB = gip.tile([128, 2, 8, 32, BPC], f16, tag="giRZB")
            giN = gip.tile([128, 2, 4, 32, BPC], f16, tag="giN")
            giR = [giRZB, giRZB]
            giNl = [giN, giN]
            x1T = hist1.tile([128, 2, 4, RW, BPC], f16, tag="x1T")
            hists = [x1T, x2T]

            # input DMAs ordered by first use: gi chunks 0/1 (emb, wih0,
            # bias), L0 step 0 (b_hh0, whh0), then gi1/L1 weights
            embT_sb = wres.tile([128, 4, R], f16, tag="embT")
            nc.sync.dma_start(embT_sb[:], embT[:])
            bgiB = wres.tile([128, 2, 2, 12, GC_NR], f16, tag="bgiB")
            nc.sync.dma_start(bgiB[:], bGiB[:])
            wih0 = wres.tile([128, 4, 2, G], f16, tag="wih0")
            for d in range(2):
                nc.sync.dma_start(wih0[:, :, d, :], WihT0[:, :, d, :])
            whh0 = wres.tile([128, 4, 2, G], f16, tag="whh0")
            # split by gate range so step 0's r-group matmuls start as
            # soon as the first third lands
            for j3 in range(3):
                nc.sync.dma_start(whh0[:, :, :, 512 * j3:512 * (j3 + 1)],
                                  WhhT0[:, :, :, 512 * j3:512 * (j3 + 1)])
            wih1 = wres.tile([128, 8, 2, G], f16, tag="wih1")
            for d in range(2):
                nc.sync.dma_start(wih1[:, :, d, :], WihT1[:, :, d, :])
            whh1 = wres.tile([128, 4, 2, G], f16, tag="whh1")
            nc.sync.dma_start(whh1[:], WhhT1[:])
            whhs = [whh0, whh1]
            # resident logits operands ride the idle DMA behind the scan's
            # critical loads (the gi ring freed the SBUF fcw8 needs)
            nc.sync.dma_start(biasT[:], biasT_d[:])
            nc.sync.dma_start(e8[:], e8_d[:])
            for ch in range(2):
                nc.sync.dma_start(fcw8a[:, ch], fcw8_d[ch])

            # ---------- pipelined two-layer scan ----------
            # LAG >= 8 is a hard correctness bound: gi1's chunk for steps
            # [8c, 8c+8) is emitted at slot 8c+8, and layer 1's step 8c is
            # emitted at slot 8c+LAG.  With LAG < 8 the consumer is emitted
            # BEFORE the gi1 write exists, so the dependency tracker lets
            # layer 1 read layer 0's gi values from the shared ring.
            LAG = 10
            with (
                tc.tile_pool(name="spr", bufs=2, space="PSUM") as przp,
                tc.tile_pool(name="spn", bufs=2, space="PSUM") as pnp,
                tc.tile_pool(name="spz", bufs=2, space="PSUM") as pzp,
                tc.tile_pool(name="gcp", bufs=2, space="PSUM") as gcp,
                tc.tile_pool(name="sch", bufs=8) as chp,
            ):
                def mk_step(l, t):
                    # split one step's emission into phases so the two
                    # layers' chains can be phase-shifted half a slot: the
                    # in-order ACT/DVE queues then serve L0's and L1's ops
                    # in the order they actually become ready
                    whh = whhs[l]
                    histT = hists[l]
                    gR = giR[l]
                    st = {}

                    def rhs(d, k):
                        if t == 0:
                            return h0T_sb[:, l, d, k, :]
                        return histT[:, d, k, (t - 1) % RW, :]

                    def gate_group(pool, jlo, jhi, nm):
                        ps = pool.tile([128, 2, jhi - jlo, BPC], f32,
                                       tag=nm, name=f"{nm}{l}")
                        for d in range(2):
                            inj = (bnT[:, l, d] if jlo == 8
                                   else gR[:, d, jlo:jhi, t % RW, :])
                            nc.tensor.matmul(
                                ps[:, d, :, :], ident[:], inj,
                                start=(d == 0), stop=False)
                            for j in range(jlo, jhi):
                                for k in range(4):
                                    nc.tensor.matmul(
                                        ps[:, d, j - jlo, :],
                                        whh[:, k, d, 128 * j:128 * (j + 1)],
                                        rhs(d, k), start=False,
                                        stop=(d == 1 and j == jhi - 1
                                              and k == 3))
                        return ps

                    def pe():
                        st["pr"] = gate_group(przp, 0, 4, "pr")
                        st["pn"] = gate_group(pnp, 8, 12, "pn")
                        st["pz"] = gate_group(pzp, 4, 8, "pz")

                    def sig_r():
                        st["rp"] = chp.tile([128, 2, 3, 4, BPC], f32,
                                            tag="rp", name=f"rp{l}")
                        nc.scalar.activation(st["rp"][:, :, 0], st["pr"][:],
                                             AF.Sigmoid)

                    def n1_muladd():
                        st["n1"] = chp.tile([128, 2, 4, BPC], f32,
                                            tag="n1", name=f"n1{l}")
                        nc.vector.tensor_mul(st["n1"][:], st["pn"][:],
                                             st["rp"][:, :, 0])
                        nc.vector.tensor_add(st["n1"][:], st["n1"][:],
                                             giNl[l][:, :, :, t % RW, :])

                    def sig_z():
                        nc.scalar.activation(st["rp"][:, :, 1], st["pz"][:],
                                             AF.Sigmoid)

                    def tanh():
                        nc.scalar.activation(st["rp"][:, :, 2], st["n1"][:],
                                             AF.Tanh)

                    def ozm1():
                        st["oz"] = chp.tile([128, 2, 4, BPC], f32,
                                            tag="oz", name=f"oz{l}")
                        st["m1"] = chp.tile([128, 2, 4, BPC], f32,
                                            tag="m1", name=f"m1{l}")
                        hprev = (h0T_sb[:, l] if t == 0
                                 else histT[:, :, :, (t - 1) % RW, :])
                        nc.vector.tensor_scalar(st["oz"][:],
                                                st["rp"][:, :, 1],
                                                -1.0, 1.0, OP.mult, OP.add)
                        nc.vector.tensor_mul(st["m1"][:],
                                             st["rp"][:, :, 1], hprev)

                    def tail():
                        m2 = chp.tile([128, 2, 4, BPC], f32, tag="m2",
                                      name=f"m2{l}")
                        nc.vector.tensor_mul(m2[:], st["oz"][:],
                                             st["rp"][:, :, 2])
                        nc.vector.tensor_add(histT[:, :, :, t % RW, :],
                                             st["m1"][:], m2[:])
                    return pe, sig_r, n1_muladd, sig_z, tanh, ozm1, tail

                def emit_slot_steps(steps):
                    ph = {l: mk_step(l, t) for l, t in steps}
                    if 0 in ph and 1 in ph:
                        pe0, sr0, nm0, sz0, th0, oz0, tl0 = ph[0]
                        pe1, sr1, nm1, sz1, th1, oz1, tl1 = ph[1]
                        pe0(); sr0(); pe1()
                        nm0(); sz0(); sr1()
                        th0(); oz0(); nm1()
                        sz1(); tl0(); th1()
                        oz1(); tl1()
                    else:
                        for l in ph:
                            pe, sr, nm, sz, th, oz, tl = ph[l]
                            pe(); sr(); nm(); sz(); th(); oz(); tl()

                def gi_chunk(l, c):
                    t0, t1 = GCH * c, GCH * (c + 1)
                    nr = GC_NR
                    w0 = t0 % RW
                    wih, kc = (wih0, 4) if l == 0 else (wih1, 8)
                    for d in range(2):
                        for grp, (j0, nj) in enumerate([(0, 8), (8, 4)]):
                            ps = gcp.tile([128, nj, nr], f32, tag="gc",
                                          name=f"gc{grp}")
                            # bias seeds the whole group (zeroes the bank)
                            nc.tensor.matmul(
                                ps[:], ident[:],
                                bgiB[:, l, d, j0:j0 + nj, :],
                                start=True, stop=False)
                            for jj in range(nj):
                                j = j0 + jj
                                for kk in range(kc):
                                    if l == 0:
                                        xs = embT_sb[:, kk,
                                                     BPC * t0:BPC * t1]
                                    else:
                                        r0 = t0 % RW
                                        xs = x1T[:, kk // 4, kk % 4,
                                                 r0:r0 + GCH, :]
                                        xs = xs.rearrange("p t b -> p (t b)")
                                    nc.tensor.matmul(
                                        ps[:, jj, :],
                                        wih[:, kk, d, 128 * j:128 * (j + 1)],
                                        xs, start=False,
                                        stop=(jj == nj - 1 and kk == kc - 1))
                            if grp == 0:
                                dst = giRZB[:, d, 0:8, w0:w0 + GCH, :]
                            else:
                                dst = giN[:, d, :, w0:w0 + GCH, :]
                            dst = dst.rearrange("p j t b -> p j (t b)")
                            if (2 * d + grp) % 2 == 0:
                                nc.scalar.copy(dst, ps[:])
                            else:
                                nc.vector.tensor_copy(dst, ps[:])

                NGC = (T + GCH - 1) // GCH
                gi_chunk(0, 0)
                gi_chunk(0, 1)
                for s in range(T + LAG):
                    steps = []
                    if s < T:
                        steps.append((0, s))
                    if s >= LAG:
                        steps.append((1, s - LAG))
                    emit_slot_steps(steps)
                    if (s + 5) % GCH == 0 and 2 <= (s + 5) // GCH < NGC:
                        gi_chunk(0, (s + 5) // GCH)
                    if s >= 8 and (s - 8) % GCH == 0 and (s - 8) // GCH < NGC:
                        gi_chunk(1, (s - 8) // GCH)
                    if s >= LAG and (s - LAG + 1) % 8 == 0:
                        # cast each finished 8-step eighth to fp8 as it
                        # completes: small queue bubbles, and the ring WAR
                        # clears ~24 slots before the slot is rewritten
                        t0 = (s - LAG + 1) - 8
                        w = t0 % RW
                        csrc = x2T[:, :, :, w:w + GCH, :]
                        cdst = x2f8[:, :, :, w:w + GCH, :]
                        if (t0 // 8) % 2 == 0:
                            nc.scalar.copy(cdst, csrc)
                        else:
                            nc.vector.tensor_copy(cdst, csrc)
                    if s >= LAG and (s - LAG + 1) % 32 == 0:
                        # a quarter of the scan output is staged to DRAM,
                        # all-gathered, and scattered into x2all
                        q = (s - LAG + 1) // 32 - 1
                        nc.sync.dma_start(
                            x2qi[q][:],
                            x2f8[:].rearrange("p d k t b -> p (d k) (t b)"))
                        nc.gpsimd.collective_compute(
                            "AllGather", OP.bypass,
                            replica_groups=[list(range(NC_))],
                            ins=[x2qi[q].opt()], outs=[x2qo[q].opt()])
                        # gpsimd queue: this DMA waits ~41us for the
                        # collective, which would head-of-line-block the
                        # scan's bias DMAs if it sat on the SP sequencer
                        nc.gpsimd.dma_start(
                            x2all[q][:],
                            x2qo[q][:].rearrange("c p h r -> p h c r"))

        # ------- logits + log_softmax (vocab-parallel over the 8 cores) ----
        # Every core computes its VS-wide vocab shard for ALL 4096 rows.
        # Blocks: (quarter i, src core c) -> 128 rows of x2all.  Per block:
        # 4 chunk-pairs of 5 DoubleRow matmuls (4 x2 pairs + bias pair),
        # exp+accum straight off PSUM, PSUM->lt copies split DVE/Pool.
        # Per quarter: one [128, 8] partial-sum AllReduce; subtract + out
        # DMA of quarter g interleaves into quarter g+1's compute.
        ngrp = 4 if nblk_lim is None else min(nblk_lim, 4)
        with (
            tc.tile_pool(name="fwp", bufs=1) as fwp,
            tc.tile_pool(name="ltp", bufs=14) as ltp,
            tc.tile_pool(name="scrp", bufs=4) as scrp,
            tc.tile_pool(name="accp", bufs=16) as accp,
            tc.tile_pool(name="arp", bufs=2) as arp,
            tc.tile_pool(name="pcp", bufs=4) as pcp,
            tc.tile_pool(name="lpsp", bufs=4, space="PSUM") as lpsp,
        ):
            # remaining weight chunks land in freed scan space while the
            # first blocks chew on the early-loaded ch0/ch1 pairs
            fcw8b = fwp.tile([128, VSC - 2, 8, CH], f8, tag="fcw8b")
            for ch in range(2, VSC):
                nc.sync.dma_start(fcw8b[:, ch - 2], fcw8_d[ch])

            def fw_ch(ch):
                return fcw8a[:, ch] if ch < 2 else fcw8b[:, ch - 2]

            pend = []

            def emit_piece():
                if pend:
                    pend.pop(0)()

            def emit_block(q, c, arin, slot, nblk_):
                lt = ltp.tile([128, VSC, CH], f16, tag="lt")
                acc = accp.tile([128, 4], f32, tag="acc")
                for kp in range(4):
                    ps = lpsp.tile([128, 2, CH], f32, tag="lp")
                    for j in range(2):
                        fw = fw_ch(2 * kp + j)
                        for hp in range(4):
                            lhsT = x2all[q][:, 2 * hp:2 * hp + 2, c, :]
                            nc.tensor.matmul(
                                ps[:, j, :], lhsT,
                                fw[:, 2 * hp:2 * hp + 2, :],
                                start=(hp == 0), stop=False, perf_mode=DR)
                        nc.tensor.matmul(
                            ps[:, j, :], e8[:, :, 2 * kp + j, :],
                            biasT[:], start=False, stop=True, perf_mode=DR)
                    # exp of both chunks + row-sum accum, off PSUM
                    scr = scrp.tile([128, 2, CH], f16, tag="scr")
                    nc.scalar.activation(scr[:], ps[:], AF.Exp,
                                         accum_out=acc[:, kp:kp + 1])
                    # copy l out of PSUM (GPSIMD cannot touch PSUM, so the
                    # copies split DVE/ACT; Pool gets SBUF-side subtracts)
                    dst = lt[:, 2 * kp:2 * kp + 2, :]
                    if kp == 3:
                        nc.scalar.copy(dst, ps[:])
                    else:
                        nc.vector.tensor_copy(dst, ps[:])
                nc.vector.tensor_reduce(arin[:, slot:slot + 1], acc[:],
                                        axis=AX.X, op=OP.add)
                return lt

            def finalize(g, q, blks, lts, last=False):
                # partial sums were AllGathered (15us constant beats the
                # AllReduce's 28us): sum the 8 cores' partials locally
                arb = arp.tile([128, 8, NC_], f32, tag="arb")
                nc.sync.dma_start(arb[:], aro[g][:].rearrange(
                    "c p b -> p b c"))
                ssum = arp.tile([128, 8], f32, tag="ssum")
                nc.vector.tensor_reduce(ssum[:], arb[:], axis=AX.X,
                                        op=OP.add)
                lnS = arp.tile([128, 8], f32, tag="lnS")
                nc.scalar.activation(lnS[:], ssum[:], AF.Ln)

                def piece(i, c, kp, q=q, lts=lts, lnS=lnS, last=last):
                    w = min(2 * CH, VS - 2 * CH * kp)
                    pc = pcp.tile([128, 2, CH], f16, tag="pc")
                    src = lts[i][:, 2 * kp:2 * kp + 2, :]
                    # Pool's subs are 3.6x DVE's: fine mid-phase (DVE is
                    # busy with copies), but the final drain runs on an
                    # otherwise-idle DVE
                    if kp % 2 == 0 and not last:
                        nc.gpsimd.tensor_scalar_sub(pc[:], src,
                                                    lnS[:, i:i + 1])
                    else:
                        nc.vector.tensor_scalar_sub(pc[:], src,
                                                    lnS[:, i:i + 1])
                    nc.sync.dma_start(
                        out_d[c, 128 * q:128 * (q + 1),
                              2 * CH * kp:2 * CH * kp + w],
                        pc[:].rearrange("p a b -> p (a b)")[:, 0:w])
                return [lambda i=i, c=c, kp=kp: piece(i, c, kp)
                        for i, c in enumerate(blks) for kp in range(4)]

            # groups of (quarter, src-core) blocks; the first is small so
            # the sum-exchange pipeline warms early, and the last is small
            # so the serial tail (exchange + drain) after the final blocks
            # is short
            groups = [(0, list(range(8))), (1, list(range(8))),
                      (2, list(range(8))), (3, list(range(8)))][:ngrp]
            nblk_ = 0
            for g, (q, blks) in enumerate(groups):
                arin = arp.tile([128, 8], f32, tag="arin", name=f"ari{g}")
                lts = []
                for slot, c in enumerate(blks):
                    lts.append(emit_block(q, c, arin, slot, nblk_))
                    nblk_ += 1
                    for _ in range(4):
                        emit_piece()
                if len(blks) < 8:
                    nc.vector.memset(arin[:, len(blks):], 0.0)
                nc.sync.dma_start(ari[g][:], arin[:])
                nc.gpsimd.collective_compute(
                    "AllGather", OP.bypass,
                    replica_groups=[list(range(NC_))],
                    ins=[ari[g].opt()], outs=[aro[g].opt()])
                pend.extend(finalize(g, q, blks, lts,
                                     last=(g == len(groups) - 1)))
            while pend:
                emit_piece()

    nc.compile()
    return nc


def _get_nc():
    if "nc" not in _BUILT:
        _BUILT["nc"] = _build_nc()
    return _BUILT["nc"]


def _prep_inputs(inputs):
    """Host-side shard + relayout. Returns in_maps for 8 cores."""
    f16 = np.float16

    tgt = np.asarray(inputs["target"])
    ctx = np.asarray(inputs["context"], np.float32)
    emb_t = np.asarray(inputs["embed_table"], np.float32)
    fc_w = np.asarray(inputs["fc_w"], np.float32)
    fc_b = np.asarray(inputs["fc_b"], np.float32)
    w_ih0 = np.asarray(inputs["w_ih0"], np.float32)
    w_hh0 = np.asarray(inputs["w_hh0"], np.float32)
    w_ih1 = np.asarray(inputs["w_ih1"], np.float32)
    w_hh1 = np.asarray(inputs["w_hh1"], np.float32)
    b_ih0 = np.asarray(inputs["b_ih0"], np.float32)
    b_hh0 = np.asarray(inputs["b_hh0"], np.float32)
    b_ih1 = np.asarray(inputs["b_ih1"], np.float32)
    b_hh1 = np.asarray(inputs["b_hh1"], np.float32)

    def wT(w, kc):     # [2, G, I] -> [128, kc, 2, G]
        return np.ascontiguousarray(
            w.transpose(2, 0, 1).reshape(kc, 128, 2, G).transpose(1, 0, 2, 3)
        ).astype(f16)

    gmask_rz = (np.arange(G) < 2 * H)

    def bgi(b_ih, b_hh):   # [128, 2, 12, GC_NR]
        v = b_ih + np.where(gmask_rz[None, :], b_hh, 0.0)
        v = v.reshape(2, 12, 128).transpose(2, 0, 1)
        return np.ascontiguousarray(np.broadcast_to(
            v[:, :, :, None], (128, 2, 12, GC_NR))).astype(f16)

    def bhhnB(b_hh):   # [128, 2, 4, S, BPC] broadcast of the n-part
        bn = b_hh[:, 2 * H:].reshape(2, 4, 128).transpose(2, 0, 1)
        return np.ascontiguousarray(np.broadcast_to(
            bn[:, :, :, None, None], (128, 2, 4, S, BPC))).astype(f16)

    import ml_dtypes
    f8 = ml_dtypes.float8_e4m3
    # fc_w rows padded so shard 7's 4096-col window stays in-bounds
    fcw_pad = np.zeros((NC_ * VS + VSC * CH, 2 * H), np.float32)
    fcw_pad[:V] = fc_w

    # bias-pair lhsT: chunk ch's selector is 1.0 at [p=ch, i=0, :]
    e8 = np.zeros((128, 2, VSC, 128), np.float32)
    for ch in range(VSC):
        e8[ch, 0, ch, :] = 1.0

    bGiB = np.ascontiguousarray(np.stack(
        [bgi(b_ih0, b_hh0), bgi(b_ih1, b_hh1)], axis=1))

    shared = {
        "WihT0": wT(w_ih0, 4), "WhhT0": wT(w_hh0, 4),
        "WihT1": wT(w_ih1, 8), "WhhT1": wT(w_hh1, 4),
        "bGiB": bGiB,
        "bHhnB0": bhhnB(b_hh0), "bHhnB1": bhhnB(b_hh1),
        "e8": e8.astype(f8),
    }

    emb = emb_t[tgt]                      # [B, S, E]
    ctx4 = ctx.reshape(L, 2, B, H)        # [l, d, b, h]

    in_maps = []
    for c in range(NC_):
        bs = slice(BPC * c, BPC * (c + 1))
        er = emb[bs].transpose(1, 0, 2).reshape(R, E)   # row = 4t + b
        embTc = np.ascontiguousarray(
            er.T.reshape(4, 128, R).transpose(1, 0, 2)).astype(f16)
        cc = ctx4[:, :, bs, :]                          # [l, d, 4, h]
        h0Tc = np.ascontiguousarray(
            cc.transpose(3, 0, 1, 2).reshape(4, 128, L, 2, BPC)
            .transpose(1, 2, 3, 0, 4)).astype(f16)
        # vocab shard c: fc_w rows [VS*c, VS*c + VSC*CH), cols >= VS are
        # pad -> NEG bias so their exp vanishes from this core's partials
        wsh = fcw_pad[VS * c:VS * c + VSC * CH]
        fcw8 = np.ascontiguousarray(
            wsh.reshape(VSC, CH, 8, 128).transpose(0, 3, 2, 1)).astype(f8)
        bsh = np.full((VSC * CH,), NEG, np.float32)
        bsh[:VS] = fc_b[VS * c:VS * (c + 1)]
        biasT = np.zeros((128, 2, CH), np.float32)
        biasT[:VSC, 0, :] = bsh.reshape(VSC, CH)
        m = {"embT": embTc, "h0T": h0Tc,
             "fcw8": fcw8, "biasT": biasT.astype(f8)}
        m.update(shared)
        in_maps.append(m)
    return in_maps


def _unshard(results):
    full = np.empty((B, S, V), np.float32)
    for k in range(NC_):
        o = np.asarray(results[k]["out"], np.float32)   # [src, 4t+b, vv]
        o = o.reshape(NC_, S, BPC, VS).transpose(0, 2, 1, 3)
        full[:, :, VS * k:VS * (k + 1)] = o.reshape(B, S, VS)
    return full


def kernel(**inputs):
    from concourse.bass_utils import run_bass_kernel_spmd
    nc = _get_nc()
    in_maps = _prep_inputs(inputs)
    res = run_bass_kernel_spmd(nc, in_maps, core_ids=list(range(NC_)))
    return _unshard(res.results)



# revision 86
# speedup vs baseline: 1.4139x; 1.0086x over previous
"""Trainium2 Bass kernel for nn_PlainDecoder (2-layer 2-direction GRU decoder
+ vocab projection + log_softmax), vocab-parallel across 8 NeuronCores.

Sharding:
  - GRU scan: data-parallel over batch (32 batches -> 4 per core). Each core
    runs both "directions" of both layers for its 4 batches.
  - Logits/log_softmax: VOCAB-parallel. As each 32-step quarter of the scan
    finishes, its fp8 output is AllGathered (DRAM bounce) so every core
    holds all 4096 (t, b) rows; each core then computes its resident
    4000-wide fc_w shard (fp8 DoubleRow matmuls: two 128-contractions per
    instruction at 0.5 cycles/row, bias folded in as a 5th pair) for all
    rows. Per row-group, the per-shard exp-sums are exchanged with a small
    AllGather and summed locally; log_softmax subtracts ln(S) as pieces
    stream out. Collective latency hides behind the compute pipeline.

Scan layout (gate-major / weight-stationary): the recurrent gate matmuls put
the GATE dim on PSUM partitions: lhsT = Whh^T chunk [128 (h-sub), 128 (gate
sub)], rhs = h^T [128 (h-sub), 4 (batch)]. The output h'^T lands directly in
the [h-sub partitions, batch] layout the next step's rhs needs -> zero
transposes. Precomputed gi (+ fused biases) ride into PSUM via identity-
matmul injections; b_hh's n-part is (t, b)-constant and injects from a tiny
resident tile. Histories and the gi buffers are 32-step rings, freeing SBUF
for the resident fc_w shard. The two layer scans are software-pipelined
(layer 1 lags layer 0 by LAG slots) with both layers' gi built chunk-by-
chunk on spare PE cycles inside the slot loop.

The scan state is float16 (fp16 matmuls run at 1 cycle/row at any p-state,
and fp16's 10-bit mantissa keeps the recurrent rounding walk ~8x below
bf16's); everything on the logits path is fp8-e4m3.
"""

import os
import sys
from contextlib import ExitStack

for _p in ("/opt/trn_rl_repo", "/root/.axon_site/_ro/trn_rl_repo"):
    if os.path.isdir(_p) and _p not in sys.path:
        sys.path.insert(0, _p)

import numpy as np  # noqa: E402

V, E, H, L, B, S = 32000, 512, 512, 2, 32, 128
NC_ = 8                      # cores
BPC = B // NC_               # batches per core = 4
R = BPC * S                  # rows per core = 512 (row = 4*t + b)
G = 3 * H                    # 1536 gates per direction
CH = 512                     # vocab chunk width
VS = V // NC_                # per-core vocab shard = 4000
VSC = 8                      # chunks per shard -> 4096 padded
NEG = -80.0                  # pad bias -> exp() ~ 0
GCH = 8                      # gi chunk = 8 timesteps
GC_NR = GCH * BPC            # rows per gi chunk

_BUILT = {}


def _build_nc(T=S, n_cores=NC_, sim=False, nblk_lim=None, skip_gi=False):
    """Build the Bass program (same NEFF for all cores; per-core data only).

    sim is accepted for compatibility; the program has no collectives so the
    TimelineSim build is identical.
    """
    import concourse.bass as bass  # noqa: F401
    import concourse.mybir as mybir
    import concourse.tile as tile
    from concourse import bacc
    from concourse.masks import make_identity

    dt = mybir.dt
    f32 = dt.float32
    f16 = dt.float16
    AF = mybir.ActivationFunctionType
    OP = mybir.AluOpType
    AX = mybir.AxisListType
    DR = mybir.MatmulPerfMode.DoubleRow

    nc = bacc.Bacc("TRN2", target_bir_lowering=False, debug=False,
                   num_devices=n_cores)

    TR = BPC * T               # rows actually scanned

    # ---------------- DRAM I/O ----------------
    embT = nc.dram_tensor("embT", [128, 4, R], f16, kind="ExternalInput")
    h0T = nc.dram_tensor("h0T", [128, 2, 2, 4, BPC], f16,
                         kind="ExternalInput")          # (p, l, d, k, b)
    WhhT0 = nc.dram_tensor("WhhT0", [128, 4, 2, G], f16, kind="ExternalInput")
    WhhT1 = nc.dram_tensor("WhhT1", [128, 4, 2, G], f16, kind="ExternalInput")
    WihT0 = nc.dram_tensor("WihT0", [128, 4, 2, G], f16, kind="ExternalInput")
    WihT1 = nc.dram_tensor("WihT1", [128, 8, 2, G], f16, kind="ExternalInput")
    # b_ih (+b_hh for r/z) broadcast over a gi-chunk's (t, b) columns:
    # rides each gi chunk's PSUM as one identity-matmul injection
    bGiB = nc.dram_tensor("bGiB", [128, 2, 2, 12, GC_NR], f16,
                          kind="ExternalInput")
    # b_hh n-part, broadcast over (t, b): injected into the n-gate PSUM.
    bHhnB0 = nc.dram_tensor("bHhnB0", [128, 2, 4, S, BPC], f16,
                            kind="ExternalInput")
    bHhnB1 = nc.dram_tensor("bHhnB1", [128, 2, 4, S, BPC], f16,
                            kind="ExternalInput")
    f8 = dt.float8e4
    # Vocab-parallel logits: this core holds fc_w rows [VS*k, VS*k + VSC*CH)
    # (zero-padded), fp8-e4m3.  [chunk, p, h-tile, col]
    fcw8_d = nc.dram_tensor("fcw8", [VSC, 128, 8, CH], f8,
                            kind="ExternalInput")
    # fc_b shard as the rhs of a 5th DoubleRow pair: chunk ch's bias lives
    # on partition ch (pad cols carry NEG so their exp vanishes)
    biasT_d = nc.dram_tensor("biasT", [128, 2, CH], f8,
                             kind="ExternalInput")
    # lhsT of the bias pair for chunk ch: 1.0 at [p=ch, i=0, :], else 0
    e8_d = nc.dram_tensor("e8", [128, 2, VSC, 128], f8,
                          kind="ExternalInput")

    # this core's vocab slice for ALL rows: [src_core, row, vs]
    out_d = nc.dram_tensor("out", [NC_, R, VS], f16, kind="ExternalOutput")

    with tile.TileContext(nc) as tc, ExitStack() as top:
        constp = top.enter_context(tc.tile_pool(name="const", bufs=1))
        ident = constp.tile([128, 128], f16)
        make_identity(nc, ident[:])
        h0T_sb = constp.tile([128, 2, 2, 4, BPC], f16)
        nc.sync.dma_start(h0T_sb[:], h0T[:])
        # b_hh n-part is constant over (t, b): one tiny tile per layer
        # feeds every step's pn injection (no ring traffic at all)
        bnT = constp.tile([128, 2, 2, 4, BPC], f16, tag="bnT")
        nc.sync.dma_start(bnT[:, 0], bHhnB0[:, :, :, 0, :])
        nc.sync.dma_start(bnT[:, 1], bHhnB1[:, :, :, 0, :])

        # resident logits operands (fcw8 itself loads post-scan into the
        # space the scan frees); gathered fp8 scan outputs
        gathp = top.enter_context(tc.tile_pool(name="gath", bufs=1))
        biasT = gathp.tile([128, 2, CH], f8, tag="biasT")
        e8 = gathp.tile([128, 2, VSC, 128], f8, tag="e8")
        # first two fc_w chunks load during the scan so the first logits
        # blocks start immediately; the rest load into freed scan space
        fcw8a = gathp.tile([128, 2, 8, CH], f8, tag="fcw8a")
        # all cores' scan outputs: [p, h-tile, src_core, local row (4t+b)],
        # one tile per quarter so block reads only dep on their own gather
        QR = R // 4
        x2all = [gathp.tile([128, 8, NC_, QR], f8, name=f"x2all{q}")
                 for q in range(4)]
        # DRAM bounce buffers for the x2 all-gathers + sum all-reduces
        dramp = top.enter_context(tc.tile_pool(name="dram", bufs=1,
                                               space="DRAM"))
        x2qi = [dramp.tile([128, 8, QR], f8, name=f"x2qi{q}")
                for q in range(4)]
        x2qo = [dramp.tile([NC_, 128, 8, QR], f8, name=f"x2qo{q}")
                for q in range(4)]
        ari = [dramp.tile([128, 8], f32, name=f"ari{g}") for g in range(6)]
        aro = [dramp.tile([NC_, 128, 8], f32, name=f"aro{g}")
               for g in range(6)]

        with ExitStack() as scan_stack:
            wres = scan_stack.enter_context(tc.tile_pool(name="wres", bufs=1))
            gip = scan_stack.enter_context(tc.tile_pool(name="gip", bufs=1))
            hist1 = scan_stack.enter_context(tc.tile_pool(name="hist1",
                                                          bufs=1))
            # histories are 32-step rings (slot = t % RW): the recurrence
            # only needs t-1, gi1 builds lag <= 12 slots, and x2 is cast to
            # fp8 + staged to DRAM the moment each 32-step quarter finishes
            RW = 32
            x2T = hist1.tile([128, 2, 4, RW, BPC], f16, tag="x2T")
            x2f8 = hist1.tile([128, 2, 4, RW, BPC], f8, tag="x2f8")

            # One SHARED gi ring for both layers (32-step window, slot =
            # t % RW): L0 consumes slot t at slot t, and gi1's chunk for
            # steps [t0, t0+GCH) is only written after L0's reads of those
            # slots — so layer 1's gi overwrites layer 0's in place, and
            # the ring reuses each slot every 32 steps with ~9 slots of
            # margin. Both gi builds run chunked inside the slot loop.
            giRZB = gip.tile([128, 2, 8, 32, BPC], f16, tag="giRZB")
            giN = gip.tile([128, 2, 4, 32, BPC], f16, tag="giN")
            giR = [giRZB, giRZB]
            giNl = [giN, giN]
            x1T = hist1.tile([128, 2, 4, RW, BPC], f16, tag="x1T")
            hists = [x1T, x2T]

            # input DMAs ordered by first use: gi chunks 0/1 (emb, wih0,
            # bias), L0 step 0 (b_hh0, whh0), then gi1/L1 weights
            embT_sb = wres.tile([128, 4, R], f16, tag="embT")
            nc.sync.dma_start(embT_sb[:], embT[:])
            bgiB = wres.tile([128, 2, 2, 12, GC_NR], f16, tag="bgiB")
            nc.sync.dma_start(bgiB[:], bGiB[:])
            wih0 = wres.tile([128, 4, 2, G], f16, tag="wih0")
            for d in range(2):
                nc.sync.dma_start(wih0[:, :, d, :], WihT0[:, :, d, :])
            whh0 = wres.tile([128, 4, 2, G], f16, tag="whh0")
            # split by gate range so step 0's r-group matmuls start as
            # soon as the first third lands
            for j3 in range(3):
                nc.sync.dma_start(whh0[:, :, :, 512 * j3:512 * (j3 + 1)],
                                  WhhT0[:, :, :, 512 * j3:512 * (j3 + 1)])
            wih1 = wres.tile([128, 8, 2, G], f16, tag="wih1")
            for d in range(2):
                nc.sync.dma_start(wih1[:, :, d, :], WihT1[:, :, d, :])
            whh1 = wres.tile([128, 4, 2, G], f16, tag="whh1")
            nc.sync.dma_start(whh1[:], WhhT1[:])
            whhs = [whh0, whh1]
            # resident logits operands ride the idle DMA behind the scan's
            # critical loads (the gi ring freed the SBUF fcw8 needs)
            nc.sync.dma_start(biasT[:], biasT_d[:])
            nc.sync.dma_start(e8[:], e8_d[:])
            for ch in range(2):
                nc.sync.dma_start(fcw8a[:, ch], fcw8_d[ch])

            # ---------- pipelined two-layer scan ----------
            # LAG >= 8 is a hard correctness bound: gi1's chunk for steps
            # [8c, 8c+8) is emitted at slot 8c+8, and layer 1's step 8c is
            # emitted at slot 8c+LAG.  With LAG < 8 the consumer is emitted
            # BEFORE the gi1 write exists, so the dependency tracker lets
            # layer 1 read layer 0's gi values from the shared ring.
            LAG = 10
            with (
                tc.tile_pool(name="spr", bufs=2, space="PSUM") as przp,
                tc.tile_pool(name="spn", bufs=2, space="PSUM") as pnp,
                tc.tile_pool(name="spz", bufs=2, space="PSUM") as pzp,
                tc.tile_pool(name="gcp", bufs=2, space="PSUM") as gcp,
                tc.tile_pool(name="sch", bufs=8) as chp,
            ):
                def mk_step(l, t):
                    # split one step's emission into phases so the two
                    # layers' chains can be phase-shifted half a slot: the
                    # in-order ACT/DVE queues then serve L0's and L1's ops
                    # in the order they actually become ready
                    whh = whhs[l]
                    histT = hists[l]
                    gR = giR[l]
                    st = {}

                    def rhs(d, k):
                        if t == 0:
                            return h0T_sb[:, l, d, k, :]
                        return histT[:, d, k, (t - 1) % RW, :]

                    def gate_group(pool, jlo, jhi, nm):
                        ps = pool.tile([128, 2, jhi - jlo, BPC], f32,
                                       tag=nm, name=f"{nm}{l}")
                        for d in range(2):
                            inj = (bnT[:, l, d] if jlo == 8
                                   else gR[:, d, jlo:jhi, t % RW, :])
                            nc.tensor.matmul(
                                ps[:, d, :, :], ident[:], inj,
                                start=(d == 0), stop=False)
                            for j in range(jlo, jhi):
                                for k in range(4):
                                    nc.tensor.matmul(
                                        ps[:, d, j - jlo, :],
                                        whh[:, k, d, 128 * j:128 * (j + 1)],
                                        rhs(d, k), start=False,
                                        stop=(d == 1 and j == jhi - 1
                                              and k == 3))
                        return ps

                    def pe():
                        st["pr"] = gate_group(przp, 0, 4, "pr")
                        st["pn"] = gate_group(pnp, 8, 12, "pn")
                        st["pz"] = gate_group(pzp, 4, 8, "pz")

                    def sig_r():
                        st["rp"] = chp.tile([128, 2, 3, 4, BPC], f32,
                                            tag="rp", name=f"rp{l}")
                        nc.scalar.activation(st["rp"][:, :, 0], st["pr"][:],
                                             AF.Sigmoid)

                    def n1_muladd():
                        st["n1"] = chp.tile([128, 2, 4, BPC], f32,
                                            tag="n1", name=f"n1{l}")
                        nc.vector.tensor_mul(st["n1"][:], st["pn"][:],
                                             st["rp"][:, :, 0])
                        nc.vector.tensor_add(st["n1"][:], st["n1"][:],
                                             giNl[l][:, :, :, t % RW, :])

                    def sig_z():
                        nc.scalar.activation(st["rp"][:, :, 1], st["pz"][:],
                                             AF.Sigmoid)

                    def tanh():
                        nc.scalar.activation(st["rp"][:, :, 2], st["n1"][:],
                                             AF.Tanh)

                    def ozm1():
                        st["oz"] = chp.tile([128, 2, 4, BPC], f32,
                                            tag="oz", name=f"oz{l}")
                        st["m1"] = chp.tile([128, 2, 4, BPC], f32,
                                            tag="m1", name=f"m1{l}")
                        hprev = (h0T_sb[:, l] if t == 0
                                 else histT[:, :, :, (t - 1) % RW, :])
                        nc.vector.tensor_scalar(st["oz"][:],
                                                st["rp"][:, :, 1],
                                                -1.0, 1.0, OP.mult, OP.add)
                        nc.vector.tensor_mul(st["m1"][:],
                                             st["rp"][:, :, 1], hprev)

                    def tail():
                        m2 = chp.tile([128, 2, 4, BPC], f32, tag="m2",
                                      name=f"m2{l}")
                        nc.vector.tensor_mul(m2[:], st["oz"][:],
                                             st["rp"][:, :, 2])
                        nc.vector.tensor_add(histT[:, :, :, t % RW, :],
                                             st["m1"][:], m2[:])
                    return pe, sig_r, n1_muladd, sig_z, tanh, ozm1, tail

                def emit_slot_steps(steps):
                    ph = {l: mk_step(l, t) for l, t in steps}
                    if 0 in ph and 1 in ph:
                        pe0, sr0, nm0, sz0, th0, oz0, tl0 = ph[0]
                        pe1, sr1, nm1, sz1, th1, oz1, tl1 = ph[1]
                        pe0(); sr0(); pe1()
                        nm0(); sz0(); sr1()
                        th0(); oz0(); nm1()
                        sz1(); tl0(); th1()
                        oz1(); tl1()
                    else:
                        for l in ph:
                            pe, sr, nm, sz, th, oz, tl = ph[l]
                            pe(); sr(); nm(); sz(); th(); oz(); tl()

                def gi_chunk(l, c):
                    t0, t1 = GCH * c, GCH * (c + 1)
                    nr = GC_NR
                    w0 = t0 % RW
                    wih, kc = (wih0, 4) if l == 0 else (wih1, 8)
                    for d in range(2):
                        for grp, (j0, nj) in enumerate([(0, 8), (8, 4)]):
                            ps = gcp.tile([128, nj, nr], f32, tag="gc",
                                          name=f"gc{grp}")
                            # bias seeds the whole group (zeroes the bank)
                            nc.tensor.matmul(
                                ps[:], ident[:],
                                bgiB[:, l, d, j0:j0 + nj, :],
                                start=True, stop=False)
                            for jj in range(nj):
                                j = j0 + jj
                                for kk in range(kc):
                                    if l == 0:
                                        xs = embT_sb[:, kk,
                                                     BPC * t0:BPC * t1]
                                    else:
                                        r0 = t0 % RW
                                        xs = x1T[:, kk // 4, kk % 4,
                                                 r0:r0 + GCH, :]
                                        xs = xs.rearrange("p t b -> p (t b)")
                                    nc.tensor.matmul(
                                        ps[:, jj, :],
                                        wih[:, kk, d, 128 * j:128 * (j + 1)],
                                        xs, start=False,
                                        stop=(jj == nj - 1 and kk == kc - 1))
                            if grp == 0:
                                dst = giRZB[:, d, 0:8, w0:w0 + GCH, :]
                            else:
                                dst = giN[:, d, :, w0:w0 + GCH, :]
                            dst = dst.rearrange("p j t b -> p j (t b)")
                            if (2 * d + grp) % 2 == 0:
                                nc.scalar.copy(dst, ps[:])
                            else:
                                nc.vector.tensor_copy(dst, ps[:])

                NGC = (T + GCH - 1) // GCH
                gi_chunk(0, 0)
                gi_chunk(0, 1)
                for s in range(T + LAG):
                    steps = []
                    if s < T:
                        steps.append((0, s))
                    if s >= LAG:
                        steps.append((1, s - LAG))
                    emit_slot_steps(steps)
                    if (s + 5) % GCH == 0 and 2 <= (s + 5) // GCH < NGC:
                        gi_chunk(0, (s + 5) // GCH)
                    if s >= 8 and (s - 8) % GCH == 0 and (s - 8) // GCH < NGC:
                        gi_chunk(1, (s - 8) // GCH)
                    if s >= LAG and (s - LAG + 1) % 8 == 0:
                        # cast each finished 8-step eighth to fp8 as it
                        # completes: small queue bubbles, and the ring WAR
                        # clears ~24 slots before the slot is rewritten
                        t0 = (s - LAG + 1) - 8
                        w = t0 % RW
                        csrc = x2T[:, :, :, w:w + GCH, :]
                        cdst = x2f8[:, :, :, w:w + GCH, :]
                        if (t0 // 8) % 2 == 0:
                            nc.scalar.copy(cdst, csrc)
                        else:
                            nc.vector.tensor_copy(cdst, csrc)
                    if s >= LAG and (s - LAG + 1) % 32 == 0:
                        # a quarter of the scan output is staged to DRAM,
                        # all-gathered, and scattered into x2all
                        q = (s - LAG + 1) // 32 - 1
                        nc.sync.dma_start(
                            x2qi[q][:],
                            x2f8[:].rearrange("p d k t b -> p (d k) (t b)"))
                        nc.gpsimd.collective_compute(
                            "AllGather", OP.bypass,
                            replica_groups=[list(range(NC_))],
                            ins=[x2qi[q].opt()], outs=[x2qo[q].opt()])
                        # gpsimd queue: this DMA waits ~41us for the
                        # collective, which would head-of-line-block the
                        # scan's bias DMAs if it sat on the SP sequencer
                        nc.gpsimd.dma_start(
                            x2all[q][:],
                            x2qo[q][:].rearrange("c p h r -> p h c r"))

        # ------- logits + log_softmax (vocab-parallel over the 8 cores) ----
        # Every core computes its VS-wide vocab shard for ALL 4096 rows.
        # Blocks: (quarter i, src core c) -> 128 rows of x2all.  Per block:
        # 4 chunk-pairs of 5 DoubleRow matmuls (4 x2 pairs + bias pair),
        # exp+accum straight off PSUM, PSUM->lt copies split DVE/Pool.
        # Per quarter: one [128, 8] partial-sum AllReduce; subtract + out
        # DMA of quarter g interleaves into quarter g+1's compute.
        ngrp = 4 if nblk_lim is None else min(nblk_lim, 4)
        with (
            tc.tile_pool(name="fwp", bufs=1) as fwp,
            tc.tile_pool(name="ltp", bufs=14) as ltp,
            tc.tile_pool(name="scrp", bufs=4) as scrp,
            tc.tile_pool(name="accp", bufs=16) as accp,
            tc.tile_pool(name="arp", bufs=2) as arp,
            tc.tile_pool(name="pcp", bufs=4) as pcp,
            tc.tile_pool(name="lpsp", bufs=4, space="PSUM") as lpsp,
        ):
            # remaining weight chunks land in freed scan space while the
            # first blocks chew on the early-loaded ch0/ch1 pairs
            fcw8b = fwp.tile([128, VSC - 2, 8, CH], f8, tag="fcw8b")
            for ch in range(2, VSC):
                nc.sync.dma_start(fcw8b[:, ch - 2], fcw8_d[ch])

            def fw_ch(ch):
                return fcw8a[:, ch] if ch < 2 else fcw8b[:, ch - 2]

            pend = []

            def emit_piece():
                if pend:
                    pend.pop(0)()

            def emit_block(q, c, arin, slot, nblk_):
                lt = ltp.tile([128, VSC, CH], f16, tag="lt")
                acc = accp.tile([128, 4], f32, tag="acc")
                for kp in range(4):
                    ps = lpsp.tile([128, 2, CH], f32, tag="lp")
                    for j in range(2):
                        fw = fw_ch(2 * kp + j)
                        for hp in range(4):
                            lhsT = x2all[q][:, 2 * hp:2 * hp + 2, c, :]
                            nc.tensor.matmul(
                                ps[:, j, :], lhsT,
                                fw[:, 2 * hp:2 * hp + 2, :],
                                start=(hp == 0), stop=False, perf_mode=DR)
                        nc.tensor.matmul(
                            ps[:, j, :], e8[:, :, 2 * kp + j, :],
                            biasT[:], start=False, stop=True, perf_mode=DR)
                    # exp of both chunks + row-sum accum, off PSUM
                    scr = scrp.tile([128, 2, CH], f16, tag="scr")
                    nc.scalar.activation(scr[:], ps[:], AF.Exp,
                                         accum_out=acc[:, kp:kp + 1])
                    # copy l out of PSUM (GPSIMD cannot touch PSUM, so the
                    # copies split DVE/ACT; Pool gets SBUF-side subtracts)
                    dst = lt[:, 2 * kp:2 * kp + 2, :]
                    if kp == 3:
                        nc.scalar.copy(dst, ps[:])
                    else:
                        nc.vector.tensor_copy(dst, ps[:])
                nc.vector.tensor_reduce(arin[:, slot:slot + 1], acc[:],
                                        axis=AX.X, op=OP.add)
                return lt

            def finalize(g, q, blks, lts, last=False):
                # partial sums were AllGathered (15us constant beats the
                # AllReduce's 28us): sum the 8 cores' partials locally
                arb = arp.tile([128, 8, NC_], f32, tag="arb")
                nc.sync.dma_start(arb[:], aro[g][:].rearrange(
                    "c p b -> p b c"))
                ssum = arp.tile([128, 8], f32, tag="ssum")
                nc.vector.tensor_reduce(ssum[:], arb[:], axis=AX.X,
                                        op=OP.add)
                lnS = arp.tile([128, 8], f32, tag="lnS")
                nc.scalar.activation(lnS[:], ssum[:], AF.Ln)

                def piece(i, c, kp, q=q, lts=lts, lnS=lnS, last=last):
                    w = min(2 * CH, VS - 2 * CH * kp)
                    pc = pcp.tile([128, 2, CH], f16, tag="pc")
                    src = lts[i][:, 2 * kp:2 * kp + 2, :]
                    # Pool's subs are 3.6x DVE's: fine mid-phase (DVE is
                    # busy with copies), but the final drain runs on an
                    # otherwise-idle DVE
                    if kp % 2 == 0 and not last:
                        nc.gpsimd.tensor_scalar_sub(pc[:], src,
                                                    lnS[:, i:i + 1])
                    else:
                        nc.vector.tensor_scalar_sub(pc[:], src,
                                                    lnS[:, i:i + 1])
                    nc.sync.dma_start(
                        out_d[c, 128 * q:128 * (q + 1),
                              2 * CH * kp:2 * CH * kp + w],
                        pc[:].rearrange("p a b -> p (a b)")[:, 0:w])
                return [lambda i=i, c=c, kp=kp: piece(i, c, kp)
                        for i, c in enumerate(blks) for kp in range(4)]

            # groups of (quarter, src-core) blocks; the first is small so
            # the sum-exchange pipeline warms early, and the last is small
            # so the serial tail (exchange + drain) after the final blocks
            # is short
            groups = [(0, list(range(8))), (1, list(range(8))),
                      (2, list(range(8))), (3, [0, 1, 2, 3, 4]),
                      (3, [5, 6, 7])][:ngrp if ngrp < 4 else 5]
            nblk_ = 0
            for g, (q, blks) in enumerate(groups):
                arin = arp.tile([128, 8], f32, tag="arin", name=f"ari{g}")
                lts = []
                for slot, c in enumerate(blks):
                    lts.append(emit_block(q, c, arin, slot, nblk_))
                    nblk_ += 1
                    for _ in range(4):
                        emit_piece()
                if len(blks) < 8:
                    nc.vector.memset(arin[:, len(blks):], 0.0)
                nc.sync.dma_start(ari[g][:], arin[:])
                nc.gpsimd.collective_compute(
                    "AllGather", OP.bypass,
                    replica_groups=[list(range(NC_))],
                    ins=[ari[g].opt()], outs=[aro[g].opt()])
                pend.extend(finalize(g, q, blks, lts,
                                     last=(g >= len(groups) - 2)))
            while pend:
                emit_piece()

    nc.compile()
    return nc


def _get_nc():
    if "nc" not in _BUILT:
        _BUILT["nc"] = _build_nc()
    return _BUILT["nc"]


def _prep_inputs(inputs):
    """Host-side shard + relayout. Returns in_maps for 8 cores."""
    f16 = np.float16

    tgt = np.asarray(inputs["target"])
    ctx = np.asarray(inputs["context"], np.float32)
    emb_t = np.asarray(inputs["embed_table"], np.float32)
    fc_w = np.asarray(inputs["fc_w"], np.float32)
    fc_b = np.asarray(inputs["fc_b"], np.float32)
    w_ih0 = np.asarray(inputs["w_ih0"], np.float32)
    w_hh0 = np.asarray(inputs["w_hh0"], np.float32)
    w_ih1 = np.asarray(inputs["w_ih1"], np.float32)
    w_hh1 = np.asarray(inputs["w_hh1"], np.float32)
    b_ih0 = np.asarray(inputs["b_ih0"], np.float32)
    b_hh0 = np.asarray(inputs["b_hh0"], np.float32)
    b_ih1 = np.asarray(inputs["b_ih1"], np.float32)
    b_hh1 = np.asarray(inputs["b_hh1"], np.float32)

    def wT(w, kc):     # [2, G, I] -> [128, kc, 2, G]
        return np.ascontiguousarray(
            w.transpose(2, 0, 1).reshape(kc, 128, 2, G).transpose(1, 0, 2, 3)
        ).astype(f16)

    gmask_rz = (np.arange(G) < 2 * H)

    def bgi(b_ih, b_hh):   # [128, 2, 12, GC_NR]
        v = b_ih + np.where(gmask_rz[None, :], b_hh, 0.0)
        v = v.reshape(2, 12, 128).transpose(2, 0, 1)
        return np.ascontiguousarray(np.broadcast_to(
            v[:, :, :, None], (128, 2, 12, GC_NR))).astype(f16)

    def bhhnB(b_hh):   # [128, 2, 4, S, BPC] broadcast of the n-part
        bn = b_hh[:, 2 * H:].reshape(2, 4, 128).transpose(2, 0, 1)
        return np.ascontiguousarray(np.broadcast_to(
            bn[:, :, :, None, None], (128, 2, 4, S, BPC))).astype(f16)

    import ml_dtypes
    f8 = ml_dtypes.float8_e4m3
    # fc_w rows padded so shard 7's 4096-col window stays in-bounds
    fcw_pad = np.zeros((NC_ * VS + VSC * CH, 2 * H), np.float32)
    fcw_pad[:V] = fc_w

    # bias-pair lhsT: chunk ch's selector is 1.0 at [p=ch, i=0, :]
    e8 = np.zeros((128, 2, VSC, 128), np.float32)
    for ch in range(VSC):
        e8[ch, 0, ch, :] = 1.0

    bGiB = np.ascontiguousarray(np.stack(
        [bgi(b_ih0, b_hh0), bgi(b_ih1, b_hh1)], axis=1))

    shared = {
        "WihT0": wT(w_ih0, 4), "WhhT0": wT(w_hh0, 4),
        "WihT1": wT(w_ih1, 8), "WhhT1": wT(w_hh1, 4),
        "bGiB": bGiB,
        "bHhnB0": bhhnB(b_hh0), "bHhnB1": bhhnB(b_hh1),
        "e8": e8.astype(f8),
    }

    emb = emb_t[tgt]                      # [B, S, E]
    ctx4 = ctx.reshape(L, 2, B, H)        # [l, d, b, h]

    in_maps = []
    for c in range(NC_):
        bs = slice(BPC * c, BPC * (c + 1))
        er = emb[bs].transpose(1, 0, 2).reshape(R, E)   # row = 4t + b
        embTc = np.ascontiguousarray(
            er.T.reshape(4, 128, R).transpose(1, 0, 2)).astype(f16)
        cc = ctx4[:, :, bs, :]                          # [l, d, 4, h]
        h0Tc = np.ascontiguousarray(
            cc.transpose(3, 0, 1, 2).reshape(4, 128, L, 2, BPC)
            .transpose(1, 2, 3, 0, 4)).astype(f16)
        # vocab shard c: fc_w rows [VS*c, VS*c + VSC*CH), cols >= VS are
        # pad -> NEG bias so their exp vanishes from this core's partials
        wsh = fcw_pad[VS * c:VS * c + VSC * CH]
        fcw8 = np.ascontiguousarray(
            wsh.reshape(VSC, CH, 8, 128).transpose(0, 3, 2, 1)).astype(f8)
        bsh = np.full((VSC * CH,), NEG, np.float32)
        bsh[:VS] = fc_b[VS * c:VS * (c + 1)]
        biasT = np.zeros((128, 2, CH), np.float32)
        biasT[:VSC, 0, :] = bsh.reshape(VSC, CH)
        m = {"embT": embTc, "h0T": h0Tc,
             "fcw8": fcw8, "biasT": biasT.astype(f8)}
        m.update(shared)
        in_maps.append(m)
    return in_maps


def _unshard(results):
    full = np.empty((B, S, V), np.float32)
    for k in range(NC_):
        o = np.asarray(results[k]["out"], np.float32)   # [src, 4t+b, vv]
        o = o.reshape(NC_, S, BPC, VS).transpose(0, 2, 1, 3)
        full[:, :, VS * k:VS * (k + 1)] = o.reshape(B, S, VS)
    return full


def kernel(**inputs):
    from concourse.bass_utils import run_bass_kernel_spmd
    nc = _get_nc()
    in_maps = _prep_inputs(inputs)
    res = run_bass_kernel_spmd(nc, in_maps, core_ids=list(range(NC_)))
    return _unshard(res.results)



# revision 87
# speedup vs baseline: 1.4267x; 1.0090x over previous
"""Trainium2 Bass kernel for nn_PlainDecoder (2-layer 2-direction GRU decoder
+ vocab projection + log_softmax), vocab-parallel across 8 NeuronCores.

Sharding:
  - GRU scan: data-parallel over batch (32 batches -> 4 per core). Each core
    runs both "directions" of both layers for its 4 batches.
  - Logits/log_softmax: VOCAB-parallel. As each 32-step quarter of the scan
    finishes, its fp8 output is AllGathered (DRAM bounce) so every core
    holds all 4096 (t, b) rows; each core then computes its resident
    4000-wide fc_w shard (fp8 DoubleRow matmuls: two 128-contractions per
    instruction at 0.5 cycles/row, bias folded in as a 5th pair) for all
    rows. Per row-group, the per-shard exp-sums are exchanged with a small
    AllGather and summed locally; log_softmax subtracts ln(S) as pieces
    stream out. Collective latency hides behind the compute pipeline.

Scan layout (gate-major / weight-stationary): the recurrent gate matmuls put
the GATE dim on PSUM partitions: lhsT = Whh^T chunk [128 (h-sub), 128 (gate
sub)], rhs = h^T [128 (h-sub), 4 (batch)]. The output h'^T lands directly in
the [h-sub partitions, batch] layout the next step's rhs needs -> zero
transposes. Precomputed gi (+ fused biases) ride into PSUM via identity-
matmul injections; b_hh's n-part is (t, b)-constant and injects from a tiny
resident tile. Histories and the gi buffers are 32-step rings, freeing SBUF
for the resident fc_w shard. The two layer scans are software-pipelined
(layer 1 lags layer 0 by LAG slots) with both layers' gi built chunk-by-
chunk on spare PE cycles inside the slot loop.

The scan state is float16 (fp16 matmuls run at 1 cycle/row at any p-state,
and fp16's 10-bit mantissa keeps the recurrent rounding walk ~8x below
bf16's); everything on the logits path is fp8-e4m3.
"""

import os
import sys
from contextlib import ExitStack

for _p in ("/opt/trn_rl_repo", "/root/.axon_site/_ro/trn_rl_repo"):
    if os.path.isdir(_p) and _p not in sys.path:
        sys.path.insert(0, _p)

import numpy as np  # noqa: E402

V, E, H, L, B, S = 32000, 512, 512, 2, 32, 128
NC_ = 8                      # cores
BPC = B // NC_               # batches per core = 4
R = BPC * S                  # rows per core = 512 (row = 4*t + b)
G = 3 * H                    # 1536 gates per direction
CH = 512                     # vocab chunk width
VS = V // NC_                # per-core vocab shard = 4000
VSC = 8                      # chunks per shard -> 4096 padded
NEG = -80.0                  # pad bias -> exp() ~ 0
GCH = 8                      # gi chunk = 8 timesteps
GC_NR = GCH * BPC            # rows per gi chunk

_BUILT = {}


def _build_nc(T=S, n_cores=NC_, sim=False, nblk_lim=None, skip_gi=False):
    """Build the Bass program (same NEFF for all cores; per-core data only).

    sim is accepted for compatibility; the program has no collectives so the
    TimelineSim build is identical.
    """
    import concourse.bass as bass  # noqa: F401
    import concourse.mybir as mybir
    import concourse.tile as tile
    from concourse import bacc
    from concourse.masks import make_identity

    dt = mybir.dt
    f32 = dt.float32
    f16 = dt.float16
    AF = mybir.ActivationFunctionType
    OP = mybir.AluOpType
    AX = mybir.AxisListType
    DR = mybir.MatmulPerfMode.DoubleRow

    nc = bacc.Bacc("TRN2", target_bir_lowering=False, debug=False,
                   num_devices=n_cores)

    TR = BPC * T               # rows actually scanned

    # ---------------- DRAM I/O ----------------
    embT = nc.dram_tensor("embT", [128, 4, R], f16, kind="ExternalInput")
    h0T = nc.dram_tensor("h0T", [128, 2, 2, 4, BPC], f16,
                         kind="ExternalInput")          # (p, l, d, k, b)
    WhhT0 = nc.dram_tensor("WhhT0", [128, 4, 2, G], f16, kind="ExternalInput")
    WhhT1 = nc.dram_tensor("WhhT1", [128, 4, 2, G], f16, kind="ExternalInput")
    WihT0 = nc.dram_tensor("WihT0", [128, 4, 2, G], f16, kind="ExternalInput")
    WihT1 = nc.dram_tensor("WihT1", [128, 8, 2, G], f16, kind="ExternalInput")
    # b_ih (+b_hh for r/z) broadcast over a gi-chunk's (t, b) columns:
    # rides each gi chunk's PSUM as one identity-matmul injection
    bGiB = nc.dram_tensor("bGiB", [128, 2, 2, 12, GC_NR], f16,
                          kind="ExternalInput")
    # b_hh n-part, broadcast over (t, b): injected into the n-gate PSUM.
    bHhnB0 = nc.dram_tensor("bHhnB0", [128, 2, 4, S, BPC], f16,
                            kind="ExternalInput")
    bHhnB1 = nc.dram_tensor("bHhnB1", [128, 2, 4, S, BPC], f16,
                            kind="ExternalInput")
    f8 = dt.float8e4
    # Vocab-parallel logits: this core holds fc_w rows [VS*k, VS*k + VSC*CH)
    # (zero-padded), fp8-e4m3.  [chunk, p, h-tile, col]
    fcw8_d = nc.dram_tensor("fcw8", [VSC, 128, 8, CH], f8,
                            kind="ExternalInput")
    # fc_b shard as the rhs of a 5th DoubleRow pair: chunk ch's bias lives
    # on partition ch (pad cols carry NEG so their exp vanishes)
    biasT_d = nc.dram_tensor("biasT", [128, 2, CH], f8,
                             kind="ExternalInput")
    # lhsT of the bias pair for chunk ch: 1.0 at [p=ch, i=0, :], else 0
    e8_d = nc.dram_tensor("e8", [128, 2, VSC, 128], f8,
                          kind="ExternalInput")

    # this core's vocab slice for ALL rows: [src_core, row, vs]
    out_d = nc.dram_tensor("out", [NC_, R, VS], f16, kind="ExternalOutput")

    with tile.TileContext(nc) as tc, ExitStack() as top:
        constp = top.enter_context(tc.tile_pool(name="const", bufs=1))
        ident = constp.tile([128, 128], f16)
        make_identity(nc, ident[:])
        h0T_sb = constp.tile([128, 2, 2, 4, BPC], f16)
        nc.sync.dma_start(h0T_sb[:], h0T[:])
        # b_hh n-part is constant over (t, b): one tiny tile per layer
        # feeds every step's pn injection (no ring traffic at all)
        bnT = constp.tile([128, 2, 2, 4, BPC], f16, tag="bnT")
        nc.sync.dma_start(bnT[:, 0], bHhnB0[:, :, :, 0, :])
        nc.sync.dma_start(bnT[:, 1], bHhnB1[:, :, :, 0, :])

        # resident logits operands (fcw8 itself loads post-scan into the
        # space the scan frees); gathered fp8 scan outputs
        gathp = top.enter_context(tc.tile_pool(name="gath", bufs=1))
        biasT = gathp.tile([128, 2, CH], f8, tag="biasT")
        e8 = gathp.tile([128, 2, VSC, 128], f8, tag="e8")
        # first two fc_w chunks load during the scan so the first logits
        # blocks start immediately; the rest load into freed scan space
        fcw8a = gathp.tile([128, 2, 8, CH], f8, tag="fcw8a")
        # all cores' scan outputs: [p, h-tile, src_core, local row (4t+b)],
        # one tile per quarter so block reads only dep on their own gather
        QR = R // 4
        x2all = [gathp.tile([128, 8, NC_, QR], f8, name=f"x2all{q}")
                 for q in range(4)]
        # DRAM bounce buffers for the x2 all-gathers + sum all-reduces
        dramp = top.enter_context(tc.tile_pool(name="dram", bufs=1,
                                               space="DRAM"))
        x2qi = [dramp.tile([128, 8, QR], f8, name=f"x2qi{q}")
                for q in range(4)]
        x2qo = [dramp.tile([NC_, 128, 8, QR], f8, name=f"x2qo{q}")
                for q in range(4)]
        ari = [dramp.tile([128, 8], f32, name=f"ari{g}") for g in range(6)]
        aro = [dramp.tile([NC_, 128, 8], f32, name=f"aro{g}")
               for g in range(6)]

        with ExitStack() as scan_stack:
            wres = scan_stack.enter_context(tc.tile_pool(name="wres", bufs=1))
            gip = scan_stack.enter_context(tc.tile_pool(name="gip", bufs=1))
            hist1 = scan_stack.enter_context(tc.tile_pool(name="hist1",
                                                          bufs=1))
            # histories are 32-step rings (slot = t % RW): the recurrence
            # only needs t-1, gi1 builds lag <= 12 slots, and x2 is cast to
            # fp8 + staged to DRAM the moment each 32-step quarter finishes
            RW = 32
            x2T = hist1.tile([128, 2, 4, RW, BPC], f16, tag="x2T")
            x2f8 = hist1.tile([128, 2, 4, RW, BPC], f8, tag="x2f8")

            # One SHARED gi ring for both layers (32-step window, slot =
            # t % RW): L0 consumes slot t at slot t, and gi1's chunk for
            # steps [t0, t0+GCH) is only written after L0's reads of those
            # slots — so layer 1's gi overwrites layer 0's in place, and
            # the ring reuses each slot every 32 steps with ~9 slots of
            # margin. Both gi builds run chunked inside the slot loop.
            giRZB = gip.tile([128, 2, 8, 32, BPC], f16, tag="giRZB")
            giN = gip.tile([128, 2, 4, 32, BPC], f16, tag="giN")
            giR = [giRZB, giRZB]
            giNl = [giN, giN]
            x1T = hist1.tile([128, 2, 4, RW, BPC], f16, tag="x1T")
            hists = [x1T, x2T]

            # input DMAs ordered by first use: gi chunks 0/1 (emb, wih0,
            # bias), L0 step 0 (b_hh0, whh0), then gi1/L1 weights
            embT_sb = wres.tile([128, 4, R], f16, tag="embT")
            nc.sync.dma_start(embT_sb[:], embT[:])
            bgiB = wres.tile([128, 2, 2, 12, GC_NR], f16, tag="bgiB")
            nc.sync.dma_start(bgiB[:], bGiB[:])
            wih0 = wres.tile([128, 4, 2, G], f16, tag="wih0")
            for d in range(2):
                nc.sync.dma_start(wih0[:, :, d, :], WihT0[:, :, d, :])
            whh0 = wres.tile([128, 4, 2, G], f16, tag="whh0")
            # split by gate range so step 0's r-group matmuls start as
            # soon as the first third lands
            for j3 in range(3):
                nc.sync.dma_start(whh0[:, :, :, 512 * j3:512 * (j3 + 1)],
                                  WhhT0[:, :, :, 512 * j3:512 * (j3 + 1)])
            wih1 = wres.tile([128, 8, 2, G], f16, tag="wih1")
            for d in range(2):
                nc.sync.dma_start(wih1[:, :, d, :], WihT1[:, :, d, :])
            whh1 = wres.tile([128, 4, 2, G], f16, tag="whh1")
            nc.sync.dma_start(whh1[:], WhhT1[:])
            whhs = [whh0, whh1]
            # resident logits operands ride the idle DMA behind the scan's
            # critical loads (the gi ring freed the SBUF fcw8 needs)
            nc.sync.dma_start(biasT[:], biasT_d[:])
            nc.sync.dma_start(e8[:], e8_d[:])
            for ch in range(2):
                nc.sync.dma_start(fcw8a[:, ch], fcw8_d[ch])

            # ---------- pipelined two-layer scan ----------
            # LAG >= 8 is a hard correctness bound: gi1's chunk for steps
            # [8c, 8c+8) is emitted at slot 8c+8, and layer 1's step 8c is
            # emitted at slot 8c+LAG.  With LAG < 8 the consumer is emitted
            # BEFORE the gi1 write exists, so the dependency tracker lets
            # layer 1 read layer 0's gi values from the shared ring.
            LAG = 10
            with (
                tc.tile_pool(name="spr", bufs=2, space="PSUM") as przp,
                tc.tile_pool(name="spn", bufs=2, space="PSUM") as pnp,
                tc.tile_pool(name="spz", bufs=2, space="PSUM") as pzp,
                tc.tile_pool(name="gcp", bufs=2, space="PSUM") as gcp,
                tc.tile_pool(name="sch", bufs=8) as chp,
            ):
                def mk_step(l, t):
                    # split one step's emission into phases so the two
                    # layers' chains can be phase-shifted half a slot: the
                    # in-order ACT/DVE queues then serve L0's and L1's ops
                    # in the order they actually become ready
                    whh = whhs[l]
                    histT = hists[l]
                    gR = giR[l]
                    st = {}

                    def rhs(d, k):
                        if t == 0:
                            return h0T_sb[:, l, d, k, :]
                        return histT[:, d, k, (t - 1) % RW, :]

                    def gate_group(pool, jlo, jhi, nm):
                        ps = pool.tile([128, 2, jhi - jlo, BPC], f32,
                                       tag=nm, name=f"{nm}{l}")
                        for d in range(2):
                            inj = (bnT[:, l, d] if jlo == 8
                                   else gR[:, d, jlo:jhi, t % RW, :])
                            nc.tensor.matmul(
                                ps[:, d, :, :], ident[:], inj,
                                start=(d == 0), stop=False)
                            for j in range(jlo, jhi):
                                for k in range(4):
                                    nc.tensor.matmul(
                                        ps[:, d, j - jlo, :],
                                        whh[:, k, d, 128 * j:128 * (j + 1)],
                                        rhs(d, k), start=False,
                                        stop=(d == 1 and j == jhi - 1
                                              and k == 3))
                        return ps

                    def pe():
                        st["pr"] = gate_group(przp, 0, 4, "pr")
                        st["pn"] = gate_group(pnp, 8, 12, "pn")
                        st["pz"] = gate_group(pzp, 4, 8, "pz")

                    def sig_r():
                        st["rp"] = chp.tile([128, 2, 3, 4, BPC], f32,
                                            tag="rp", name=f"rp{l}")
                        nc.scalar.activation(st["rp"][:, :, 0], st["pr"][:],
                                             AF.Sigmoid)

                    def n1_muladd():
                        st["n1"] = chp.tile([128, 2, 4, BPC], f32,
                                            tag="n1", name=f"n1{l}")
                        nc.vector.tensor_mul(st["n1"][:], st["pn"][:],
                                             st["rp"][:, :, 0])
                        nc.vector.tensor_add(st["n1"][:], st["n1"][:],
                                             giNl[l][:, :, :, t % RW, :])

                    def sig_z():
                        nc.scalar.activation(st["rp"][:, :, 1], st["pz"][:],
                                             AF.Sigmoid)

                    def tanh():
                        nc.scalar.activation(st["rp"][:, :, 2], st["n1"][:],
                                             AF.Tanh)

                    def ozm1():
                        st["oz"] = chp.tile([128, 2, 4, BPC], f32,
                                            tag="oz", name=f"oz{l}")
                        st["m1"] = chp.tile([128, 2, 4, BPC], f32,
                                            tag="m1", name=f"m1{l}")
                        hprev = (h0T_sb[:, l] if t == 0
                                 else histT[:, :, :, (t - 1) % RW, :])
                        nc.vector.tensor_scalar(st["oz"][:],
                                                st["rp"][:, :, 1],
                                                -1.0, 1.0, OP.mult, OP.add)
                        nc.vector.tensor_mul(st["m1"][:],
                                             st["rp"][:, :, 1], hprev)

                    def tail():
                        m2 = chp.tile([128, 2, 4, BPC], f32, tag="m2",
                                      name=f"m2{l}")
                        nc.vector.tensor_mul(m2[:], st["oz"][:],
                                             st["rp"][:, :, 2])
                        nc.vector.tensor_add(histT[:, :, :, t % RW, :],
                                             st["m1"][:], m2[:])
                    return pe, sig_r, n1_muladd, sig_z, tanh, ozm1, tail

                def emit_slot_steps(steps):
                    ph = {l: mk_step(l, t) for l, t in steps}
                    if 0 in ph and 1 in ph:
                        pe0, sr0, nm0, sz0, th0, oz0, tl0 = ph[0]
                        pe1, sr1, nm1, sz1, th1, oz1, tl1 = ph[1]
                        pe0(); sr0(); pe1()
                        nm0(); sz0(); sr1()
                        th0(); oz0(); nm1()
                        sz1(); tl0(); th1()
                        oz1(); tl1()
                    else:
                        for l in ph:
                            pe, sr, nm, sz, th, oz, tl = ph[l]
                            pe(); sr(); nm(); sz(); th(); oz(); tl()

                def gi_chunk(l, c):
                    t0, t1 = GCH * c, GCH * (c + 1)
                    nr = GC_NR
                    w0 = t0 % RW
                    wih, kc = (wih0, 4) if l == 0 else (wih1, 8)
                    for d in range(2):
                        for grp, (j0, nj) in enumerate([(0, 8), (8, 4)]):
                            ps = gcp.tile([128, nj, nr], f32, tag="gc",
                                          name=f"gc{grp}")
                            # bias seeds the whole group (zeroes the bank)
                            nc.tensor.matmul(
                                ps[:], ident[:],
                                bgiB[:, l, d, j0:j0 + nj, :],
                                start=True, stop=False)
                            for jj in range(nj):
                                j = j0 + jj
                                for kk in range(kc):
                                    if l == 0:
                                        xs = embT_sb[:, kk,
                                                     BPC * t0:BPC * t1]
                                    else:
                                        r0 = t0 % RW
                                        xs = x1T[:, kk // 4, kk % 4,
                                                 r0:r0 + GCH, :]
                                        xs = xs.rearrange("p t b -> p (t b)")
                                    nc.tensor.matmul(
                                        ps[:, jj, :],
                                        wih[:, kk, d, 128 * j:128 * (j + 1)],
                                        xs, start=False,
                                        stop=(jj == nj - 1 and kk == kc - 1))
                            if grp == 0:
                                dst = giRZB[:, d, 0:8, w0:w0 + GCH, :]
                            else:
                                dst = giN[:, d, :, w0:w0 + GCH, :]
                            dst = dst.rearrange("p j t b -> p j (t b)")
                            if (2 * d + grp) % 2 == 0:
                                nc.scalar.copy(dst, ps[:])
                            else:
                                nc.vector.tensor_copy(dst, ps[:])

                NGC = (T + GCH - 1) // GCH
                gi_chunk(0, 0)
                gi_chunk(0, 1)
                for s in range(T + LAG):
                    steps = []
                    if s < T:
                        steps.append((0, s))
                    if s >= LAG:
                        steps.append((1, s - LAG))
                    emit_slot_steps(steps)
                    if (s + 5) % GCH == 0 and 2 <= (s + 5) // GCH < NGC:
                        gi_chunk(0, (s + 5) // GCH)
                    if s >= 8 and (s - 8) % GCH == 0 and (s - 8) // GCH < NGC:
                        gi_chunk(1, (s - 8) // GCH)
                    if s >= LAG and (s - LAG + 1) % 8 == 0:
                        # cast each finished 8-step eighth to fp8 as it
                        # completes: small queue bubbles, and the ring WAR
                        # clears ~24 slots before the slot is rewritten
                        t0 = (s - LAG + 1) - 8
                        w = t0 % RW
                        csrc = x2T[:, :, :, w:w + GCH, :]
                        cdst = x2f8[:, :, :, w:w + GCH, :]
                        if (t0 // 8) % 2 == 0:
                            nc.scalar.copy(cdst, csrc)
                        else:
                            nc.vector.tensor_copy(cdst, csrc)
                    if s >= LAG and (s - LAG + 1) % 32 == 0:
                        # a quarter of the scan output is staged to DRAM,
                        # all-gathered, and scattered into x2all
                        q = (s - LAG + 1) // 32 - 1
                        nc.sync.dma_start(
                            x2qi[q][:],
                            x2f8[:].rearrange("p d k t b -> p (d k) (t b)"))
                        nc.gpsimd.collective_compute(
                            "AllGather", OP.bypass,
                            replica_groups=[list(range(NC_))],
                            ins=[x2qi[q].opt()], outs=[x2qo[q].opt()])
                        # gpsimd queue: this DMA waits ~41us for the
                        # collective, which would head-of-line-block the
                        # scan's bias DMAs if it sat on the SP sequencer
                        nc.gpsimd.dma_start(
                            x2all[q][:],
                            x2qo[q][:].rearrange("c p h r -> p h c r"))

        # ------- logits + log_softmax (vocab-parallel over the 8 cores) ----
        # Every core computes its VS-wide vocab shard for ALL 4096 rows.
        # Blocks: (quarter i, src core c) -> 128 rows of x2all.  Per block:
        # 4 chunk-pairs of 5 DoubleRow matmuls (4 x2 pairs + bias pair),
        # exp+accum straight off PSUM, PSUM->lt copies split DVE/Pool.
        # Per quarter: one [128, 8] partial-sum AllReduce; subtract + out
        # DMA of quarter g interleaves into quarter g+1's compute.
        ngrp = 4 if nblk_lim is None else min(nblk_lim, 4)
        with (
            tc.tile_pool(name="fwp", bufs=1) as fwp,
            tc.tile_pool(name="ltp", bufs=13) as ltp,
            tc.tile_pool(name="scrp", bufs=2) as scrp,
            tc.tile_pool(name="accp", bufs=16) as accp,
            tc.tile_pool(name="arp", bufs=2) as arp,
            tc.tile_pool(name="pcp", bufs=4) as pcp,
            tc.tile_pool(name="lpsp", bufs=4, space="PSUM") as lpsp,
        ):
            # remaining weight chunks land in freed scan space while the
            # first blocks chew on the early-loaded ch0/ch1 pairs
            fcw8b = fwp.tile([128, VSC - 2, 8, CH], f8, tag="fcw8b")
            for ch in range(2, VSC):
                nc.sync.dma_start(fcw8b[:, ch - 2], fcw8_d[ch])

            def fw_ch(ch):
                return fcw8a[:, ch] if ch < 2 else fcw8b[:, ch - 2]

            pend = []

            def emit_piece():
                if pend:
                    pend.pop(0)()

            def emit_block(q, c, arin, slot, nblk_):
                lt = ltp.tile([128, VSC, CH], f16, tag="lt")
                for kp in range(4):
                    ps = lpsp.tile([128, 2, CH], f32, tag="lp")
                    for j in range(2):
                        fw = fw_ch(2 * kp + j)
                        for hp in range(4):
                            lhsT = x2all[q][:, 2 * hp:2 * hp + 2, c, :]
                            nc.tensor.matmul(
                                ps[:, j, :], lhsT,
                                fw[:, 2 * hp:2 * hp + 2, :],
                                start=(hp == 0), stop=False, perf_mode=DR)
                        nc.tensor.matmul(
                            ps[:, j, :], e8[:, :, 2 * kp + j, :],
                            biasT[:], start=False, stop=True, perf_mode=DR)
                    # copy l out of PSUM (GPSIMD cannot touch PSUM, so the
                    # copies split DVE/ACT); the PSUM WAR clears on the
                    # copy alone -- exp reads the SBUF copy later
                    dst = lt[:, 2 * kp:2 * kp + 2, :]
                    if kp == 3:
                        nc.scalar.copy(dst, ps[:])
                    else:
                        nc.vector.tensor_copy(dst, ps[:])
                # one whole-block exp off SBUF sums straight into arin
                scr = scrp.tile([128, VSC, CH], f16, tag="scr")
                nc.scalar.activation(scr[:], lt[:], AF.Exp,
                                     accum_out=arin[:, slot:slot + 1])
                return lt

            def finalize(g, q, blks, lts, last=False):
                # partial sums were AllGathered (15us constant beats the
                # AllReduce's 28us): sum the 8 cores' partials locally
                arb = arp.tile([128, 8, NC_], f32, tag="arb")
                nc.sync.dma_start(arb[:], aro[g][:].rearrange(
                    "c p b -> p b c"))
                ssum = arp.tile([128, 8], f32, tag="ssum")
                nc.vector.tensor_reduce(ssum[:], arb[:], axis=AX.X,
                                        op=OP.add)
                lnS = arp.tile([128, 8], f32, tag="lnS")
                nc.scalar.activation(lnS[:], ssum[:], AF.Ln)

                def piece(i, c, kp, q=q, lts=lts, lnS=lnS, last=last):
                    w = min(2 * CH, VS - 2 * CH * kp)
                    pc = pcp.tile([128, 2, CH], f16, tag="pc")
                    src = lts[i][:, 2 * kp:2 * kp + 2, :]
                    # Pool's subs are 3.6x DVE's: fine mid-phase (DVE is
                    # busy with copies), but the final drain runs on an
                    # otherwise-idle DVE
                    if kp % 2 == 0 and not last:
                        nc.gpsimd.tensor_scalar_sub(pc[:], src,
                                                    lnS[:, i:i + 1])
                    else:
                        nc.vector.tensor_scalar_sub(pc[:], src,
                                                    lnS[:, i:i + 1])
                    nc.sync.dma_start(
                        out_d[c, 128 * q:128 * (q + 1),
                              2 * CH * kp:2 * CH * kp + w],
                        pc[:].rearrange("p a b -> p (a b)")[:, 0:w])
                return [lambda i=i, c=c, kp=kp: piece(i, c, kp)
                        for i, c in enumerate(blks) for kp in range(4)]

            # groups of (quarter, src-core) blocks; the first is small so
            # the sum-exchange pipeline warms early, and the last is small
            # so the serial tail (exchange + drain) after the final blocks
            # is short
            groups = [(0, list(range(8))), (1, list(range(8))),
                      (2, list(range(8))), (3, [0, 1, 2, 3, 4]),
                      (3, [5, 6, 7])][:ngrp if ngrp < 4 else 5]
            nblk_ = 0
            for g, (q, blks) in enumerate(groups):
                arin = arp.tile([128, 8], f32, tag="arin", name=f"ari{g}")
                lts = []
                for slot, c in enumerate(blks):
                    lts.append(emit_block(q, c, arin, slot, nblk_))
                    nblk_ += 1
                    for _ in range(4):
                        emit_piece()
                if len(blks) < 8:
                    nc.vector.memset(arin[:, len(blks):], 0.0)
                nc.sync.dma_start(ari[g][:], arin[:])
                nc.gpsimd.collective_compute(
                    "AllGather", OP.bypass,
                    replica_groups=[list(range(NC_))],
                    ins=[ari[g].opt()], outs=[aro[g].opt()])
                pend.extend(finalize(g, q, blks, lts,
                                     last=(g >= len(groups) - 2)))
            while pend:
                emit_piece()

    nc.compile()
    return nc


def _get_nc():
    if "nc" not in _BUILT:
        _BUILT["nc"] = _build_nc()
    return _BUILT["nc"]


def _prep_inputs(inputs):
    """Host-side shard + relayout. Returns in_maps for 8 cores."""
    f16 = np.float16

    tgt = np.asarray(inputs["target"])
    ctx = np.asarray(inputs["context"], np.float32)
    emb_t = np.asarray(inputs["embed_table"], np.float32)
    fc_w = np.asarray(inputs["fc_w"], np.float32)
    fc_b = np.asarray(inputs["fc_b"], np.float32)
    w_ih0 = np.asarray(inputs["w_ih0"], np.float32)
    w_hh0 = np.asarray(inputs["w_hh0"], np.float32)
    w_ih1 = np.asarray(inputs["w_ih1"], np.float32)
    w_hh1 = np.asarray(inputs["w_hh1"], np.float32)
    b_ih0 = np.asarray(inputs["b_ih0"], np.float32)
    b_hh0 = np.asarray(inputs["b_hh0"], np.float32)
    b_ih1 = np.asarray(inputs["b_ih1"], np.float32)
    b_hh1 = np.asarray(inputs["b_hh1"], np.float32)

    def wT(w, kc):     # [2, G, I] -> [128, kc, 2, G]
        return np.ascontiguousarray(
            w.transpose(2, 0, 1).reshape(kc, 128, 2, G).transpose(1, 0, 2, 3)
        ).astype(f16)

    gmask_rz = (np.arange(G) < 2 * H)

    def bgi(b_ih, b_hh):   # [128, 2, 12, GC_NR]
        v = b_ih + np.where(gmask_rz[None, :], b_hh, 0.0)
        v = v.reshape(2, 12, 128).transpose(2, 0, 1)
        return np.ascontiguousarray(np.broadcast_to(
            v[:, :, :, None], (128, 2, 12, GC_NR))).astype(f16)

    def bhhnB(b_hh):   # [128, 2, 4, S, BPC] broadcast of the n-part
        bn = b_hh[:, 2 * H:].reshape(2, 4, 128).transpose(2, 0, 1)
        return np.ascontiguousarray(np.broadcast_to(
            bn[:, :, :, None, None], (128, 2, 4, S, BPC))).astype(f16)

    import ml_dtypes
    f8 = ml_dtypes.float8_e4m3
    # fc_w rows padded so shard 7's 4096-col window stays in-bounds
    fcw_pad = np.zeros((NC_ * VS + VSC * CH, 2 * H), np.float32)
    fcw_pad[:V] = fc_w

    # bias-pair lhsT: chunk ch's selector is 1.0 at [p=ch, i=0, :]
    e8 = np.zeros((128, 2, VSC, 128), np.float32)
    for ch in range(VSC):
        e8[ch, 0, ch, :] = 1.0

    bGiB = np.ascontiguousarray(np.stack(
        [bgi(b_ih0, b_hh0), bgi(b_ih1, b_hh1)], axis=1))

    shared = {
        "WihT0": wT(w_ih0, 4), "WhhT0": wT(w_hh0, 4),
        "WihT1": wT(w_ih1, 8), "WhhT1": wT(w_hh1, 4),
        "bGiB": bGiB,
        "bHhnB0": bhhnB(b_hh0), "bHhnB1": bhhnB(b_hh1),
        "e8": e8.astype(f8),
    }

    emb = emb_t[tgt]                      # [B, S, E]
    ctx4 = ctx.reshape(L, 2, B, H)        # [l, d, b, h]

    in_maps = []
    for c in range(NC_):
        bs = slice(BPC * c, BPC * (c + 1))
        er = emb[bs].transpose(1, 0, 2).reshape(R, E)   # row = 4t + b
        embTc = np.ascontiguousarray(
            er.T.reshape(4, 128, R).transpose(1, 0, 2)).astype(f16)
        cc = ctx4[:, :, bs, :]                          # [l, d, 4, h]
        h0Tc = np.ascontiguousarray(
            cc.transpose(3, 0, 1, 2).reshape(4, 128, L, 2, BPC)
            .transpose(1, 2, 3, 0, 4)).astype(f16)
        # vocab shard c: fc_w rows [VS*c, VS*c + VSC*CH), cols >= VS are
        # pad -> NEG bias so their exp vanishes from this core's partials
        wsh = fcw_pad[VS * c:VS * c + VSC * CH]
        fcw8 = np.ascontiguousarray(
            wsh.reshape(VSC, CH, 8, 128).transpose(0, 3, 2, 1)).astype(f8)
        bsh = np.full((VSC * CH,), NEG, np.float32)
        bsh[:VS] = fc_b[VS * c:VS * (c + 1)]
        biasT = np.zeros((128, 2, CH), np.float32)
        biasT[:VSC, 0, :] = bsh.reshape(VSC, CH)
        m = {"embT": embTc, "h0T": h0Tc,
             "fcw8": fcw8, "biasT": biasT.astype(f8)}
        m.update(shared)
        in_maps.append(m)
    return in_maps


def _unshard(results):
    full = np.empty((B, S, V), np.float32)
    for k in range(NC_):
        o = np.asarray(results[k]["out"], np.float32)   # [src, 4t+b, vv]
        o = o.reshape(NC_, S, BPC, VS).transpose(0, 2, 1, 3)
        full[:, :, VS * k:VS * (k + 1)] = o.reshape(B, S, VS)
    return full


def kernel(**inputs):
    from concourse.bass_utils import run_bass_kernel_spmd
    nc = _get_nc()
    in_maps = _prep_inputs(inputs)
    res = run_bass_kernel_spmd(nc, in_maps, core_ids=list(range(NC_)))
    return _unshard(res.results)



# revision 89
# speedup vs baseline: 1.4675x; 1.0286x over previous
"""Trainium2 Bass kernel for nn_PlainDecoder (2-layer 2-direction GRU decoder
+ vocab projection + log_softmax), vocab-parallel across 8 NeuronCores.

Sharding:
  - GRU scan: data-parallel over batch (32 batches -> 4 per core). Each core
    runs both "directions" of both layers for its 4 batches.
  - Logits/log_softmax: VOCAB-parallel. As each 32-step quarter of the scan
    finishes, its fp8 output is AllGathered (DRAM bounce) so every core
    holds all 4096 (t, b) rows; each core then computes its resident
    4000-wide fc_w shard (fp8 DoubleRow matmuls: two 128-contractions per
    instruction at 0.5 cycles/row, bias folded in as a 5th pair) for all
    rows. Per row-group, the per-shard exp-sums are exchanged with a small
    AllGather and summed locally; log_softmax subtracts ln(S) as pieces
    stream out. Collective latency hides behind the compute pipeline.

Scan layout (gate-major / weight-stationary): the recurrent gate matmuls put
the GATE dim on PSUM partitions: lhsT = Whh^T chunk [128 (h-sub), 128 (gate
sub)], rhs = h^T [128 (h-sub), 4 (batch)]. The output h'^T lands directly in
the [h-sub partitions, batch] layout the next step's rhs needs -> zero
transposes. Precomputed gi (+ fused biases) ride into PSUM via identity-
matmul injections; b_hh's n-part is (t, b)-constant and injects from a tiny
resident tile. Histories and the gi buffers are 32-step rings, freeing SBUF
for the resident fc_w shard. The two layer scans are software-pipelined
(layer 1 lags layer 0 by LAG slots) with both layers' gi built chunk-by-
chunk on spare PE cycles inside the slot loop.

The scan state is float16 (fp16 matmuls run at 1 cycle/row at any p-state,
and fp16's 10-bit mantissa keeps the recurrent rounding walk ~8x below
bf16's); everything on the logits path is fp8-e4m3.
"""

import os
import sys
from contextlib import ExitStack

for _p in ("/opt/trn_rl_repo", "/root/.axon_site/_ro/trn_rl_repo"):
    if os.path.isdir(_p) and _p not in sys.path:
        sys.path.insert(0, _p)

import numpy as np  # noqa: E402

V, E, H, L, B, S = 32000, 512, 512, 2, 32, 128
NC_ = 8                      # cores
BPC = B // NC_               # batches per core = 4
R = BPC * S                  # rows per core = 512 (row = 4*t + b)
G = 3 * H                    # 1536 gates per direction
CH = 512                     # vocab chunk width
VS = V // NC_                # per-core vocab shard = 4000
VSC = 8                      # chunks per shard -> 4096 padded
NEG = -80.0                  # pad bias -> exp() ~ 0
GCH = 8                      # gi chunk = 8 timesteps
GC_NR = GCH * BPC            # rows per gi chunk

_BUILT = {}


def _build_nc(T=S, n_cores=NC_, sim=False, nblk_lim=None, skip_gi=False):
    """Build the Bass program (same NEFF for all cores; per-core data only).

    sim is accepted for compatibility; the program has no collectives so the
    TimelineSim build is identical.
    """
    import concourse.bass as bass  # noqa: F401
    import concourse.mybir as mybir
    import concourse.tile as tile
    from concourse import bacc
    from concourse.masks import make_identity

    dt = mybir.dt
    f32 = dt.float32
    f16 = dt.float16
    AF = mybir.ActivationFunctionType
    OP = mybir.AluOpType
    AX = mybir.AxisListType
    DR = mybir.MatmulPerfMode.DoubleRow

    nc = bacc.Bacc("TRN2", target_bir_lowering=False, debug=False,
                   num_devices=n_cores)

    TR = BPC * T               # rows actually scanned

    # ---------------- DRAM I/O ----------------
    embT = nc.dram_tensor("embT", [128, 4, R], f16, kind="ExternalInput")
    h0T = nc.dram_tensor("h0T", [128, 2, 2, 4, BPC], f16,
                         kind="ExternalInput")          # (p, l, d, k, b)
    WhhT0 = nc.dram_tensor("WhhT0", [128, 4, 2, G], f16, kind="ExternalInput")
    WhhT1 = nc.dram_tensor("WhhT1", [128, 4, 2, G], f16, kind="ExternalInput")
    WihT0 = nc.dram_tensor("WihT0", [128, 4, 2, G], f16, kind="ExternalInput")
    WihT1 = nc.dram_tensor("WihT1", [128, 8, 2, G], f16, kind="ExternalInput")
    # b_ih (+b_hh for r/z) broadcast over a gi-chunk's (t, b) columns:
    # rides each gi chunk's PSUM as one identity-matmul injection
    bGiB = nc.dram_tensor("bGiB", [128, 2, 2, 12, GC_NR], f16,
                          kind="ExternalInput")
    # b_hh n-part, broadcast over (t, b): injected into the n-gate PSUM.
    bHhnB0 = nc.dram_tensor("bHhnB0", [128, 2, 4, S, BPC], f16,
                            kind="ExternalInput")
    bHhnB1 = nc.dram_tensor("bHhnB1", [128, 2, 4, S, BPC], f16,
                            kind="ExternalInput")
    f8 = dt.float8e4
    # Vocab-parallel logits: this core holds fc_w rows [VS*k, VS*k + VSC*CH)
    # (zero-padded), fp8-e4m3.  [chunk, p, h-tile, col]
    fcw8_d = nc.dram_tensor("fcw8", [VSC, 128, 8, CH], f8,
                            kind="ExternalInput")
    # fc_b shard as the rhs of a 5th DoubleRow pair: chunk ch's bias lives
    # on partition ch (pad cols carry NEG so their exp vanishes)
    biasT_d = nc.dram_tensor("biasT", [128, 2, CH], f8,
                             kind="ExternalInput")
    # lhsT of the bias pair for chunk ch: 1.0 at [p=ch, i=0, :], else 0
    e8_d = nc.dram_tensor("e8", [128, 2, VSC, 128], f8,
                          kind="ExternalInput")

    # this core's vocab slice for ALL rows: [src_core, row, vs]
    out_d = nc.dram_tensor("out", [NC_, R, VS], f16, kind="ExternalOutput")

    with tile.TileContext(nc) as tc, ExitStack() as top:
        constp = top.enter_context(tc.tile_pool(name="const", bufs=1))
        ident = constp.tile([128, 128], f16)
        make_identity(nc, ident[:])
        h0T_sb = constp.tile([128, 2, 2, 4, BPC], f16)
        nc.sync.dma_start(h0T_sb[:], h0T[:])
        # b_hh n-part is constant over (t, b): one tiny tile per layer
        # feeds every step's pn injection (no ring traffic at all)
        bnT = constp.tile([128, 2, 2, 4, BPC], f16, tag="bnT")
        nc.sync.dma_start(bnT[:, 0], bHhnB0[:, :, :, 0, :])
        nc.sync.dma_start(bnT[:, 1], bHhnB1[:, :, :, 0, :])

        # resident logits operands (fcw8 itself loads post-scan into the
        # space the scan frees); gathered fp8 scan outputs
        gathp = top.enter_context(tc.tile_pool(name="gath", bufs=1))
        biasT = gathp.tile([128, 2, CH], f8, tag="biasT")
        e8 = gathp.tile([128, 2, VSC, 128], f8, tag="e8")
        # first two fc_w chunks load during the scan so the first logits
        # blocks start immediately; the rest load into freed scan space
        fcw8a = gathp.tile([128, 2, 8, CH], f8, tag="fcw8a")
        # all cores' scan outputs: [p, h-tile, src_core, local row (4t+b)],
        # one tile per quarter so block reads only dep on their own gather
        QR = R // 4
        x2all = [gathp.tile([128, 8, NC_, QR], f8, name=f"x2all{q}")
                 for q in range(4)]
        # DRAM bounce buffers for the x2 all-gathers + sum all-reduces
        dramp = top.enter_context(tc.tile_pool(name="dram", bufs=1,
                                               space="DRAM"))
        x2qi = [dramp.tile([128, 8, QR], f8, name=f"x2qi{q}")
                for q in range(4)]
        x2qo = [dramp.tile([NC_, 128, 8, QR], f8, name=f"x2qo{q}")
                for q in range(4)]
        ari = [dramp.tile([128, 8], f32, name=f"ari{g}") for g in range(6)]
        aro = [dramp.tile([NC_, 128, 8], f32, name=f"aro{g}")
               for g in range(6)]

        with ExitStack() as scan_stack:
            wres = scan_stack.enter_context(tc.tile_pool(name="wres", bufs=1))
            gip = scan_stack.enter_context(tc.tile_pool(name="gip", bufs=1))
            hist1 = scan_stack.enter_context(tc.tile_pool(name="hist1",
                                                          bufs=1))
            # histories are 32-step rings (slot = t % RW): the recurrence
            # only needs t-1, gi1 builds lag <= 12 slots, and x2 is cast to
            # fp8 + staged to DRAM the moment each 32-step quarter finishes
            RW = 32
            x2T = hist1.tile([128, 2, 4, RW, BPC], f16, tag="x2T")
            x2f8 = hist1.tile([128, 2, 4, RW, BPC], f8, tag="x2f8")

            # One SHARED gi ring for both layers (32-step window, slot =
            # t % RW): L0 consumes slot t at slot t, and gi1's chunk for
            # steps [t0, t0+GCH) is only written after L0's reads of those
            # slots — so layer 1's gi overwrites layer 0's in place, and
            # the ring reuses each slot every 32 steps with ~9 slots of
            # margin. Both gi builds run chunked inside the slot loop.
            giRZB = gip.tile([128, 2, 8, 32, BPC], f16, tag="giRZB")
            giN = gip.tile([128, 2, 4, 32, BPC], f16, tag="giN")
            giR = [giRZB, giRZB]
            giNl = [giN, giN]
            x1T = hist1.tile([128, 2, 4, RW, BPC], f16, tag="x1T")
            hists = [x1T, x2T]

            # input DMAs ordered by first use: gi chunks 0/1 (emb, wih0,
            # bias), L0 step 0 (b_hh0, whh0), then gi1/L1 weights
            embT_sb = wres.tile([128, 4, R], f16, tag="embT")
            nc.sync.dma_start(embT_sb[:], embT[:])
            bgiB = wres.tile([128, 2, 2, 12, GC_NR], f16, tag="bgiB")
            nc.sync.dma_start(bgiB[:], bGiB[:])
            wih0 = wres.tile([128, 4, 2, G], f16, tag="wih0")
            for d in range(2):
                nc.sync.dma_start(wih0[:, :, d, :], WihT0[:, :, d, :])
            whh0 = wres.tile([128, 4, 2, G], f16, tag="whh0")
            # split by gate range so step 0's r-group matmuls start as
            # soon as the first third lands
            for j3 in range(3):
                nc.sync.dma_start(whh0[:, :, :, 512 * j3:512 * (j3 + 1)],
                                  WhhT0[:, :, :, 512 * j3:512 * (j3 + 1)])
            wih1 = wres.tile([128, 8, 2, G], f16, tag="wih1")
            for d in range(2):
                nc.sync.dma_start(wih1[:, :, d, :], WihT1[:, :, d, :])
            whh1 = wres.tile([128, 4, 2, G], f16, tag="whh1")
            nc.sync.dma_start(whh1[:], WhhT1[:])
            whhs = [whh0, whh1]
            # resident logits operands ride the idle DMA behind the scan's
            # critical loads (the gi ring freed the SBUF fcw8 needs)
            nc.sync.dma_start(biasT[:], biasT_d[:])
            nc.sync.dma_start(e8[:], e8_d[:])
            for ch in range(2):
                nc.sync.dma_start(fcw8a[:, ch], fcw8_d[ch])

            # ---------- pipelined two-layer scan ----------
            # LAG >= 8 is a hard correctness bound: gi1's chunk for steps
            # [8c, 8c+8) is emitted at slot 8c+8, and layer 1's step 8c is
            # emitted at slot 8c+LAG.  With LAG < 8 the consumer is emitted
            # BEFORE the gi1 write exists, so the dependency tracker lets
            # layer 1 read layer 0's gi values from the shared ring.
            LAG = 10
            with (
                tc.tile_pool(name="spr", bufs=2, space="PSUM") as przp,
                tc.tile_pool(name="spn", bufs=2, space="PSUM") as pnp,
                tc.tile_pool(name="spz", bufs=2, space="PSUM") as pzp,
                tc.tile_pool(name="gcp", bufs=2, space="PSUM") as gcp,
                tc.tile_pool(name="sch", bufs=8) as chp,
            ):
                def mk_step(l, t):
                    # split one step's emission into phases so the two
                    # layers' chains can be phase-shifted half a slot: the
                    # in-order ACT/DVE queues then serve L0's and L1's ops
                    # in the order they actually become ready
                    whh = whhs[l]
                    histT = hists[l]
                    gR = giR[l]
                    st = {}

                    def rhs(d, k):
                        if t == 0:
                            return h0T_sb[:, l, d, k, :]
                        return histT[:, d, k, (t - 1) % RW, :]

                    def gate_group(pool, jlo, jhi, nm):
                        ps = pool.tile([128, 2, jhi - jlo, BPC], f32,
                                       tag=nm, name=f"{nm}{l}")
                        for d in range(2):
                            inj = (bnT[:, l, d] if jlo == 8
                                   else gR[:, d, jlo:jhi, t % RW, :])
                            nc.tensor.matmul(
                                ps[:, d, :, :], ident[:], inj,
                                start=(d == 0), stop=False)
                            for j in range(jlo, jhi):
                                for k in range(4):
                                    nc.tensor.matmul(
                                        ps[:, d, j - jlo, :],
                                        whh[:, k, d, 128 * j:128 * (j + 1)],
                                        rhs(d, k), start=False,
                                        stop=(d == 1 and j == jhi - 1
                                              and k == 3))
                        return ps

                    def pe():
                        st["pr"] = gate_group(przp, 0, 4, "pr")
                        st["pn"] = gate_group(pnp, 8, 12, "pn")
                        st["pz"] = gate_group(pzp, 4, 8, "pz")

                    def sig_r():
                        st["rp"] = chp.tile([128, 2, 3, 4, BPC], f32,
                                            tag="rp", name=f"rp{l}")
                        nc.scalar.activation(st["rp"][:, :, 0], st["pr"][:],
                                             AF.Sigmoid)

                    def n1_muladd():
                        st["n1"] = chp.tile([128, 2, 4, BPC], f32,
                                            tag="n1", name=f"n1{l}")
                        nc.vector.tensor_mul(st["n1"][:], st["pn"][:],
                                             st["rp"][:, :, 0])
                        nc.vector.tensor_add(st["n1"][:], st["n1"][:],
                                             giNl[l][:, :, :, t % RW, :])

                    def sig_z():
                        nc.scalar.activation(st["rp"][:, :, 1], st["pz"][:],
                                             AF.Sigmoid)

                    def tanh():
                        nc.scalar.activation(st["rp"][:, :, 2], st["n1"][:],
                                             AF.Tanh)

                    def ozm1():
                        st["m1"] = chp.tile([128, 2, 4, BPC], f32,
                                            tag="m1", name=f"m1{l}")
                        hprev = (h0T_sb[:, l] if t == 0
                                 else histT[:, :, :, (t - 1) % RW, :])
                        nc.vector.tensor_mul(st["m1"][:],
                                             st["rp"][:, :, 1], hprev)

                    def tail():
                        # m2' = (z - 1)*n = -(1-z)*n in ONE fused op, then
                        # h' = m1 - m2' -- the separate oz=1-z op is gone
                        m2 = chp.tile([128, 2, 4, BPC], f32, tag="m2",
                                      name=f"m2{l}")
                        nc.vector.scalar_tensor_tensor(
                            m2[:], st["rp"][:, :, 1], 1.0, st["rp"][:, :, 2],
                            OP.subtract, OP.mult)
                        nc.vector.tensor_sub(histT[:, :, :, t % RW, :],
                                             st["m1"][:], m2[:])
                    return pe, sig_r, n1_muladd, sig_z, tanh, ozm1, tail

                def emit_slot_steps(steps):
                    ph = {l: mk_step(l, t) for l, t in steps}
                    if 0 in ph and 1 in ph:
                        pe0, sr0, nm0, sz0, th0, oz0, tl0 = ph[0]
                        pe1, sr1, nm1, sz1, th1, oz1, tl1 = ph[1]
                        pe0(); sr0(); pe1()
                        nm0(); sz0(); sr1()
                        th0(); oz0(); nm1()
                        sz1(); tl0(); th1()
                        oz1(); tl1()
                    else:
                        for l in ph:
                            pe, sr, nm, sz, th, oz, tl = ph[l]
                            pe(); sr(); nm(); sz(); th(); oz(); tl()

                def gi_chunk(l, c):
                    t0, t1 = GCH * c, GCH * (c + 1)
                    nr = GC_NR
                    w0 = t0 % RW
                    wih, kc = (wih0, 4) if l == 0 else (wih1, 8)
                    for d in range(2):
                        for grp, (j0, nj) in enumerate([(0, 8), (8, 4)]):
                            ps = gcp.tile([128, nj, nr], f32, tag="gc",
                                          name=f"gc{grp}")
                            # bias seeds the whole group (zeroes the bank)
                            nc.tensor.matmul(
                                ps[:], ident[:],
                                bgiB[:, l, d, j0:j0 + nj, :],
                                start=True, stop=False)
                            for jj in range(nj):
                                j = j0 + jj
                                for kk in range(kc):
                                    if l == 0:
                                        xs = embT_sb[:, kk,
                                                     BPC * t0:BPC * t1]
                                    else:
                                        r0 = t0 % RW
                                        xs = x1T[:, kk // 4, kk % 4,
                                                 r0:r0 + GCH, :]
                                        xs = xs.rearrange("p t b -> p (t b)")
                                    nc.tensor.matmul(
                                        ps[:, jj, :],
                                        wih[:, kk, d, 128 * j:128 * (j + 1)],
                                        xs, start=False,
                                        stop=(jj == nj - 1 and kk == kc - 1))
                            if grp == 0:
                                dst = giRZB[:, d, 0:8, w0:w0 + GCH, :]
                            else:
                                dst = giN[:, d, :, w0:w0 + GCH, :]
                            dst = dst.rearrange("p j t b -> p j (t b)")
                            if (2 * d + grp) % 2 == 0:
                                nc.scalar.copy(dst, ps[:])
                            else:
                                nc.vector.tensor_copy(dst, ps[:])

                NGC = (T + GCH - 1) // GCH
                gi_chunk(0, 0)
                gi_chunk(0, 1)
                for s in range(T + LAG):
                    steps = []
                    if s < T:
                        steps.append((0, s))
                    if s >= LAG:
                        steps.append((1, s - LAG))
                    emit_slot_steps(steps)
                    if (s + 5) % GCH == 0 and 2 <= (s + 5) // GCH < NGC:
                        gi_chunk(0, (s + 5) // GCH)
                    if s >= 8 and (s - 8) % GCH == 0 and (s - 8) // GCH < NGC:
                        gi_chunk(1, (s - 8) // GCH)
                    if s >= LAG and (s - LAG + 1) % 8 == 0:
                        # cast each finished 8-step eighth to fp8 as it
                        # completes: small queue bubbles, and the ring WAR
                        # clears ~24 slots before the slot is rewritten
                        t0 = (s - LAG + 1) - 8
                        w = t0 % RW
                        csrc = x2T[:, :, :, w:w + GCH, :]
                        cdst = x2f8[:, :, :, w:w + GCH, :]
                        if (t0 // 8) % 2 == 0:
                            nc.scalar.copy(cdst, csrc)
                        else:
                            nc.vector.tensor_copy(cdst, csrc)
                    if s >= LAG and (s - LAG + 1) % 32 == 0:
                        # a quarter of the scan output is staged to DRAM,
                        # all-gathered, and scattered into x2all
                        q = (s - LAG + 1) // 32 - 1
                        nc.sync.dma_start(
                            x2qi[q][:],
                            x2f8[:].rearrange("p d k t b -> p (d k) (t b)"))
                        nc.gpsimd.collective_compute(
                            "AllGather", OP.bypass,
                            replica_groups=[list(range(NC_))],
                            ins=[x2qi[q].opt()], outs=[x2qo[q].opt()])
                        # gpsimd queue: this DMA waits ~41us for the
                        # collective, which would head-of-line-block the
                        # scan's bias DMAs if it sat on the SP sequencer
                        nc.gpsimd.dma_start(
                            x2all[q][:],
                            x2qo[q][:].rearrange("c p h r -> p h c r"))

        # ------- logits + log_softmax (vocab-parallel over the 8 cores) ----
        # Every core computes its VS-wide vocab shard for ALL 4096 rows.
        # Blocks: (quarter i, src core c) -> 128 rows of x2all.  Per block:
        # 4 chunk-pairs of 5 DoubleRow matmuls (4 x2 pairs + bias pair),
        # exp+accum straight off PSUM, PSUM->lt copies split DVE/Pool.
        # Per quarter: one [128, 8] partial-sum AllReduce; subtract + out
        # DMA of quarter g interleaves into quarter g+1's compute.
        ngrp = 4 if nblk_lim is None else min(nblk_lim, 4)
        with (
            tc.tile_pool(name="fwp", bufs=1) as fwp,
            tc.tile_pool(name="ltp", bufs=13) as ltp,
            tc.tile_pool(name="scrp", bufs=2) as scrp,
            tc.tile_pool(name="accp", bufs=16) as accp,
            tc.tile_pool(name="arp", bufs=2) as arp,
            tc.tile_pool(name="pcp", bufs=4) as pcp,
            tc.tile_pool(name="lpsp", bufs=4, space="PSUM") as lpsp,
        ):
            # remaining weight chunks land in freed scan space while the
            # first blocks chew on the early-loaded ch0/ch1 pairs
            fcw8b = fwp.tile([128, VSC - 2, 8, CH], f8, tag="fcw8b")
            for ch in range(2, VSC):
                nc.sync.dma_start(fcw8b[:, ch - 2], fcw8_d[ch])

            def fw_ch(ch):
                return fcw8a[:, ch] if ch < 2 else fcw8b[:, ch - 2]

            pend = []

            def emit_piece():
                if pend:
                    pend.pop(0)()

            def emit_block(q, c, arin, slot, nblk_):
                lt = ltp.tile([128, VSC, CH], f16, tag="lt")
                for kp in range(4):
                    ps = lpsp.tile([128, 2, CH], f32, tag="lp")
                    for j in range(2):
                        fw = fw_ch(2 * kp + j)
                        for hp in range(4):
                            lhsT = x2all[q][:, 2 * hp:2 * hp + 2, c, :]
                            nc.tensor.matmul(
                                ps[:, j, :], lhsT,
                                fw[:, 2 * hp:2 * hp + 2, :],
                                start=(hp == 0), stop=False, perf_mode=DR)
                        nc.tensor.matmul(
                            ps[:, j, :], e8[:, :, 2 * kp + j, :],
                            biasT[:], start=False, stop=True, perf_mode=DR)
                    # copy l out of PSUM (GPSIMD cannot touch PSUM, so the
                    # copies split DVE/ACT); the PSUM WAR clears on the
                    # copy alone -- exp reads the SBUF copy later
                    dst = lt[:, 2 * kp:2 * kp + 2, :]
                    nc.vector.tensor_copy(dst, ps[:])
                # one whole-block exp off SBUF sums straight into arin
                scr = scrp.tile([128, VSC, CH], f16, tag="scr")
                nc.scalar.activation(scr[:], lt[:], AF.Exp,
                                     accum_out=arin[:, slot:slot + 1])
                return lt

            def finalize(g, q, blks, lts, last=False):
                # partial sums were AllGathered (15us constant beats the
                # AllReduce's 28us): sum the 8 cores' partials locally
                arb = arp.tile([128, 8, NC_], f32, tag="arb")
                nc.sync.dma_start(arb[:], aro[g][:].rearrange(
                    "c p b -> p b c"))
                ssum = arp.tile([128, 8], f32, tag="ssum")
                nc.vector.tensor_reduce(ssum[:], arb[:], axis=AX.X,
                                        op=OP.add)
                lnS = arp.tile([128, 8], f32, tag="lnS")
                nc.scalar.activation(lnS[:], ssum[:], AF.Ln)

                def piece(i, c, kp, q=q, lts=lts, lnS=lnS, last=last):
                    w = min(2 * CH, VS - 2 * CH * kp)
                    pc = pcp.tile([128, 2, CH], f16, tag="pc")
                    src = lts[i][:, 2 * kp:2 * kp + 2, :]
                    # Pool's subs are 3.6x DVE's: fine mid-phase (DVE is
                    # busy with copies), but the final drain runs on an
                    # otherwise-idle DVE
                    if kp % 2 == 0 and not last:
                        nc.gpsimd.tensor_scalar_sub(pc[:], src,
                                                    lnS[:, i:i + 1])
                    else:
                        nc.vector.tensor_scalar_sub(pc[:], src,
                                                    lnS[:, i:i + 1])
                    nc.sync.dma_start(
                        out_d[c, 128 * q:128 * (q + 1),
                              2 * CH * kp:2 * CH * kp + w],
                        pc[:].rearrange("p a b -> p (a b)")[:, 0:w])
                return [lambda i=i, c=c, kp=kp: piece(i, c, kp)
                        for i, c in enumerate(blks) for kp in range(4)]

            # groups of (quarter, src-core) blocks; the first is small so
            # the sum-exchange pipeline warms early, and the last is small
            # so the serial tail (exchange + drain) after the final blocks
            # is short
            groups = [(0, list(range(8))), (1, list(range(8))),
                      (2, list(range(8))), (3, [0, 1, 2, 3, 4]),
                      (3, [5, 6, 7])][:ngrp if ngrp < 4 else 5]
            nblk_ = 0
            for g, (q, blks) in enumerate(groups):
                arin = arp.tile([128, 8], f32, tag="arin", name=f"ari{g}")
                lts = []
                for slot, c in enumerate(blks):
                    lts.append(emit_block(q, c, arin, slot, nblk_))
                    nblk_ += 1
                    for _ in range(4):
                        emit_piece()
                if len(blks) < 8:
                    nc.vector.memset(arin[:, len(blks):], 0.0)
                nc.sync.dma_start(ari[g][:], arin[:])
                nc.gpsimd.collective_compute(
                    "AllGather", OP.bypass,
                    replica_groups=[list(range(NC_))],
                    ins=[ari[g].opt()], outs=[aro[g].opt()])
                pend.extend(finalize(g, q, blks, lts,
                                     last=(g >= len(groups) - 2)))
            while pend:
                emit_piece()

    nc.compile()
    return nc


def _get_nc():
    if "nc" not in _BUILT:
        _BUILT["nc"] = _build_nc()
    return _BUILT["nc"]


def _prep_inputs(inputs):
    """Host-side shard + relayout. Returns in_maps for 8 cores."""
    f16 = np.float16

    tgt = np.asarray(inputs["target"])
    ctx = np.asarray(inputs["context"], np.float32)
    emb_t = np.asarray(inputs["embed_table"], np.float32)
    fc_w = np.asarray(inputs["fc_w"], np.float32)
    fc_b = np.asarray(inputs["fc_b"], np.float32)
    w_ih0 = np.asarray(inputs["w_ih0"], np.float32)
    w_hh0 = np.asarray(inputs["w_hh0"], np.float32)
    w_ih1 = np.asarray(inputs["w_ih1"], np.float32)
    w_hh1 = np.asarray(inputs["w_hh1"], np.float32)
    b_ih0 = np.asarray(inputs["b_ih0"], np.float32)
    b_hh0 = np.asarray(inputs["b_hh0"], np.float32)
    b_ih1 = np.asarray(inputs["b_ih1"], np.float32)
    b_hh1 = np.asarray(inputs["b_hh1"], np.float32)

    def wT(w, kc):     # [2, G, I] -> [128, kc, 2, G]
        return np.ascontiguousarray(
            w.transpose(2, 0, 1).reshape(kc, 128, 2, G).transpose(1, 0, 2, 3)
        ).astype(f16)

    gmask_rz = (np.arange(G) < 2 * H)

    def bgi(b_ih, b_hh):   # [128, 2, 12, GC_NR]
        v = b_ih + np.where(gmask_rz[None, :], b_hh, 0.0)
        v = v.reshape(2, 12, 128).transpose(2, 0, 1)
        return np.ascontiguousarray(np.broadcast_to(
            v[:, :, :, None], (128, 2, 12, GC_NR))).astype(f16)

    def bhhnB(b_hh):   # [128, 2, 4, S, BPC] broadcast of the n-part
        bn = b_hh[:, 2 * H:].reshape(2, 4, 128).transpose(2, 0, 1)
        return np.ascontiguousarray(np.broadcast_to(
            bn[:, :, :, None, None], (128, 2, 4, S, BPC))).astype(f16)

    import ml_dtypes
    f8 = ml_dtypes.float8_e4m3
    # fc_w rows padded so shard 7's 4096-col window stays in-bounds
    fcw_pad = np.zeros((NC_ * VS + VSC * CH, 2 * H), np.float32)
    fcw_pad[:V] = fc_w

    # bias-pair lhsT: chunk ch's selector is 1.0 at [p=ch, i=0, :]
    e8 = np.zeros((128, 2, VSC, 128), np.float32)
    for ch in range(VSC):
        e8[ch, 0, ch, :] = 1.0

    bGiB = np.ascontiguousarray(np.stack(
        [bgi(b_ih0, b_hh0), bgi(b_ih1, b_hh1)], axis=1))

    shared = {
        "WihT0": wT(w_ih0, 4), "WhhT0": wT(w_hh0, 4),
        "WihT1": wT(w_ih1, 8), "WhhT1": wT(w_hh1, 4),
        "bGiB": bGiB,
        "bHhnB0": bhhnB(b_hh0), "bHhnB1": bhhnB(b_hh1),
        "e8": e8.astype(f8),
    }

    emb = emb_t[tgt]                      # [B, S, E]
    ctx4 = ctx.reshape(L, 2, B, H)        # [l, d, b, h]

    in_maps = []
    for c in range(NC_):
        bs = slice(BPC * c, BPC * (c + 1))
        er = emb[bs].transpose(1, 0, 2).reshape(R, E)   # row = 4t + b
        embTc = np.ascontiguousarray(
            er.T.reshape(4, 128, R).transpose(1, 0, 2)).astype(f16)
        cc = ctx4[:, :, bs, :]                          # [l, d, 4, h]
        h0Tc = np.ascontiguousarray(
            cc.transpose(3, 0, 1, 2).reshape(4, 128, L, 2, BPC)
            .transpose(1, 2, 3, 0, 4)).astype(f16)
        # vocab shard c: fc_w rows [VS*c, VS*c + VSC*CH), cols >= VS are
        # pad -> NEG bias so their exp vanishes from this core's partials
        wsh = fcw_pad[VS * c:VS * c + VSC * CH]
        fcw8 = np.ascontiguousarray(
            wsh.reshape(VSC, CH, 8, 128).transpose(0, 3, 2, 1)).astype(f8)
        bsh = np.full((VSC * CH,), NEG, np.float32)
        bsh[:VS] = fc_b[VS * c:VS * (c + 1)]
        biasT = np.zeros((128, 2, CH), np.float32)
        biasT[:VSC, 0, :] = bsh.reshape(VSC, CH)
        m = {"embT": embTc, "h0T": h0Tc,
             "fcw8": fcw8, "biasT": biasT.astype(f8)}
        m.update(shared)
        in_maps.append(m)
    return in_maps


def _unshard(results):
    full = np.empty((B, S, V), np.float32)
    for k in range(NC_):
        o = np.asarray(results[k]["out"], np.float32)   # [src, 4t+b, vv]
        o = o.reshape(NC_, S, BPC, VS).transpose(0, 2, 1, 3)
        full[:, :, VS * k:VS * (k + 1)] = o.reshape(B, S, VS)
    return full


def kernel(**inputs):
    from concourse.bass_utils import run_bass_kernel_spmd
    nc = _get_nc()
    in_maps = _prep_inputs(inputs)
    res = run_bass_kernel_spmd(nc, in_maps, core_ids=list(range(NC_)))
    return _unshard(res.results)

